# revision 15
# baseline (speedup 1.0000x reference)
"""Trainium2 Bass kernel for nn_DeformableConvLayer.

Math (validated vs reference in numpy):
  xf   = sum_c w_icfd[c] * x[:, c] + b_icfd                       (B,H,W)
  mean = mean(xf, (h,w));  dy/dx = mean*w_off + b_off             (per b, 1600 stencils)
  The whole translate+fuse stage is a dense 19x19 conv with a data-dependent
  per-b kernel K_b[ky,kx] = sum_s w_fus[g_s]*hat(dy_s-ky)*hat(dx_s-kx),
  hat(t) = max(0, 1-|t|)  (bilinear weights == hat at integer taps).
  inp  = conv2d(xf, K_b, zero-pad) + 64*b_fus + xf
  y    = conv2d(inp, w_conv 3x3, zero-pad) + b_conv               (B,64,H,W)

Sharding: data-parallel, one batch element per NeuronCore (B=8, 8 cores).
Wide data paths run in bf16: x is cast to bf16 during the SWDGE load DMA,
all conv matmuls are bf16 (fp32 PSUM accumulation), and y is stored as bf16
and widened to fp32 on the host.  Stage-1 runs as Toeplitz-banded matmuls on
the tensor engine; the banded lhsT tables are materialized from K_b via
row-reversed staircase reads of a padded DRAM buffer (contiguous 4.8KB
descriptors).  xf stays on-chip: stage-0 PSUM is evacuated (bias fused) to a
staging tile and scattered to the padded xf tile by SBUF->SBUF DMA.
"""
import numpy as np
import ml_dtypes

import concourse.bacc as bacc
import concourse.bass as bass
import concourse.tile as tile
from concourse import mybir
from concourse.bass import ds, ts

F32 = mybir.dt.float32
BF16 = mybir.dt.bfloat16
NPBF = np.dtype(ml_dtypes.bfloat16)

B, C, H, W = 8, 64, 256, 256
G, DFC = 25, 64
R = 9
NT = 2 * R + 1            # 19 taps
C0 = 145                  # reversed-K row anchor in K_dram
HW = H * W
XHW = 274                 # xf_pad per-half width: 9 | 256 | 9


def _consts(params):
    """Host-side constant tensors derived from the (small) param inputs."""
    w_icfd = params["w_icfd"].astype(np.float32)
    w_off = params["w_off"].astype(np.float32)
    b_off = params["b_off"].astype(np.float32)
    w_fus = params["w_fus"].astype(np.float32)
    b_fus = float(params["b_fus"])
    w_conv = params["w_conv"].astype(np.float32)
    b_conv = params["b_conv"].astype(np.float32)

    W0 = np.zeros((128, 2), np.float32)
    for half in range(2):
        W0[half * 64:(half + 1) * 64, half] = w_icfd

    W2 = np.zeros((18, 128), np.float32)
    for g in range(2):
        for ky2 in range(3):
            for kx2 in range(3):
                W2[g * 9 + ky2 * 3 + kx2, g * 64:(g + 1) * 64] = w_conv[:, 0, ky2, kx2]
    BC = np.zeros((128, 1), np.float32)
    BC[0:64, 0] = b_conv
    BC[64:128, 0] = b_conv

    taps_rev = (R - np.arange(NT)).astype(np.float32)     # [9, 8, ..., -9]
    taps_fwd = (np.arange(NT) - R).astype(np.float32)     # [-9, ..., 9]
    TAPSF = np.tile(taps_fwd[None, :], (128, 1))
    TAPSR = np.tile(taps_rev[None, :], (128, 1))

    # s-chunk layout: s = c*128 + p, 13 chunks; tail (s>=1600) padded with zeros
    WF = np.zeros((128, 13), np.float32)
    WOFF = np.zeros((128, 26), np.float32)    # cols 0..12 y, 13..25 x
    BOFF = np.zeros((128, 26), np.float32)
    for c in range(13):
        for p in range(128):
            s = c * 128 + p
            if s < 1600:
                WF[p, c] = w_fus[s // 64]
                WOFF[p, c] = w_off[2 * s]
                BOFF[p, c] = b_off[2 * s]
                WOFF[p, 13 + c] = w_off[2 * s + 1]
                BOFF[p, 13 + c] = b_off[2 * s + 1]

    C_total = DFC * b_fus
    return dict(
        W0=W0.astype(NPBF), W2=W2.astype(NPBF), BC=BC,
        TAPSF=TAPSF, TAPSR=TAPSR, WF=WF, WOFF=WOFF, BOFF=BOFF,
        I128=np.eye(128, dtype=np.float32).astype(NPBF),
        ONESR=np.ones((1, 256), np.float32),
        ONESC=np.ones((128, 1), np.float32),
        ONES2=np.ones((2, 128), np.float32),
        CVEC=np.full((1, 128), C_total, np.float32),
        b_icfd=float(params["b_icfd"]),
    )


def build(params, num_devices=8):
    cs = _consts(params)
    nc = bacc.Bacc("TRN2", target_bir_lowering=False, debug=False,
                   num_devices=num_devices)
    xb = nc.dram_tensor("xb", [C, H, W], F32, kind="ExternalInput")
    y = nc.dram_tensor("y", [64, H, W], BF16, kind="ExternalOutput")
    K_dram = nc.dram_tensor("k_scr", [280, NT], BF16, kind="Internal")
    inp_dram = nc.dram_tensor("inp_scr", [260, 792], BF16, kind="Internal")

    ct = {k: nc.inline_tensor(v, name=f"c_{k}") for k, v in cs.items()
          if isinstance(v, np.ndarray)}
    b_icfd = cs["b_icfd"]

    def _graph(tc):
        with (
            tc.tile_pool(name="consts", bufs=1) as cp,
            tc.tile_pool(name="persist", bufs=1) as pp,
        ):
            # ---- load constants ----
            sb = {}
            for i, k in enumerate(("W0", "W2", "BC", "TAPSF", "TAPSR", "WF",
                                   "WOFF", "BOFF", "I128", "ONESR", "ONESC",
                                   "ONES2", "CVEC")):
                dt = BF16 if cs[k].dtype == NPBF else F32
                t = cp.tile(list(cs[k].shape), dt, tag=k, name=f"sb_{k}")
                eng = (nc.sync, nc.scalar)[i % 2]
                eng.dma_start(out=t, in_=ct[k][:, :])
                sb[k] = t
            zbf = cp.tile([128, 792], BF16, tag="zbf")
            nc.vector.memset(zbf, 0.0)
            bic2 = cp.tile([2, 1], F32, tag="bic2")
            nc.vector.memset(bic2, b_icfd)

            # ---- zero scratch DRAM (early, off critical path) ----
            nc.sync.dma_start(
                out=bass.AP(tensor=K_dram, offset=0, ap=[[NT, 128], [1, NT]]),
                in_=zbf[:, 0:NT])
            nc.sync.dma_start(
                out=bass.AP(tensor=K_dram, offset=128 * NT,
                            ap=[[NT, 128], [1, NT]]),
                in_=zbf[:, 0:NT])
            nc.sync.dma_start(
                out=bass.AP(tensor=K_dram, offset=256 * NT,
                            ap=[[NT, 24], [1, NT]]),
                in_=zbf[0:24, 0:NT])
            nc.scalar.dma_start(out=inp_dram[0:128, :], in_=zbf[:, 0:792])
            nc.scalar.dma_start(out=inp_dram[128:256, :], in_=zbf[:, 0:792])
            nc.scalar.dma_start(out=inp_dram[256:260, :], in_=zbf[0:4, 0:792])

            # ---- persistent xf tile: [128, 274*2] bf16, halves side by side
            xf_pad = pp.tile([128, 2 * XHW], BF16, tag="xf_pad")
            nc.vector.memset(xf_pad, 0.0)

            def _heat(n, tag):
                # keep-warm matmuls into a scratch PSUM bank: fill PE idle
                # gaps so real matmuls dispatch into a warm p-state streak
                with tc.tile_pool(name=f"heat_{tag}", bufs=1,
                                  space="PSUM") as hp:
                    pdum = hp.tile([2, 512], F32, tag=f"pdum{tag}")
                    for _ in range(n):
                        nc.tensor.matmul(pdum, sb["W0"], zbf[:, 0:512],
                                         start=True, stop=True)

            _heat(10, "a")
            # ---- phase B: x cast-load + stage-0 matmul + evac + scatter ----
            NCH = 8                       # chunks of 16 rows
            with (
                tc.tile_pool(name="bpool", bufs=3) as bp,
                tc.tile_pool(name="psum0", bufs=4, space="PSUM") as p0p,
            ):
                for ch in range(NCH):
                    sbx = bp.tile([128, 4096], BF16, tag="sbx", bufs=6)
                    for half in range(2):   # partition = half*64 + c
                        srcp = bass.AP(tensor=xb,
                                       offset=(half * 128 + ch * 16) * W,
                                       ap=[[HW, 64], [1, 4096]])
                        nc.gpsimd.dma_start(out=sbx[ts(half, 64), :], in_=srcp)
                    s0b = bp.tile([2, 4096], BF16, tag="s0b", bufs=3)
                    for q in range(4):
                        p0 = p0p.tile([2, 1024], F32, tag="p0", name="p0t")
                        for j in range(2):
                            nc.tensor.matmul(
                                p0[:, ts(j, 512)],
                                sb["W0"],
                                sbx[:, ds(q * 1024 + j * 512, 512)],
                                start=True, stop=True)
                        # evac PSUM -> bf16 staging with b_icfd bias fused
                        if (ch * 4 + q) % 2 == 0:
                            nc.scalar.activation(
                                out=s0b[:, ts(q, 1024)], in_=p0,
                                func=mybir.ActivationFunctionType.Identity,
                                bias=bic2[:, 0:1], scale=1.0)
                        else:
                            nc.vector.tensor_scalar_add(
                                out=s0b[:, ts(q, 1024)], in0=p0,
                                scalar1=bic2[:, 0:1])
                    for half in range(2):   # scatter 16 rows into xf_pad
                        nc.sync.dma_start(
                            out=xf_pad[ch * 16:ch * 16 + 16,
                                       ds(half * XHW + R, 256)],
                            in_=s0b[half:half + 1, :])

            # ---- phase C: column sums -> mean ----
            colsums = pp.tile([128, 1], F32, tag="colsums")
            nc.vector.tensor_reduce(out=colsums,
                                    in_=xf_pad[:, 0:2 * XHW],
                                    axis=mybir.AxisListType.X,
                                    op=mybir.AluOpType.add)
            with tc.tile_pool(name="psA", bufs=1, space="PSUM") as psA:
                pm = psA.tile([1, 1], F32, tag="pm")
                nc.tensor.matmul(pm, colsums, sb["ONESC"], start=True, stop=True)
                ts2 = pp.tile([1, 1], F32, tag="ts2")
                nc.scalar.copy(out=ts2, in_=pm)
                pmb = psA.tile([128, 1], F32, tag="pmb")
                nc.tensor.matmul(pmb, sb["ONES2"][0:1, :], ts2,
                                 start=True, stop=True)
                mean_bc = pp.tile([128, 1], F32, tag="mean_bc")
                nc.scalar.activation(out=mean_bc, in_=pmb,
                                     func=mybir.ActivationFunctionType.Copy,
                                     scale=1.0 / HW)

                # ---- phase D: offsets, hats, K matmul ----
                dyx = pp.tile([128, 26], F32, tag="dyx")
                nc.vector.tensor_scalar_mul(out=dyx, in0=sb["WOFF"],
                                            scalar1=mean_bc[:, 0:1])
                nc.vector.tensor_add(out=dyx, in0=dyx, in1=sb["BOFF"])
                HH = pp.tile([128, 26 * NT], F32, tag="HH")
                HH3 = HH[:].rearrange("p (a b) -> p a b", a=26)
                nc.vector.tensor_tensor(
                    out=HH3[:, 0:13, :],
                    in0=dyx[:, 0:13].unsqueeze(2).to_broadcast([128, 13, NT]),
                    in1=sb["TAPSF"][:].unsqueeze(1).to_broadcast([128, 13, NT]),
                    op=mybir.AluOpType.subtract)
                nc.vector.tensor_tensor(
                    out=HH3[:, 13:26, :],
                    in0=dyx[:, 13:26].unsqueeze(2).to_broadcast([128, 13, NT]),
                    in1=sb["TAPSR"][:].unsqueeze(1).to_broadcast([128, 13, NT]),
                    op=mybir.AluOpType.subtract)
                nc.vector.tensor_scalar(out=HH, in0=HH, scalar1=0.0,
                                        scalar2=None,
                                        op0=mybir.AluOpType.abs_max)
                nc.scalar.activation(out=HH, in_=HH,
                                     func=mybir.ActivationFunctionType.Relu,
                                     scale=-1.0, bias=1.0)
                WHY = pp.tile([128, 13 * NT], F32, tag="WHY")
                nc.vector.tensor_tensor(
                    out=WHY[:].rearrange("p (a b) -> p a b", a=13),
                    in0=HH3[:, 0:13, :],
                    in1=sb["WF"][:].unsqueeze(2).to_broadcast([128, 13, NT]),
                    op=mybir.AluOpType.mult)
                WHY3 = WHY[:].rearrange("p (a b) -> p a b", a=13)
                pK = psA.tile([NT, NT], F32, tag="pK")
                for c in range(13):
                    nc.tensor.matmul(pK, WHY3[:, c, :], HH3[:, 13 + c, :],
                                     start=(c == 0), stop=(c == 12))
                Ksb = pp.tile([NT, NT], BF16, tag="Ksb")
                nc.scalar.copy(out=Ksb, in_=pK)

            # ---- phase E: reversed K write + contiguous staircase T tables
            # K_dram[C0 - d] = Ksb[d]  (bf16 cast during SWDGE write)
            nc.sync.dma_start(
                out=bass.AP(tensor=K_dram, offset=C0 * NT,
                            ap=[[-NT, NT], [1, NT]]),
                in_=Ksb)
            T_A = pp.tile([128, 128 * NT], BF16, tag="T_A")
            T_B = pp.tile([9, 128 * NT], BF16, tag="T_B")
            T_C = pp.tile([9, 128 * NT], BF16, tag="T_C")
            # T_A[p, a, e] = K[9+p-a] = K_dram[C0-9-p+a]
            nc.sync.dma_start(
                out=T_A[:].rearrange("p (a b) -> p a b", a=128),
                in_=bass.AP(tensor=K_dram, offset=(C0 - 9) * NT,
                            ap=[[-NT, 128], [NT, 128], [1, NT]]))
            # T_B[p, a, e] = K[p-a] = K_dram[C0-p+a]
            nc.gpsimd.dma_start(
                out=T_B[:].rearrange("p (a b) -> p a b", a=128),
                in_=bass.AP(tensor=K_dram, offset=C0 * NT,
                            ap=[[-NT, 9], [NT, 128], [1, NT]]))
            # T_C[p, a, e] = K[137+p-a] = K_dram[C0-137-p+a]
            nc.gpsimd.dma_start(
                out=T_C[:].rearrange("p (a b) -> p a b", a=128),
                in_=bass.AP(tensor=K_dram, offset=(C0 - 137) * NT,
                            ap=[[-NT, 9], [NT, 128], [1, NT]]))
            T_A3 = T_A[:].rearrange("p (a b) -> p a b", a=128)
            T_B3 = T_B[:].rearrange("p (a b) -> p a b", a=128)
            T_C3 = T_C[:].rearrange("p (a b) -> p a b", a=128)
            # matmul operands must start at partition 0/32/64: copy the 9
            # boundary rows of half 0 (119..127) into a base-0 tile
            xf_b0 = pp.tile([9, XHW], BF16, tag="xf_b0")
            nc.sync.dma_start(out=xf_b0, in_=xf_pad[119:128, 0:XHW])

            _heat(26, "b")
            # ---- phase F: stage-1 Toeplitz matmuls -> inp_dram ----
            with tc.tile_pool(name="psum1", bufs=2, space="PSUM") as p1p:
                for t in range(2):
                    pinp = p1p.tile([128, W], F32, tag="pinp")
                    nmm = NT * 2 + 2
                    i = 0
                    for kxp in range(NT):
                        sl = 18 - kxp
                        nc.tensor.matmul(pinp, T_A3[:, :, kxp],
                                         xf_pad[:, ds(t * XHW + sl, W)],
                                         start=(i == 0), stop=(i == nmm - 1)); i += 1
                        if t == 0:
                            nc.tensor.matmul(pinp, T_C3[0:9, :, kxp],
                                             xf_pad[0:9, ds(XHW + sl, W)],
                                             start=False, stop=(i == nmm - 1)); i += 1
                        else:
                            nc.tensor.matmul(pinp, T_B3[0:9, :, kxp],
                                             xf_b0[:, ds(sl, W)],
                                             start=False, stop=(i == nmm - 1)); i += 1
                    nc.tensor.matmul(pinp, sb["I128"],
                                     xf_pad[:, ds(t * XHW + R, W)],
                                     start=False, stop=False); i += 1
                    nc.tensor.matmul(pinp, sb["CVEC"], sb["ONESR"][0:1, 0:W],
                                     start=False, stop=True); i += 1
                    s1 = pp.tile([128, W], BF16, tag=f"s1_{t}", name=f"s1stage{t}")
                    nc.vector.tensor_copy(out=s1, in_=pinp)
                    # inp3[r, j, e] = inp_padded[r, j + e]; s1 covers padded
                    # cols 1..256 of rows 1+128t..128+128t
                    for j in range(3):
                        dst = bass.AP(
                            tensor=inp_dram,
                            offset=(1 + 128 * t) * 792 + j * 264 + (1 - j),
                            ap=[[792, 128], [1, W]])
                        eng = (nc.sync, nc.gpsimd)[t]
                        eng.dma_start(out=dst, in_=s1)

            _heat(30, "c")
            # ---- phase G: im2col + stage-2 + store ----
            with (
                tc.tile_pool(name="gpool", bufs=3) as gp,
                tc.tile_pool(name="psum2", bufs=4, space="PSUM") as p2p,
            ):
                for ch in range(8):               # h2-chunks of 16
                    im = gp.tile([18, 4096], BF16, tag="im", bufs=4)
                    for g in range(2):
                        srcp = bass.AP(
                            tensor=inp_dram,
                            offset=(g * 128 + ch * 16) * 792,
                            ap=[[264, 9], [792, 16], [1, W]])
                        nc.sync.dma_start(
                            out=im[ts(g, 9), :].rearrange(
                                "p (d e) -> p d e", d=16),
                            in_=srcp)
                    for pair in range(2):         # 8 rows per store
                        ysb = gp.tile([128, 2048], BF16, tag="ysb",
                                      name="ystage", bufs=3)
                        py = p2p.tile([128, 2048], F32, tag="py", bufs=2)
                        for j in range(4):
                            nc.tensor.matmul(
                                py[:, ts(j, 512)], sb["W2"],
                                im[:, ds(pair * 2048 + j * 512, 512)],
                                start=True, stop=True)
                        # single evac per store unit (+ b_conv bias, bf16)
                        u = ch * 2 + pair
                        if (u + u // 2) % 2 == 0:
                            nc.scalar.activation(
                                out=ysb, in_=py,
                                func=mybir.ActivationFunctionType.Identity,
                                bias=sb["BC"][:, 0:1], scale=1.0)
                        else:
                            nc.vector.tensor_scalar_add(
                                out=ysb, in0=py, scalar1=sb["BC"][:, 0:1])
                        dst = bass.AP(
                            tensor=y,
                            offset=(ch * 16 + pair * 8) * W,
                            ap=[[128 * W, 2], [HW, 64], [1, 2048]])
                        seng = (nc.gpsimd, nc.sync)[pair]
                        seng.dma_start(out=dst, in_=ysb[:])
    with tile.TileContext(nc) as tc:
        _graph(tc)
    nc.finalize()
    return nc


def kernel(**inputs):
    x = np.ascontiguousarray(inputs["x"], dtype=np.float32)
    params = {k: np.asarray(v) for k, v in inputs.items() if k != "x"}
    nc = build(params, num_devices=8)
    from concourse.bass_utils import run_bass_kernel_spmd
    in_maps = [{"xb": np.ascontiguousarray(x[b])} for b in range(B)]
    res = run_bass_kernel_spmd(nc, in_maps, core_ids=list(range(B)))
    return np.stack([np.asarray(res.results[b]["y"], dtype=np.float32)
                     for b in range(B)])


# revision 16
# speedup vs baseline: 1.0881x; 1.0881x over previous
"""Trainium2 Bass kernel for nn_DeformableConvLayer.

Math (validated vs reference in numpy):
  xf   = sum_c w_icfd[c] * x[:, c] + b_icfd                       (B,H,W)
  mean = mean(xf, (h,w));  dy/dx = mean*w_off + b_off             (per b, 1600 stencils)
  The whole translate+fuse stage is a dense 19x19 conv with a data-dependent
  per-b kernel K_b[ky,kx] = sum_s w_fus[g_s]*hat(dy_s-ky)*hat(dx_s-kx),
  hat(t) = max(0, 1-|t|)  (bilinear weights == hat at integer taps).
  inp  = conv2d(xf, K_b, zero-pad) + 64*b_fus + xf
  y    = conv2d(inp, w_conv 3x3, zero-pad) + b_conv               (B,64,H,W)

Sharding: data-parallel, one batch element per NeuronCore (B=8, 8 cores).
Wide data paths run in bf16: x is cast to bf16 during the SWDGE load DMA,
all conv matmuls are bf16 (fp32 PSUM accumulation), and y is stored as bf16
and widened to fp32 on the host.  Stage-1 runs as Toeplitz-banded matmuls on
the tensor engine; the banded lhsT tables are materialized from K_b via
row-reversed staircase reads of a padded DRAM buffer (contiguous 4.8KB
descriptors).  xf stays on-chip: stage-0 PSUM is evacuated (bias fused) to a
staging tile and scattered to the padded xf tile by SBUF->SBUF DMA.
"""
import numpy as np
import ml_dtypes

import concourse.bacc as bacc
import concourse.bass as bass
import concourse.tile as tile
from concourse import mybir
from concourse.bass import ds, ts

F32 = mybir.dt.float32
BF16 = mybir.dt.bfloat16
NPBF = np.dtype(ml_dtypes.bfloat16)

B, C, H, W = 8, 64, 256, 256
G, DFC = 25, 64
R = 9
NT = 2 * R + 1            # 19 taps
C0 = 145                  # reversed-K row anchor in K_dram
HW = H * W
XHW = 274                 # xf_pad per-half width: 9 | 256 | 9


def _consts(params):
    """Host-side constant tensors derived from the (small) param inputs."""
    w_icfd = params["w_icfd"].astype(np.float32)
    w_off = params["w_off"].astype(np.float32)
    b_off = params["b_off"].astype(np.float32)
    w_fus = params["w_fus"].astype(np.float32)
    b_fus = float(params["b_fus"])
    w_conv = params["w_conv"].astype(np.float32)
    b_conv = params["b_conv"].astype(np.float32)

    W0 = np.zeros((128, 2), np.float32)
    for half in range(2):
        W0[half * 64:(half + 1) * 64, half] = w_icfd

    W2 = np.zeros((18, 128), np.float32)
    for g in range(2):
        for ky2 in range(3):
            for kx2 in range(3):
                W2[g * 9 + ky2 * 3 + kx2, g * 64:(g + 1) * 64] = w_conv[:, 0, ky2, kx2]
    BC = np.zeros((128, 1), np.float32)
    BC[0:64, 0] = b_conv
    BC[64:128, 0] = b_conv

    taps_rev = (R - np.arange(NT)).astype(np.float32)     # [9, 8, ..., -9]
    taps_fwd = (np.arange(NT) - R).astype(np.float32)     # [-9, ..., 9]
    TAPSF = np.tile(taps_fwd[None, :], (128, 1))
    TAPSR = np.tile(taps_rev[None, :], (128, 1))

    # s-chunk layout: s = c*128 + p, 13 chunks; tail (s>=1600) padded with zeros
    WF = np.zeros((128, 13), np.float32)
    WOFF = np.zeros((128, 26), np.float32)    # cols 0..12 y, 13..25 x
    BOFF = np.zeros((128, 26), np.float32)
    for c in range(13):
        for p in range(128):
            s = c * 128 + p
            if s < 1600:
                WF[p, c] = w_fus[s // 64]
                WOFF[p, c] = w_off[2 * s]
                BOFF[p, c] = b_off[2 * s]
                WOFF[p, 13 + c] = w_off[2 * s + 1]
                BOFF[p, 13 + c] = b_off[2 * s + 1]

    C_total = DFC * b_fus
    return dict(
        W0=W0.astype(NPBF), W2=W2.astype(NPBF), BC=BC,
        TAPSF=TAPSF, TAPSR=TAPSR, WF=WF, WOFF=WOFF, BOFF=BOFF,
        I128=np.eye(128, dtype=np.float32).astype(NPBF),
        ONESR=np.ones((1, 256), np.float32),
        ONESC=np.ones((128, 1), np.float32),
        ONES2=np.ones((2, 128), np.float32),
        CVEC=np.full((1, 128), C_total, np.float32),
        b_icfd=float(params["b_icfd"]),
    )


def build(params, num_devices=8):
    cs = _consts(params)
    nc = bacc.Bacc("TRN2", target_bir_lowering=False, debug=False,
                   num_devices=num_devices)
    xb = nc.dram_tensor("xb", [C, H, W], F32, kind="ExternalInput")
    y = nc.dram_tensor("y", [64, H, W], BF16, kind="ExternalOutput")
    K_dram = nc.dram_tensor("k_scr", [280, NT], BF16, kind="Internal")
    inp_dram = nc.dram_tensor("inp_scr", [260, 792], BF16, kind="Internal")

    ct = {k: nc.inline_tensor(v, name=f"c_{k}") for k, v in cs.items()
          if isinstance(v, np.ndarray)}
    b_icfd = cs["b_icfd"]

    def _graph(tc):
        with (
            tc.tile_pool(name="consts", bufs=1) as cp,
            tc.tile_pool(name="persist", bufs=1) as pp,
        ):
            # ---- load constants ----
            sb = {}
            for i, k in enumerate(("W0", "W2", "BC", "TAPSF", "TAPSR", "WF",
                                   "WOFF", "BOFF", "I128", "ONESR", "ONESC",
                                   "ONES2", "CVEC")):
                dt = BF16 if cs[k].dtype == NPBF else F32
                t = cp.tile(list(cs[k].shape), dt, tag=k, name=f"sb_{k}")
                eng = (nc.sync, nc.scalar)[i % 2]
                eng.dma_start(out=t, in_=ct[k][:, :])
                sb[k] = t
            zbf = cp.tile([128, 792], BF16, tag="zbf")
            nc.vector.memset(zbf, 0.0)
            bic2 = cp.tile([2, 1], F32, tag="bic2")
            nc.vector.memset(bic2, b_icfd)

            # ---- zero scratch DRAM (early, off critical path) ----
            nc.sync.dma_start(
                out=bass.AP(tensor=K_dram, offset=0, ap=[[NT, 128], [1, NT]]),
                in_=zbf[:, 0:NT])
            nc.sync.dma_start(
                out=bass.AP(tensor=K_dram, offset=128 * NT,
                            ap=[[NT, 128], [1, NT]]),
                in_=zbf[:, 0:NT])
            nc.sync.dma_start(
                out=bass.AP(tensor=K_dram, offset=256 * NT,
                            ap=[[NT, 24], [1, NT]]),
                in_=zbf[0:24, 0:NT])
            nc.scalar.dma_start(out=inp_dram[0:128, :], in_=zbf[:, 0:792])
            nc.scalar.dma_start(out=inp_dram[128:256, :], in_=zbf[:, 0:792])
            nc.scalar.dma_start(out=inp_dram[256:260, :], in_=zbf[0:4, 0:792])

            # ---- persistent xf tile: [128, 274*2] bf16, halves side by side
            xf_pad = pp.tile([128, 2 * XHW], BF16, tag="xf_pad")
            nc.vector.memset(xf_pad, 0.0)

            def _heat(n, tag):
                # keep-warm matmuls into a scratch PSUM bank: fill PE idle
                # gaps so real matmuls dispatch into a warm p-state streak
                with tc.tile_pool(name=f"heat_{tag}", bufs=1,
                                  space="PSUM") as hp:
                    pdum = hp.tile([2, 512], F32, tag=f"pdum{tag}")
                    for _ in range(n):
                        nc.tensor.matmul(pdum, sb["W0"], zbf[:, 0:512],
                                         start=True, stop=True)

            _heat(10, "a")
            # ---- phase B: x cast-load + stage-0 matmul + evac + scatter ----
            NCH = 8                       # chunks of 16 rows
            with (
                tc.tile_pool(name="bpool", bufs=3) as bp,
                tc.tile_pool(name="psum0", bufs=2, space="PSUM") as p0p,
            ):
                for ch in range(NCH):
                    sbx = bp.tile([128, 4096], BF16, tag="sbx", bufs=6)
                    for half in range(2):   # partition = half*64 + c
                        srcp = bass.AP(tensor=xb,
                                       offset=(half * 128 + ch * 16) * W,
                                       ap=[[HW, 64], [1, 4096]])
                        nc.gpsimd.dma_start(out=sbx[ts(half, 64), :], in_=srcp)
                    s0b = bp.tile([2, 4096], BF16, tag="s0b", bufs=3)
                    for q in range(2):
                        p0 = p0p.tile([2, 2048], F32, tag="p0", name="p0t")
                        for j in range(4):
                            nc.tensor.matmul(
                                p0[:, ts(j, 512)],
                                sb["W0"],
                                sbx[:, ds(q * 2048 + j * 512, 512)],
                                start=True, stop=True)
                        # evac PSUM -> bf16 staging with b_icfd bias fused
                        if (ch * 2 + q) % 2 == 0:
                            nc.scalar.activation(
                                out=s0b[:, ts(q, 2048)], in_=p0,
                                func=mybir.ActivationFunctionType.Identity,
                                bias=bic2[:, 0:1], scale=1.0)
                        else:
                            nc.vector.tensor_scalar_add(
                                out=s0b[:, ts(q, 2048)], in0=p0,
                                scalar1=bic2[:, 0:1])
                    for half in range(2):   # scatter 16 rows into xf_pad
                        nc.sync.dma_start(
                            out=xf_pad[ch * 16:ch * 16 + 16,
                                       ds(half * XHW + R, 256)],
                            in_=s0b[half:half + 1, :])

            # ---- phase C: column sums -> mean ----
            colsums = pp.tile([128, 1], F32, tag="colsums")
            nc.vector.tensor_reduce(out=colsums,
                                    in_=xf_pad[:, 0:2 * XHW],
                                    axis=mybir.AxisListType.X,
                                    op=mybir.AluOpType.add)
            with tc.tile_pool(name="psA", bufs=1, space="PSUM") as psA:
                pm = psA.tile([1, 1], F32, tag="pm")
                nc.tensor.matmul(pm, colsums, sb["ONESC"], start=True, stop=True)
                ts2 = pp.tile([1, 1], F32, tag="ts2")
                nc.scalar.copy(out=ts2, in_=pm)
                pmb = psA.tile([128, 1], F32, tag="pmb")
                nc.tensor.matmul(pmb, sb["ONES2"][0:1, :], ts2,
                                 start=True, stop=True)
                mean_bc = pp.tile([128, 1], F32, tag="mean_bc")
                nc.scalar.activation(out=mean_bc, in_=pmb,
                                     func=mybir.ActivationFunctionType.Copy,
                                     scale=1.0 / HW)

                # ---- phase D: offsets, hats, K matmul ----
                dyx = pp.tile([128, 26], F32, tag="dyx")
                nc.vector.tensor_scalar_mul(out=dyx, in0=sb["WOFF"],
                                            scalar1=mean_bc[:, 0:1])
                nc.vector.tensor_add(out=dyx, in0=dyx, in1=sb["BOFF"])
                HH = pp.tile([128, 26 * NT], F32, tag="HH")
                HH3 = HH[:].rearrange("p (a b) -> p a b", a=26)
                nc.vector.tensor_tensor(
                    out=HH3[:, 0:13, :],
                    in0=dyx[:, 0:13].unsqueeze(2).to_broadcast([128, 13, NT]),
                    in1=sb["TAPSF"][:].unsqueeze(1).to_broadcast([128, 13, NT]),
                    op=mybir.AluOpType.subtract)
                nc.vector.tensor_tensor(
                    out=HH3[:, 13:26, :],
                    in0=dyx[:, 13:26].unsqueeze(2).to_broadcast([128, 13, NT]),
                    in1=sb["TAPSR"][:].unsqueeze(1).to_broadcast([128, 13, NT]),
                    op=mybir.AluOpType.subtract)
                nc.vector.tensor_scalar(out=HH, in0=HH, scalar1=0.0,
                                        scalar2=None,
                                        op0=mybir.AluOpType.abs_max)
                nc.scalar.activation(out=HH, in_=HH,
                                     func=mybir.ActivationFunctionType.Relu,
                                     scale=-1.0, bias=1.0)
                WHY = pp.tile([128, 13 * NT], F32, tag="WHY")
                nc.vector.tensor_tensor(
                    out=WHY[:].rearrange("p (a b) -> p a b", a=13),
                    in0=HH3[:, 0:13, :],
                    in1=sb["WF"][:].unsqueeze(2).to_broadcast([128, 13, NT]),
                    op=mybir.AluOpType.mult)
                WHY3 = WHY[:].rearrange("p (a b) -> p a b", a=13)
                pK = psA.tile([NT, NT], F32, tag="pK")
                for c in range(13):
                    nc.tensor.matmul(pK, WHY3[:, c, :], HH3[:, 13 + c, :],
                                     start=(c == 0), stop=(c == 12))
                Ksb = pp.tile([NT, NT], BF16, tag="Ksb")
                nc.scalar.copy(out=Ksb, in_=pK)

            # ---- phase E: reversed K write + contiguous staircase T tables
            # K_dram[C0 - d] = Ksb[d]  (bf16 cast during SWDGE write)
            nc.sync.dma_start(
                out=bass.AP(tensor=K_dram, offset=C0 * NT,
                            ap=[[-NT, NT], [1, NT]]),
                in_=Ksb)
            T_A = pp.tile([128, 128 * NT], BF16, tag="T_A")
            T_B = pp.tile([9, 128 * NT], BF16, tag="T_B")
            T_C = pp.tile([9, 128 * NT], BF16, tag="T_C")
            # T_A[p, a, e] = K[9+p-a] = K_dram[C0-9-p+a]
            nc.sync.dma_start(
                out=T_A[:].rearrange("p (a b) -> p a b", a=128),
                in_=bass.AP(tensor=K_dram, offset=(C0 - 9) * NT,
                            ap=[[-NT, 128], [NT, 128], [1, NT]]))
            # T_B[p, a, e] = K[p-a] = K_dram[C0-p+a]
            nc.gpsimd.dma_start(
                out=T_B[:].rearrange("p (a b) -> p a b", a=128),
                in_=bass.AP(tensor=K_dram, offset=C0 * NT,
                            ap=[[-NT, 9], [NT, 128], [1, NT]]))
            # T_C[p, a, e] = K[137+p-a] = K_dram[C0-137-p+a]
            nc.gpsimd.dma_start(
                out=T_C[:].rearrange("p (a b) -> p a b", a=128),
                in_=bass.AP(tensor=K_dram, offset=(C0 - 137) * NT,
                            ap=[[-NT, 9], [NT, 128], [1, NT]]))
            T_A3 = T_A[:].rearrange("p (a b) -> p a b", a=128)
            T_B3 = T_B[:].rearrange("p (a b) -> p a b", a=128)
            T_C3 = T_C[:].rearrange("p (a b) -> p a b", a=128)
            # matmul operands must start at partition 0/32/64: copy the 9
            # boundary rows of half 0 (119..127) into a base-0 tile
            xf_b0 = pp.tile([9, XHW], BF16, tag="xf_b0")
            nc.sync.dma_start(out=xf_b0, in_=xf_pad[119:128, 0:XHW])

            _heat(26, "b")
            # ---- phase F: stage-1 Toeplitz matmuls -> inp_dram ----
            with tc.tile_pool(name="psum1", bufs=2, space="PSUM") as p1p:
                for t in range(2):
                    pinp = p1p.tile([128, W], F32, tag="pinp")
                    nmm = NT * 2 + 2
                    i = 0
                    for kxp in range(NT):
                        sl = 18 - kxp
                        nc.tensor.matmul(pinp, T_A3[:, :, kxp],
                                         xf_pad[:, ds(t * XHW + sl, W)],
                                         start=(i == 0), stop=(i == nmm - 1)); i += 1
                        if t == 0:
                            nc.tensor.matmul(pinp, T_C3[0:9, :, kxp],
                                             xf_pad[0:9, ds(XHW + sl, W)],
                                             start=False, stop=(i == nmm - 1)); i += 1
                        else:
                            nc.tensor.matmul(pinp, T_B3[0:9, :, kxp],
                                             xf_b0[:, ds(sl, W)],
                                             start=False, stop=(i == nmm - 1)); i += 1
                    nc.tensor.matmul(pinp, sb["I128"],
                                     xf_pad[:, ds(t * XHW + R, W)],
                                     start=False, stop=False); i += 1
                    nc.tensor.matmul(pinp, sb["CVEC"], sb["ONESR"][0:1, 0:W],
                                     start=False, stop=True); i += 1
                    s1 = pp.tile([128, W], BF16, tag=f"s1_{t}", name=f"s1stage{t}")
                    nc.vector.tensor_copy(out=s1, in_=pinp)
                    # inp3[r, j, e] = inp_padded[r, j + e]; s1 covers padded
                    # cols 1..256 of rows 1+128t..128+128t
                    for j in range(3):
                        dst = bass.AP(
                            tensor=inp_dram,
                            offset=(1 + 128 * t) * 792 + j * 264 + (1 - j),
                            ap=[[792, 128], [1, W]])
                        eng = (nc.sync, nc.gpsimd)[t]
                        eng.dma_start(out=dst, in_=s1)

            _heat(30, "c")
            # ---- phase G: im2col + stage-2 + store ----
            with (
                tc.tile_pool(name="gpool", bufs=3) as gp,
                tc.tile_pool(name="psum2", bufs=4, space="PSUM") as p2p,
            ):
                for ch in range(8):               # h2-chunks of 16
                    im = gp.tile([18, 4096], BF16, tag="im", bufs=4)
                    for g in range(2):
                        srcp = bass.AP(
                            tensor=inp_dram,
                            offset=(g * 128 + ch * 16) * 792,
                            ap=[[264, 9], [792, 16], [1, W]])
                        nc.sync.dma_start(
                            out=im[ts(g, 9), :].rearrange(
                                "p (d e) -> p d e", d=16),
                            in_=srcp)
                    for pair in range(2):         # 8 rows per store
                        ysb = gp.tile([128, 2048], BF16, tag="ysb",
                                      name="ystage", bufs=3)
                        py = p2p.tile([128, 2048], F32, tag="py", bufs=2)
                        for j in range(4):
                            nc.tensor.matmul(
                                py[:, ts(j, 512)], sb["W2"],
                                im[:, ds(pair * 2048 + j * 512, 512)],
                                start=True, stop=True)
                        # single evac per store unit (+ b_conv bias, bf16)
                        u = ch * 2 + pair
                        if (u + u // 2) % 2 == 0:
                            nc.scalar.activation(
                                out=ysb, in_=py,
                                func=mybir.ActivationFunctionType.Identity,
                                bias=sb["BC"][:, 0:1], scale=1.0)
                        else:
                            nc.vector.tensor_scalar_add(
                                out=ysb, in0=py, scalar1=sb["BC"][:, 0:1])
                        dst = bass.AP(
                            tensor=y,
                            offset=(ch * 16 + pair * 8) * W,
                            ap=[[128 * W, 2], [HW, 64], [1, 2048]])
                        nc.gpsimd.dma_start(out=dst, in_=ysb[:])
    with tile.TileContext(nc) as tc:
        _graph(tc)
    nc.finalize()
    return nc


def kernel(**inputs):
    x = np.ascontiguousarray(inputs["x"], dtype=np.float32)
    params = {k: np.asarray(v) for k, v in inputs.items() if k != "x"}
    nc = build(params, num_devices=8)
    from concourse.bass_utils import run_bass_kernel_spmd
    in_maps = [{"xb": np.ascontiguousarray(x[b])} for b in range(B)]
    res = run_bass_kernel_spmd(nc, in_maps, core_ids=list(range(B)))
    return np.stack([np.asarray(res.results[b]["y"], dtype=np.float32)
                     for b in range(B)])


# revision 17
# speedup vs baseline: 1.1370x; 1.0449x over previous
"""Trainium2 Bass kernel for nn_DeformableConvLayer.

Math (validated vs reference in numpy):
  xf   = sum_c w_icfd[c] * x[:, c] + b_icfd                       (B,H,W)
  mean = mean(xf, (h,w));  dy/dx = mean*w_off + b_off             (per b, 1600 stencils)
  The whole translate+fuse stage is a dense 19x19 conv with a data-dependent
  per-b kernel K_b[ky,kx] = sum_s w_fus[g_s]*hat(dy_s-ky)*hat(dx_s-kx),
  hat(t) = max(0, 1-|t|)  (bilinear weights == hat at integer taps).
  inp  = conv2d(xf, K_b, zero-pad) + 64*b_fus + xf
  y    = conv2d(inp, w_conv 3x3, zero-pad) + b_conv               (B,64,H,W)

Sharding: data-parallel, one batch element per NeuronCore (B=8, 8 cores).
Wide data paths run in bf16: x is cast to bf16 during the SWDGE load DMA,
all conv matmuls are bf16 (fp32 PSUM accumulation), and y is stored as bf16
and widened to fp32 on the host.  Stage-1 runs as Toeplitz-banded matmuls on
the tensor engine; the banded lhsT tables are materialized from K_b via
row-reversed staircase reads of a padded DRAM buffer (contiguous 4.8KB
descriptors).  xf stays on-chip: stage-0 PSUM is evacuated (bias fused) to a
staging tile and scattered to the padded xf tile by SBUF->SBUF DMA.
"""
import numpy as np
import ml_dtypes

import concourse.bacc as bacc
import concourse.bass as bass
import concourse.tile as tile
from concourse import mybir
from concourse.bass import ds, ts

F32 = mybir.dt.float32
BF16 = mybir.dt.bfloat16
NPBF = np.dtype(ml_dtypes.bfloat16)

B, C, H, W = 8, 64, 256, 256
G, DFC = 25, 64
R = 9
NT = 2 * R + 1            # 19 taps
C0 = 145                  # reversed-K row anchor in K_dram
HW = H * W
XHW = 274                 # xf_pad per-half width: 9 | 256 | 9


def _consts(params):
    """Host-side constant tensors derived from the (small) param inputs."""
    w_icfd = params["w_icfd"].astype(np.float32)
    w_off = params["w_off"].astype(np.float32)
    b_off = params["b_off"].astype(np.float32)
    w_fus = params["w_fus"].astype(np.float32)
    b_fus = float(params["b_fus"])
    w_conv = params["w_conv"].astype(np.float32)
    b_conv = params["b_conv"].astype(np.float32)

    W0 = np.zeros((128, 2), np.float32)
    for half in range(2):
        W0[half * 64:(half + 1) * 64, half] = w_icfd

    W2 = np.zeros((18, 128), np.float32)
    for g in range(2):
        for ky2 in range(3):
            for kx2 in range(3):
                W2[g * 9 + ky2 * 3 + kx2, g * 64:(g + 1) * 64] = w_conv[:, 0, ky2, kx2]
    BC = np.zeros((128, 1), np.float32)
    BC[0:64, 0] = b_conv
    BC[64:128, 0] = b_conv

    taps_rev = (R - np.arange(NT)).astype(np.float32)     # [9, 8, ..., -9]
    taps_fwd = (np.arange(NT) - R).astype(np.float32)     # [-9, ..., 9]
    TAPSF = np.tile(taps_fwd[None, :], (128, 1))
    TAPSR = np.tile(taps_rev[None, :], (128, 1))

    # s-chunk layout: s = c*128 + p, 13 chunks; tail (s>=1600) padded with zeros
    WF = np.zeros((128, 13), np.float32)
    WOFF = np.zeros((128, 26), np.float32)    # cols 0..12 y, 13..25 x
    BOFF = np.zeros((128, 26), np.float32)
    for c in range(13):
        for p in range(128):
            s = c * 128 + p
            if s < 1600:
                WF[p, c] = w_fus[s // 64]
                WOFF[p, c] = w_off[2 * s]
                BOFF[p, c] = b_off[2 * s]
                WOFF[p, 13 + c] = w_off[2 * s + 1]
                BOFF[p, 13 + c] = b_off[2 * s + 1]

    C_total = DFC * b_fus
    return dict(
        W0=W0.astype(NPBF), W2=W2.astype(NPBF), BC=BC,
        TAPSF=TAPSF, TAPSR=TAPSR, WF=WF, WOFF=WOFF, BOFF=BOFF,
        I128=np.eye(128, dtype=np.float32).astype(NPBF),
        ONESR=np.ones((1, 256), np.float32),
        ONESC=np.ones((128, 1), np.float32),
        ONES2=np.ones((2, 128), np.float32),
        CVEC=np.full((1, 128), C_total, np.float32),
        b_icfd=float(params["b_icfd"]),
    )


def build(params, num_devices=8):
    cs = _consts(params)
    nc = bacc.Bacc("TRN2", target_bir_lowering=False, debug=False,
                   num_devices=num_devices)
    xb = nc.dram_tensor("xb", [C, H, W], F32, kind="ExternalInput")
    y = nc.dram_tensor("y", [64, H, W], BF16, kind="ExternalOutput")
    K_dram = nc.dram_tensor("k_scr", [280, NT], BF16, kind="Internal")
    inp_dram = nc.dram_tensor("inp_scr", [260, 792], BF16, kind="Internal")

    ct = {k: nc.inline_tensor(v, name=f"c_{k}") for k, v in cs.items()
          if isinstance(v, np.ndarray)}
    b_icfd = cs["b_icfd"]

    def _graph(tc):
        with (
            tc.tile_pool(name="consts", bufs=1) as cp,
            tc.tile_pool(name="persist", bufs=1) as pp,
        ):
            # ---- load constants ----
            sb = {}
            for i, k in enumerate(("W0", "W2", "BC", "TAPSF", "TAPSR", "WF",
                                   "WOFF", "BOFF", "I128", "ONESR", "ONESC",
                                   "ONES2", "CVEC")):
                dt = BF16 if cs[k].dtype == NPBF else F32
                t = cp.tile(list(cs[k].shape), dt, tag=k, name=f"sb_{k}")
                nc.sync.dma_start(out=t, in_=ct[k][:, :])
                sb[k] = t
            zbf = cp.tile([128, 792], BF16, tag="zbf")
            nc.vector.memset(zbf, 0.0)
            bic2 = cp.tile([2, 1], F32, tag="bic2")
            nc.vector.memset(bic2, b_icfd)

            # ---- zero scratch DRAM (early, off critical path) ----
            nc.sync.dma_start(
                out=bass.AP(tensor=K_dram, offset=0, ap=[[NT, 128], [1, NT]]),
                in_=zbf[:, 0:NT])
            nc.sync.dma_start(
                out=bass.AP(tensor=K_dram, offset=128 * NT,
                            ap=[[NT, 128], [1, NT]]),
                in_=zbf[:, 0:NT])
            nc.sync.dma_start(
                out=bass.AP(tensor=K_dram, offset=256 * NT,
                            ap=[[NT, 24], [1, NT]]),
                in_=zbf[0:24, 0:NT])
            nc.sync.dma_start(out=inp_dram[0:128, :], in_=zbf[:, 0:792])
            nc.sync.dma_start(out=inp_dram[128:256, :], in_=zbf[:, 0:792])
            nc.sync.dma_start(out=inp_dram[256:260, :], in_=zbf[0:4, 0:792])

            # ---- persistent xf tile: [128, 274*2] bf16, halves side by side
            xf_pad = pp.tile([128, 2 * XHW], BF16, tag="xf_pad")
            nc.vector.memset(xf_pad, 0.0)

            def _heat(n, tag):
                # keep-warm matmuls into a scratch PSUM bank: fill PE idle
                # gaps so real matmuls dispatch into a warm p-state streak
                with tc.tile_pool(name=f"heat_{tag}", bufs=1,
                                  space="PSUM") as hp:
                    pdum = hp.tile([2, 512], F32, tag=f"pdum{tag}")
                    for _ in range(n):
                        nc.tensor.matmul(pdum, sb["W0"], zbf[:, 0:512],
                                         start=True, stop=True)

            _heat(10, "a")
            # ---- phase B: x cast-load + stage-0 matmul + evac + scatter ----
            NCH = 8                       # chunks of 16 rows
            with (
                tc.tile_pool(name="bpool", bufs=3) as bp,
                tc.tile_pool(name="psum0", bufs=4, space="PSUM") as p0p,
            ):
                for ch in range(NCH):
                    sbx = bp.tile([128, 4096], BF16, tag="sbx", bufs=6)
                    for half in range(2):   # partition = half*64 + c
                        srcp = bass.AP(tensor=xb,
                                       offset=(half * 128 + ch * 16) * W,
                                       ap=[[HW, 64], [1, 4096]])
                        nc.gpsimd.dma_start(out=sbx[ts(half, 64), :], in_=srcp)
                    s0b = bp.tile([2, 4096], BF16, tag="s0b", bufs=3)
                    for q in range(4):
                        p0 = p0p.tile([2, 1024], F32, tag="p0", name="p0t")
                        for j in range(2):
                            nc.tensor.matmul(
                                p0[:, ts(j, 512)],
                                sb["W0"],
                                sbx[:, ds(q * 1024 + j * 512, 512)],
                                start=True, stop=True)
                        # evac PSUM -> bf16 staging with b_icfd bias fused
                        if (ch * 4 + q) % 2 == 0:
                            nc.scalar.activation(
                                out=s0b[:, ts(q, 1024)], in_=p0,
                                func=mybir.ActivationFunctionType.Identity,
                                bias=bic2[:, 0:1], scale=1.0)
                        else:
                            nc.vector.tensor_scalar_add(
                                out=s0b[:, ts(q, 1024)], in0=p0,
                                scalar1=bic2[:, 0:1])
                    for half in range(2):   # scatter 16 rows into xf_pad
                        nc.sync.dma_start(
                            out=xf_pad[ch * 16:ch * 16 + 16,
                                       ds(half * XHW + R, 256)],
                            in_=s0b[half:half + 1, :])

            # ---- phase C: column sums -> mean ----
            colsums = pp.tile([128, 1], F32, tag="colsums")
            nc.vector.tensor_reduce(out=colsums,
                                    in_=xf_pad[:, 0:2 * XHW],
                                    axis=mybir.AxisListType.X,
                                    op=mybir.AluOpType.add)
            with tc.tile_pool(name="psA", bufs=1, space="PSUM") as psA:
                pm = psA.tile([1, 1], F32, tag="pm")
                nc.tensor.matmul(pm, colsums, sb["ONESC"], start=True, stop=True)
                ts2 = pp.tile([1, 1], F32, tag="ts2")
                nc.scalar.copy(out=ts2, in_=pm)
                pmb = psA.tile([128, 1], F32, tag="pmb")
                nc.tensor.matmul(pmb, sb["ONES2"][0:1, :], ts2,
                                 start=True, stop=True)
                mean_bc = pp.tile([128, 1], F32, tag="mean_bc")
                nc.scalar.activation(out=mean_bc, in_=pmb,
                                     func=mybir.ActivationFunctionType.Copy,
                                     scale=1.0 / HW)

                # ---- phase D: offsets, hats, K matmul ----
                dyx = pp.tile([128, 26], F32, tag="dyx")
                nc.vector.tensor_scalar_mul(out=dyx, in0=sb["WOFF"],
                                            scalar1=mean_bc[:, 0:1])
                nc.vector.tensor_add(out=dyx, in0=dyx, in1=sb["BOFF"])
                HH = pp.tile([128, 26 * NT], F32, tag="HH")
                HH3 = HH[:].rearrange("p (a b) -> p a b", a=26)
                nc.vector.tensor_tensor(
                    out=HH3[:, 0:13, :],
                    in0=dyx[:, 0:13].unsqueeze(2).to_broadcast([128, 13, NT]),
                    in1=sb["TAPSF"][:].unsqueeze(1).to_broadcast([128, 13, NT]),
                    op=mybir.AluOpType.subtract)
                nc.vector.tensor_tensor(
                    out=HH3[:, 13:26, :],
                    in0=dyx[:, 13:26].unsqueeze(2).to_broadcast([128, 13, NT]),
                    in1=sb["TAPSR"][:].unsqueeze(1).to_broadcast([128, 13, NT]),
                    op=mybir.AluOpType.subtract)
                nc.vector.tensor_scalar(out=HH, in0=HH, scalar1=0.0,
                                        scalar2=None,
                                        op0=mybir.AluOpType.abs_max)
                nc.scalar.activation(out=HH, in_=HH,
                                     func=mybir.ActivationFunctionType.Relu,
                                     scale=-1.0, bias=1.0)
                WHY = pp.tile([128, 13 * NT], F32, tag="WHY")
                nc.vector.tensor_tensor(
                    out=WHY[:].rearrange("p (a b) -> p a b", a=13),
                    in0=HH3[:, 0:13, :],
                    in1=sb["WF"][:].unsqueeze(2).to_broadcast([128, 13, NT]),
                    op=mybir.AluOpType.mult)
                WHY3 = WHY[:].rearrange("p (a b) -> p a b", a=13)
                pK = psA.tile([NT, NT], F32, tag="pK")
                for c in range(13):
                    nc.tensor.matmul(pK, WHY3[:, c, :], HH3[:, 13 + c, :],
                                     start=(c == 0), stop=(c == 12))
                Ksb = pp.tile([NT, NT], BF16, tag="Ksb")
                nc.scalar.copy(out=Ksb, in_=pK)

            # ---- phase E: reversed K write + contiguous staircase T tables
            # K_dram[C0 - d] = Ksb[d]  (bf16 cast during SWDGE write)
            nc.sync.dma_start(
                out=bass.AP(tensor=K_dram, offset=C0 * NT,
                            ap=[[-NT, NT], [1, NT]]),
                in_=Ksb)
            T_A = pp.tile([128, 128 * NT], BF16, tag="T_A")
            T_B = pp.tile([9, 128 * NT], BF16, tag="T_B")
            T_C = pp.tile([9, 128 * NT], BF16, tag="T_C")
            # T_A[p, a, e] = K[9+p-a] = K_dram[C0-9-p+a]
            nc.sync.dma_start(
                out=T_A[:].rearrange("p (a b) -> p a b", a=128),
                in_=bass.AP(tensor=K_dram, offset=(C0 - 9) * NT,
                            ap=[[-NT, 128], [NT, 128], [1, NT]]))
            # T_B[p, a, e] = K[p-a] = K_dram[C0-p+a]
            nc.gpsimd.dma_start(
                out=T_B[:].rearrange("p (a b) -> p a b", a=128),
                in_=bass.AP(tensor=K_dram, offset=C0 * NT,
                            ap=[[-NT, 9], [NT, 128], [1, NT]]))
            # T_C[p, a, e] = K[137+p-a] = K_dram[C0-137-p+a]
            nc.gpsimd.dma_start(
                out=T_C[:].rearrange("p (a b) -> p a b", a=128),
                in_=bass.AP(tensor=K_dram, offset=(C0 - 137) * NT,
                            ap=[[-NT, 9], [NT, 128], [1, NT]]))
            T_A3 = T_A[:].rearrange("p (a b) -> p a b", a=128)
            T_B3 = T_B[:].rearrange("p (a b) -> p a b", a=128)
            T_C3 = T_C[:].rearrange("p (a b) -> p a b", a=128)
            # matmul operands must start at partition 0/32/64: copy the 9
            # boundary rows of half 0 (119..127) into a base-0 tile
            xf_b0 = pp.tile([9, XHW], BF16, tag="xf_b0")
            nc.sync.dma_start(out=xf_b0, in_=xf_pad[119:128, 0:XHW])

            _heat(26, "b")
            # ---- phase F: stage-1 Toeplitz matmuls -> inp_dram ----
            with tc.tile_pool(name="psum1", bufs=2, space="PSUM") as p1p:
                for t in range(2):
                    pinp = p1p.tile([128, W], F32, tag="pinp")
                    nmm = NT * 2 + 2
                    i = 0
                    for kxp in range(NT):
                        sl = 18 - kxp
                        nc.tensor.matmul(pinp, T_A3[:, :, kxp],
                                         xf_pad[:, ds(t * XHW + sl, W)],
                                         start=(i == 0), stop=(i == nmm - 1)); i += 1
                        if t == 0:
                            nc.tensor.matmul(pinp, T_C3[0:9, :, kxp],
                                             xf_pad[0:9, ds(XHW + sl, W)],
                                             start=False, stop=(i == nmm - 1)); i += 1
                        else:
                            nc.tensor.matmul(pinp, T_B3[0:9, :, kxp],
                                             xf_b0[:, ds(sl, W)],
                                             start=False, stop=(i == nmm - 1)); i += 1
                    nc.tensor.matmul(pinp, sb["I128"],
                                     xf_pad[:, ds(t * XHW + R, W)],
                                     start=False, stop=False); i += 1
                    nc.tensor.matmul(pinp, sb["CVEC"], sb["ONESR"][0:1, 0:W],
                                     start=False, stop=True); i += 1
                    s1 = pp.tile([128, W], BF16, tag=f"s1_{t}", name=f"s1stage{t}")
                    nc.vector.tensor_copy(out=s1, in_=pinp)
                    # inp3[r, j, e] = inp_padded[r, j + e]; s1 covers padded
                    # cols 1..256 of rows 1+128t..128+128t
                    for j in range(3):
                        dst = bass.AP(
                            tensor=inp_dram,
                            offset=(1 + 128 * t) * 792 + j * 264 + (1 - j),
                            ap=[[792, 128], [1, W]])
                        eng = (nc.sync, nc.gpsimd)[t]
                        eng.dma_start(out=dst, in_=s1)

            _heat(30, "c")
            # ---- phase G: im2col + stage-2 + store ----
            with (
                tc.tile_pool(name="gpool", bufs=3) as gp,
                tc.tile_pool(name="psum2", bufs=4, space="PSUM") as p2p,
            ):
                for ch in range(8):               # h2-chunks of 16
                    im = gp.tile([18, 4096], BF16, tag="im", bufs=4)
                    for g in range(2):
                        srcp = bass.AP(
                            tensor=inp_dram,
                            offset=(g * 128 + ch * 16) * 792,
                            ap=[[264, 9], [792, 16], [1, W]])
                        nc.sync.dma_start(
                            out=im[ts(g, 9), :].rearrange(
                                "p (d e) -> p d e", d=16),
                            in_=srcp)
                    for pair in range(2):         # 8 rows per store
                        ysb = gp.tile([128, 2048], BF16, tag="ysb",
                                      name="ystage", bufs=3)
                        py = p2p.tile([128, 2048], F32, tag="py", bufs=2)
                        for j in range(4):
                            nc.tensor.matmul(
                                py[:, ts(j, 512)], sb["W2"],
                                im[:, ds(pair * 2048 + j * 512, 512)],
                                start=True, stop=True)
                        # single evac per store unit (+ b_conv bias, bf16)
                        u = ch * 2 + pair
                        if (u + u // 2) % 2 == 0:
                            nc.scalar.activation(
                                out=ysb, in_=py,
                                func=mybir.ActivationFunctionType.Identity,
                                bias=sb["BC"][:, 0:1], scale=1.0)
                        else:
                            nc.vector.tensor_scalar_add(
                                out=ysb, in0=py, scalar1=sb["BC"][:, 0:1])
                        dst = bass.AP(
                            tensor=y,
                            offset=(ch * 16 + pair * 8) * W,
                            ap=[[128 * W, 2], [HW, 64], [1, 2048]])
                        nc.gpsimd.dma_start(out=dst, in_=ysb[:])
    with tile.TileContext(nc) as tc:
        _graph(tc)
    nc.finalize()
    return nc


def kernel(**inputs):
    x = np.ascontiguousarray(inputs["x"], dtype=np.float32)
    params = {k: np.asarray(v) for k, v in inputs.items() if k != "x"}
    nc = build(params, num_devices=8)
    from concourse.bass_utils import run_bass_kernel_spmd
    in_maps = [{"xb": np.ascontiguousarray(x[b])} for b in range(B)]
    res = run_bass_kernel_spmd(nc, in_maps, core_ids=list(range(B)))
    return np.stack([np.asarray(res.results[b]["y"], dtype=np.float32)
                     for b in range(B)])


# revision 18
# speedup vs baseline: 1.1455x; 1.0075x over previous
"""Trainium2 Bass kernel for nn_DeformableConvLayer.

Math (validated vs reference in numpy):
  xf   = sum_c w_icfd[c] * x[:, c] + b_icfd                       (B,H,W)
  mean = mean(xf, (h,w));  dy/dx = mean*w_off + b_off             (per b, 1600 stencils)
  The whole translate+fuse stage is a dense 19x19 conv with a data-dependent
  per-b kernel K_b[ky,kx] = sum_s w_fus[g_s]*hat(dy_s-ky)*hat(dx_s-kx),
  hat(t) = max(0, 1-|t|)  (bilinear weights == hat at integer taps).
  inp  = conv2d(xf, K_b, zero-pad) + 64*b_fus + xf
  y    = conv2d(inp, w_conv 3x3, zero-pad) + b_conv               (B,64,H,W)

Sharding: data-parallel, one batch element per NeuronCore (B=8, 8 cores).
Wide data paths run in bf16: x is cast to bf16 during the SWDGE load DMA,
all conv matmuls are bf16 (fp32 PSUM accumulation), and y is stored as bf16
and widened to fp32 on the host.  Stage-1 runs as Toeplitz-banded matmuls on
the tensor engine; the banded lhsT tables are materialized from K_b via
row-reversed staircase reads of a padded DRAM buffer (contiguous 4.8KB
descriptors).  xf stays on-chip: stage-0 PSUM is evacuated (bias fused) to a
staging tile and scattered to the padded xf tile by SBUF->SBUF DMA.
"""
import numpy as np
import ml_dtypes

import concourse.bacc as bacc
import concourse.bass as bass
import concourse.tile as tile
from concourse import mybir
from concourse.bass import ds, ts

F32 = mybir.dt.float32
BF16 = mybir.dt.bfloat16
NPBF = np.dtype(ml_dtypes.bfloat16)

B, C, H, W = 8, 64, 256, 256
G, DFC = 25, 64
R = 9
NT = 2 * R + 1            # 19 taps
C0 = 145                  # reversed-K row anchor in K_dram
HW = H * W
XHW = 274                 # xf_pad per-half width: 9 | 256 | 9


def _consts(params):
    """Host-side constant tensors derived from the (small) param inputs."""
    w_icfd = params["w_icfd"].astype(np.float32)
    w_off = params["w_off"].astype(np.float32)
    b_off = params["b_off"].astype(np.float32)
    w_fus = params["w_fus"].astype(np.float32)
    b_fus = float(params["b_fus"])
    w_conv = params["w_conv"].astype(np.float32)
    b_conv = params["b_conv"].astype(np.float32)

    W0 = np.zeros((128, 2), np.float32)
    for half in range(2):
        W0[half * 64:(half + 1) * 64, half] = w_icfd

    W2 = np.zeros((18, 128), np.float32)
    for g in range(2):
        for ky2 in range(3):
            for kx2 in range(3):
                W2[g * 9 + ky2 * 3 + kx2, g * 64:(g + 1) * 64] = w_conv[:, 0, ky2, kx2]
    BC = np.zeros((128, 1), np.float32)
    BC[0:64, 0] = b_conv
    BC[64:128, 0] = b_conv

    taps_rev = (R - np.arange(NT)).astype(np.float32)     # [9, 8, ..., -9]
    taps_fwd = (np.arange(NT) - R).astype(np.float32)     # [-9, ..., 9]
    TAPSF = np.tile(taps_fwd[None, :], (128, 1))
    TAPSR = np.tile(taps_rev[None, :], (128, 1))

    # s-chunk layout: s = c*128 + p, 13 chunks; tail (s>=1600) padded with zeros
    WF = np.zeros((128, 13), np.float32)
    WOFF = np.zeros((128, 26), np.float32)    # cols 0..12 y, 13..25 x
    BOFF = np.zeros((128, 26), np.float32)
    for c in range(13):
        for p in range(128):
            s = c * 128 + p
            if s < 1600:
                WF[p, c] = w_fus[s // 64]
                WOFF[p, c] = w_off[2 * s]
                BOFF[p, c] = b_off[2 * s]
                WOFF[p, 13 + c] = w_off[2 * s + 1]
                BOFF[p, 13 + c] = b_off[2 * s + 1]

    C_total = DFC * b_fus
    return dict(
        W0=W0.astype(NPBF), W2=W2.astype(NPBF), BC=BC,
        TAPSF=TAPSF, TAPSR=TAPSR, WF=WF, WOFF=WOFF, BOFF=BOFF,
        I128=np.eye(128, dtype=np.float32).astype(NPBF),
        ONESR=np.ones((1, 256), np.float32),
        ONESC=np.ones((128, 1), np.float32),
        ONES2=np.ones((2, 128), np.float32),
        CVEC=np.full((1, 128), C_total, np.float32),
        b_icfd=float(params["b_icfd"]),
    )


def build(params, num_devices=8):
    cs = _consts(params)
    nc = bacc.Bacc("TRN2", target_bir_lowering=False, debug=False,
                   num_devices=num_devices)
    xb = nc.dram_tensor("xb", [C, H, W], F32, kind="ExternalInput")
    y = nc.dram_tensor("y", [64, H, W], BF16, kind="ExternalOutput")
    K_dram = nc.dram_tensor("k_scr", [280, NT], BF16, kind="Internal")
    inp_dram = nc.dram_tensor("inp_scr", [260, 792], BF16, kind="Internal")

    ct = {k: nc.inline_tensor(v, name=f"c_{k}") for k, v in cs.items()
          if isinstance(v, np.ndarray)}
    b_icfd = cs["b_icfd"]

    def _graph(tc):
        with (
            tc.tile_pool(name="consts", bufs=1) as cp,
            tc.tile_pool(name="persist", bufs=1) as pp,
        ):
            # ---- load constants ----
            sb = {}
            for i, k in enumerate(("W0", "W2", "BC", "TAPSF", "TAPSR", "WF",
                                   "WOFF", "BOFF", "I128", "ONESR", "ONESC",
                                   "ONES2", "CVEC")):
                dt = BF16 if cs[k].dtype == NPBF else F32
                t = cp.tile(list(cs[k].shape), dt, tag=k, name=f"sb_{k}")
                nc.sync.dma_start(out=t, in_=ct[k][:, :])
                sb[k] = t
            zbf = cp.tile([128, 792], BF16, tag="zbf")
            nc.vector.memset(zbf, 0.0)
            bic2 = cp.tile([2, 1], F32, tag="bic2")
            nc.vector.memset(bic2, b_icfd)

            # ---- zero scratch DRAM (early, off critical path) ----
            nc.sync.dma_start(
                out=bass.AP(tensor=K_dram, offset=0, ap=[[NT, 128], [1, NT]]),
                in_=zbf[:, 0:NT])
            nc.sync.dma_start(
                out=bass.AP(tensor=K_dram, offset=128 * NT,
                            ap=[[NT, 128], [1, NT]]),
                in_=zbf[:, 0:NT])
            nc.sync.dma_start(
                out=bass.AP(tensor=K_dram, offset=256 * NT,
                            ap=[[NT, 24], [1, NT]]),
                in_=zbf[0:24, 0:NT])
            nc.sync.dma_start(out=inp_dram[0:128, :], in_=zbf[:, 0:792])
            nc.sync.dma_start(out=inp_dram[128:256, :], in_=zbf[:, 0:792])
            nc.sync.dma_start(out=inp_dram[256:260, :], in_=zbf[0:4, 0:792])

            # ---- persistent xf tile: [128, 274*2] bf16, halves side by side
            xf_pad = pp.tile([128, 2 * XHW], BF16, tag="xf_pad")
            nc.vector.memset(xf_pad, 0.0)

            def _heat(n, tag):
                # keep-warm matmuls into a scratch PSUM bank: fill PE idle
                # gaps so real matmuls dispatch into a warm p-state streak
                with tc.tile_pool(name=f"heat_{tag}", bufs=1,
                                  space="PSUM") as hp:
                    pdum = hp.tile([2, 512], F32, tag=f"pdum{tag}")
                    for _ in range(n):
                        nc.tensor.matmul(pdum, sb["W0"], zbf[:, 0:512],
                                         start=True, stop=True)

            _heat(10, "a")
            # ---- phase B: x cast-load + stage-0 matmul + evac + scatter ----
            NCH = 4                       # chunks of 32 rows
            with (
                tc.tile_pool(name="bpool", bufs=3) as bp,
                tc.tile_pool(name="psum0", bufs=4, space="PSUM") as p0p,
            ):
                for ch in range(NCH):
                    sbx = bp.tile([128, 8192], BF16, tag="sbx", bufs=3)
                    for half in range(2):   # partition = half*64 + c
                        srcp = bass.AP(tensor=xb,
                                       offset=(half * 128 + ch * 32) * W,
                                       ap=[[HW, 64], [1, 8192]])
                        nc.gpsimd.dma_start(out=sbx[ts(half, 64), :], in_=srcp)
                    s0b = bp.tile([2, 8192], BF16, tag="s0b", bufs=2)
                    for q in range(8):
                        p0 = p0p.tile([2, 1024], F32, tag="p0", name="p0t")
                        for j in range(2):
                            nc.tensor.matmul(
                                p0[:, ts(j, 512)],
                                sb["W0"],
                                sbx[:, ds(q * 1024 + j * 512, 512)],
                                start=True, stop=True)
                        # evac PSUM -> bf16 staging with b_icfd bias fused
                        if (ch * 8 + q) % 2 == 0:
                            nc.scalar.activation(
                                out=s0b[:, ts(q, 1024)], in_=p0,
                                func=mybir.ActivationFunctionType.Identity,
                                bias=bic2[:, 0:1], scale=1.0)
                        else:
                            nc.vector.tensor_scalar_add(
                                out=s0b[:, ts(q, 1024)], in0=p0,
                                scalar1=bic2[:, 0:1])
                    for half in range(2):   # scatter 32 rows into xf_pad
                        nc.sync.dma_start(
                            out=xf_pad[ch * 32:ch * 32 + 32,
                                       ds(half * XHW + R, 256)],
                            in_=s0b[half:half + 1, :])

            # ---- phase C: column sums -> mean ----
            colsums = pp.tile([128, 1], F32, tag="colsums")
            nc.vector.tensor_reduce(out=colsums,
                                    in_=xf_pad[:, 0:2 * XHW],
                                    axis=mybir.AxisListType.X,
                                    op=mybir.AluOpType.add)
            with tc.tile_pool(name="psA", bufs=1, space="PSUM") as psA:
                pm = psA.tile([1, 1], F32, tag="pm")
                nc.tensor.matmul(pm, colsums, sb["ONESC"], start=True, stop=True)
                ts2 = pp.tile([1, 1], F32, tag="ts2")
                nc.scalar.copy(out=ts2, in_=pm)
                pmb = psA.tile([128, 1], F32, tag="pmb")
                nc.tensor.matmul(pmb, sb["ONES2"][0:1, :], ts2,
                                 start=True, stop=True)
                mean_bc = pp.tile([128, 1], F32, tag="mean_bc")
                nc.scalar.activation(out=mean_bc, in_=pmb,
                                     func=mybir.ActivationFunctionType.Copy,
                                     scale=1.0 / HW)

                # ---- phase D: offsets, hats, K matmul ----
                dyx = pp.tile([128, 26], F32, tag="dyx")
                nc.vector.tensor_scalar_mul(out=dyx, in0=sb["WOFF"],
                                            scalar1=mean_bc[:, 0:1])
                nc.vector.tensor_add(out=dyx, in0=dyx, in1=sb["BOFF"])
                HH = pp.tile([128, 26 * NT], F32, tag="HH")
                HH3 = HH[:].rearrange("p (a b) -> p a b", a=26)
                nc.vector.tensor_tensor(
                    out=HH3[:, 0:13, :],
                    in0=dyx[:, 0:13].unsqueeze(2).to_broadcast([128, 13, NT]),
                    in1=sb["TAPSF"][:].unsqueeze(1).to_broadcast([128, 13, NT]),
                    op=mybir.AluOpType.subtract)
                nc.vector.tensor_tensor(
                    out=HH3[:, 13:26, :],
                    in0=dyx[:, 13:26].unsqueeze(2).to_broadcast([128, 13, NT]),
                    in1=sb["TAPSR"][:].unsqueeze(1).to_broadcast([128, 13, NT]),
                    op=mybir.AluOpType.subtract)
                nc.vector.tensor_scalar(out=HH, in0=HH, scalar1=0.0,
                                        scalar2=None,
                                        op0=mybir.AluOpType.abs_max)
                nc.scalar.activation(out=HH, in_=HH,
                                     func=mybir.ActivationFunctionType.Relu,
                                     scale=-1.0, bias=1.0)
                WHY = pp.tile([128, 13 * NT], F32, tag="WHY")
                nc.vector.tensor_tensor(
                    out=WHY[:].rearrange("p (a b) -> p a b", a=13),
                    in0=HH3[:, 0:13, :],
                    in1=sb["WF"][:].unsqueeze(2).to_broadcast([128, 13, NT]),
                    op=mybir.AluOpType.mult)
                WHY3 = WHY[:].rearrange("p (a b) -> p a b", a=13)
                pK = psA.tile([NT, NT], F32, tag="pK")
                for c in range(13):
                    nc.tensor.matmul(pK, WHY3[:, c, :], HH3[:, 13 + c, :],
                                     start=(c == 0), stop=(c == 12))
                Ksb = pp.tile([NT, NT], BF16, tag="Ksb")
                nc.scalar.copy(out=Ksb, in_=pK)

            # ---- phase E: reversed K write + contiguous staircase T tables
            # K_dram[C0 - d] = Ksb[d]  (bf16 cast during SWDGE write)
            nc.sync.dma_start(
                out=bass.AP(tensor=K_dram, offset=C0 * NT,
                            ap=[[-NT, NT], [1, NT]]),
                in_=Ksb)
            T_A = pp.tile([128, 128 * NT], BF16, tag="T_A")
            T_B = pp.tile([9, 128 * NT], BF16, tag="T_B")
            T_C = pp.tile([9, 128 * NT], BF16, tag="T_C")
            # T_A[p, a, e] = K[9+p-a] = K_dram[C0-9-p+a]
            nc.sync.dma_start(
                out=T_A[:].rearrange("p (a b) -> p a b", a=128),
                in_=bass.AP(tensor=K_dram, offset=(C0 - 9) * NT,
                            ap=[[-NT, 128], [NT, 128], [1, NT]]))
            # T_B[p, a, e] = K[p-a] = K_dram[C0-p+a]
            nc.gpsimd.dma_start(
                out=T_B[:].rearrange("p (a b) -> p a b", a=128),
                in_=bass.AP(tensor=K_dram, offset=C0 * NT,
                            ap=[[-NT, 9], [NT, 128], [1, NT]]))
            # T_C[p, a, e] = K[137+p-a] = K_dram[C0-137-p+a]
            nc.gpsimd.dma_start(
                out=T_C[:].rearrange("p (a b) -> p a b", a=128),
                in_=bass.AP(tensor=K_dram, offset=(C0 - 137) * NT,
                            ap=[[-NT, 9], [NT, 128], [1, NT]]))
            T_A3 = T_A[:].rearrange("p (a b) -> p a b", a=128)
            T_B3 = T_B[:].rearrange("p (a b) -> p a b", a=128)
            T_C3 = T_C[:].rearrange("p (a b) -> p a b", a=128)
            # matmul operands must start at partition 0/32/64: copy the 9
            # boundary rows of half 0 (119..127) into a base-0 tile
            xf_b0 = pp.tile([9, XHW], BF16, tag="xf_b0")
            nc.sync.dma_start(out=xf_b0, in_=xf_pad[119:128, 0:XHW])

            _heat(26, "b")
            # ---- phase F: stage-1 Toeplitz matmuls -> inp_dram ----
            with tc.tile_pool(name="psum1", bufs=2, space="PSUM") as p1p:
                for t in range(2):
                    pinp = p1p.tile([128, W], F32, tag="pinp")
                    nmm = NT * 2 + 2
                    i = 0
                    for kxp in range(NT):
                        sl = 18 - kxp
                        nc.tensor.matmul(pinp, T_A3[:, :, kxp],
                                         xf_pad[:, ds(t * XHW + sl, W)],
                                         start=(i == 0), stop=(i == nmm - 1)); i += 1
                        if t == 0:
                            nc.tensor.matmul(pinp, T_C3[0:9, :, kxp],
                                             xf_pad[0:9, ds(XHW + sl, W)],
                                             start=False, stop=(i == nmm - 1)); i += 1
                        else:
                            nc.tensor.matmul(pinp, T_B3[0:9, :, kxp],
                                             xf_b0[:, ds(sl, W)],
                                             start=False, stop=(i == nmm - 1)); i += 1
                    nc.tensor.matmul(pinp, sb["I128"],
                                     xf_pad[:, ds(t * XHW + R, W)],
                                     start=False, stop=False); i += 1
                    nc.tensor.matmul(pinp, sb["CVEC"], sb["ONESR"][0:1, 0:W],
                                     start=False, stop=True); i += 1
                    s1 = pp.tile([128, W], BF16, tag=f"s1_{t}", name=f"s1stage{t}")
                    nc.vector.tensor_copy(out=s1, in_=pinp)
                    # inp3[r, j, e] = inp_padded[r, j + e]; s1 covers padded
                    # cols 1..256 of rows 1+128t..128+128t
                    for j in range(3):
                        dst = bass.AP(
                            tensor=inp_dram,
                            offset=(1 + 128 * t) * 792 + j * 264 + (1 - j),
                            ap=[[792, 128], [1, W]])
                        eng = (nc.sync, nc.gpsimd)[t]
                        eng.dma_start(out=dst, in_=s1)

            _heat(30, "c")
            # ---- phase G: im2col + stage-2 + store ----
            with (
                tc.tile_pool(name="gpool", bufs=3) as gp,
                tc.tile_pool(name="psum2", bufs=4, space="PSUM") as p2p,
            ):
                for ch in range(8):               # h2-chunks of 16
                    im = gp.tile([18, 4096], BF16, tag="im", bufs=4)
                    for g in range(2):
                        srcp = bass.AP(
                            tensor=inp_dram,
                            offset=(g * 128 + ch * 16) * 792,
                            ap=[[264, 9], [792, 16], [1, W]])
                        nc.sync.dma_start(
                            out=im[ts(g, 9), :].rearrange(
                                "p (d e) -> p d e", d=16),
                            in_=srcp)
                    for pair in range(2):         # 8 rows per store
                        ysb = gp.tile([128, 2048], BF16, tag="ysb",
                                      name="ystage", bufs=3)
                        py = p2p.tile([128, 2048], F32, tag="py", bufs=2)
                        for j in range(4):
                            nc.tensor.matmul(
                                py[:, ts(j, 512)], sb["W2"],
                                im[:, ds(pair * 2048 + j * 512, 512)],
                                start=True, stop=True)
                        # single evac per store unit (+ b_conv bias, bf16)
                        u = ch * 2 + pair
                        if (u + u // 2) % 2 == 0:
                            nc.scalar.activation(
                                out=ysb, in_=py,
                                func=mybir.ActivationFunctionType.Identity,
                                bias=sb["BC"][:, 0:1], scale=1.0)
                        else:
                            nc.vector.tensor_scalar_add(
                                out=ysb, in0=py, scalar1=sb["BC"][:, 0:1])
                        dst = bass.AP(
                            tensor=y,
                            offset=(ch * 16 + pair * 8) * W,
                            ap=[[128 * W, 2], [HW, 64], [1, 2048]])
                        nc.gpsimd.dma_start(out=dst, in_=ysb[:])
    with tile.TileContext(nc) as tc:
        _graph(tc)
    nc.finalize()
    return nc


def kernel(**inputs):
    x = np.ascontiguousarray(inputs["x"], dtype=np.float32)
    params = {k: np.asarray(v) for k, v in inputs.items() if k != "x"}
    nc = build(params, num_devices=8)
    from concourse.bass_utils import run_bass_kernel_spmd
    in_maps = [{"xb": np.ascontiguousarray(x[b])} for b in range(B)]
    res = run_bass_kernel_spmd(nc, in_maps, core_ids=list(range(B)))
    return np.stack([np.asarray(res.results[b]["y"], dtype=np.float32)
                     for b in range(B)])


# revision 19
# speedup vs baseline: 1.1537x; 1.0071x over previous
"""Trainium2 Bass kernel for nn_DeformableConvLayer.

Math (validated vs reference in numpy):
  xf   = sum_c w_icfd[c] * x[:, c] + b_icfd                       (B,H,W)
  mean = mean(xf, (h,w));  dy/dx = mean*w_off + b_off             (per b, 1600 stencils)
  The whole translate+fuse stage is a dense 19x19 conv with a data-dependent
  per-b kernel K_b[ky,kx] = sum_s w_fus[g_s]*hat(dy_s-ky)*hat(dx_s-kx),
  hat(t) = max(0, 1-|t|)  (bilinear weights == hat at integer taps).
  inp  = conv2d(xf, K_b, zero-pad) + 64*b_fus + xf
  y    = conv2d(inp, w_conv 3x3, zero-pad) + b_conv               (B,64,H,W)

Sharding: data-parallel, one batch element per NeuronCore (B=8, 8 cores).
Wide data paths run in bf16: x is cast to bf16 during the SWDGE load DMA,
all conv matmuls are bf16 (fp32 PSUM accumulation), and y is stored as bf16
and widened to fp32 on the host.  Stage-1 runs as Toeplitz-banded matmuls on
the tensor engine; the banded lhsT tables are materialized from K_b via
row-reversed staircase reads of a padded DRAM buffer (contiguous 4.8KB
descriptors).  xf stays on-chip: stage-0 PSUM is evacuated (bias fused) to a
staging tile and scattered to the padded xf tile by SBUF->SBUF DMA.
"""
import numpy as np
import ml_dtypes

import concourse.bacc as bacc
import concourse.bass as bass
import concourse.tile as tile
from concourse import mybir
from concourse.bass import ds, ts

F32 = mybir.dt.float32
BF16 = mybir.dt.bfloat16
NPBF = np.dtype(ml_dtypes.bfloat16)

B, C, H, W = 8, 64, 256, 256
G, DFC = 25, 64
R = 9
NT = 2 * R + 1            # 19 taps
C0 = 145                  # reversed-K row anchor in K_dram
HW = H * W
XHW = 274                 # xf_pad per-half width: 9 | 256 | 9


def _consts(params):
    """Host-side constant tensors derived from the (small) param inputs."""
    w_icfd = params["w_icfd"].astype(np.float32)
    w_off = params["w_off"].astype(np.float32)
    b_off = params["b_off"].astype(np.float32)
    w_fus = params["w_fus"].astype(np.float32)
    b_fus = float(params["b_fus"])
    w_conv = params["w_conv"].astype(np.float32)
    b_conv = params["b_conv"].astype(np.float32)

    W0 = np.zeros((128, 2), np.float32)
    for half in range(2):
        W0[half * 64:(half + 1) * 64, half] = w_icfd

    W2 = np.zeros((18, 128), np.float32)
    for g in range(2):
        for ky2 in range(3):
            for kx2 in range(3):
                W2[g * 9 + ky2 * 3 + kx2, g * 64:(g + 1) * 64] = w_conv[:, 0, ky2, kx2]
    BC = np.zeros((128, 1), np.float32)
    BC[0:64, 0] = b_conv
    BC[64:128, 0] = b_conv

    taps_rev = (R - np.arange(NT)).astype(np.float32)     # [9, 8, ..., -9]
    taps_fwd = (np.arange(NT) - R).astype(np.float32)     # [-9, ..., 9]
    TAPSF = np.tile(taps_fwd[None, :], (128, 1))
    TAPSR = np.tile(taps_rev[None, :], (128, 1))

    # s-chunk layout: s = c*128 + p, 13 chunks; tail (s>=1600) padded with zeros
    WF = np.zeros((128, 13), np.float32)
    WOFF = np.zeros((128, 26), np.float32)    # cols 0..12 y, 13..25 x
    BOFF = np.zeros((128, 26), np.float32)
    for c in range(13):
        for p in range(128):
            s = c * 128 + p
            if s < 1600:
                WF[p, c] = w_fus[s // 64]
                WOFF[p, c] = w_off[2 * s]
                BOFF[p, c] = b_off[2 * s]
                WOFF[p, 13 + c] = w_off[2 * s + 1]
                BOFF[p, 13 + c] = b_off[2 * s + 1]

    C_total = DFC * b_fus
    return dict(
        W0=W0.astype(NPBF), W2=W2.astype(NPBF), BC=BC,
        TAPSF=TAPSF, TAPSR=TAPSR, WF=WF, WOFF=WOFF, BOFF=BOFF,
        I128=np.eye(128, dtype=np.float32).astype(NPBF),
        ONESR=np.ones((1, 256), np.float32),
        ONESC=np.ones((128, 1), np.float32),
        ONES2=np.ones((2, 128), np.float32),
        CVEC=np.full((1, 128), C_total, np.float32),
        b_icfd=float(params["b_icfd"]),
    )


def build(params, num_devices=8):
    cs = _consts(params)
    nc = bacc.Bacc("TRN2", target_bir_lowering=False, debug=False,
                   num_devices=num_devices)
    xb = nc.dram_tensor("xb", [C, H, W], F32, kind="ExternalInput")
    y = nc.dram_tensor("y", [64, H, W], BF16, kind="ExternalOutput")
    K_dram = nc.dram_tensor("k_scr", [280, NT], BF16, kind="Internal")
    inp_dram = nc.dram_tensor("inp_scr", [260, 792], BF16, kind="Internal")

    ct = {k: nc.inline_tensor(v, name=f"c_{k}") for k, v in cs.items()
          if isinstance(v, np.ndarray)}
    b_icfd = cs["b_icfd"]

    def _graph(tc):
        with (
            tc.tile_pool(name="consts", bufs=1) as cp,
            tc.tile_pool(name="persist", bufs=1) as pp,
        ):
            # ---- load constants ----
            sb = {}
            for i, k in enumerate(("W0", "W2", "BC", "TAPSF", "TAPSR", "WF",
                                   "WOFF", "BOFF", "I128", "ONESR", "ONESC",
                                   "ONES2", "CVEC")):
                dt = BF16 if cs[k].dtype == NPBF else F32
                t = cp.tile(list(cs[k].shape), dt, tag=k, name=f"sb_{k}")
                eng = (nc.sync, nc.scalar)[i % 2]
                eng.dma_start(out=t, in_=ct[k][:, :])
                sb[k] = t
            zbf = cp.tile([128, 792], BF16, tag="zbf")
            nc.vector.memset(zbf, 0.0)
            bic2 = cp.tile([2, 1], F32, tag="bic2")
            nc.vector.memset(bic2, b_icfd)

            # ---- zero scratch DRAM (early, off critical path) ----
            nc.sync.dma_start(
                out=bass.AP(tensor=K_dram, offset=0, ap=[[NT, 128], [1, NT]]),
                in_=zbf[:, 0:NT])
            nc.sync.dma_start(
                out=bass.AP(tensor=K_dram, offset=128 * NT,
                            ap=[[NT, 128], [1, NT]]),
                in_=zbf[:, 0:NT])
            nc.sync.dma_start(
                out=bass.AP(tensor=K_dram, offset=256 * NT,
                            ap=[[NT, 24], [1, NT]]),
                in_=zbf[0:24, 0:NT])
            nc.scalar.dma_start(out=inp_dram[0:128, :], in_=zbf[:, 0:792])
            nc.scalar.dma_start(out=inp_dram[128:256, :], in_=zbf[:, 0:792])
            nc.scalar.dma_start(out=inp_dram[256:260, :], in_=zbf[0:4, 0:792])

            # ---- persistent xf tile: [128, 274*2] bf16, halves side by side
            xf_pad = pp.tile([128, 2 * XHW], BF16, tag="xf_pad")
            nc.vector.memset(xf_pad, 0.0)

            def _heat(n, tag):
                # keep-warm matmuls into a scratch PSUM bank: fill PE idle
                # gaps so real matmuls dispatch into a warm p-state streak
                with tc.tile_pool(name=f"heat_{tag}", bufs=1,
                                  space="PSUM") as hp:
                    pdum = hp.tile([2, 512], F32, tag=f"pdum{tag}")
                    for _ in range(n):
                        nc.tensor.matmul(pdum, sb["W0"], zbf[:, 0:512],
                                         start=True, stop=True)

            _heat(10, "a")
            # ---- phase B: x cast-load + stage-0 matmul + evac + scatter ----
            NCH = 8                       # chunks of 16 rows
            with (
                tc.tile_pool(name="bpool", bufs=3) as bp,
                tc.tile_pool(name="psum0", bufs=4, space="PSUM") as p0p,
            ):
                for ch in range(NCH):
                    sbx = bp.tile([128, 4096], BF16, tag="sbx", bufs=6)
                    for half in range(2):   # partition = half*64 + c
                        srcp = bass.AP(tensor=xb,
                                       offset=(half * 128 + ch * 16) * W,
                                       ap=[[HW, 64], [1, 4096]])
                        nc.gpsimd.dma_start(out=sbx[ts(half, 64), :], in_=srcp)
                    s0b = bp.tile([2, 4096], BF16, tag="s0b", bufs=3)
                    for q in range(4):
                        p0 = p0p.tile([2, 1024], F32, tag="p0", name="p0t")
                        for j in range(2):
                            nc.tensor.matmul(
                                p0[:, ts(j, 512)],
                                sb["W0"],
                                sbx[:, ds(q * 1024 + j * 512, 512)],
                                start=True, stop=True)
                        # evac PSUM -> bf16 staging with b_icfd bias fused
                        if (ch * 4 + q) % 2 == 0:
                            nc.scalar.activation(
                                out=s0b[:, ts(q, 1024)], in_=p0,
                                func=mybir.ActivationFunctionType.Identity,
                                bias=bic2[:, 0:1], scale=1.0)
                        else:
                            nc.vector.tensor_scalar_add(
                                out=s0b[:, ts(q, 1024)], in0=p0,
                                scalar1=bic2[:, 0:1])
                    for half in range(2):   # scatter 16 rows into xf_pad
                        nc.sync.dma_start(
                            out=xf_pad[ch * 16:ch * 16 + 16,
                                       ds(half * XHW + R, 256)],
                            in_=s0b[half:half + 1, :])

            # ---- phase C: column sums -> mean ----
            colsums = pp.tile([128, 1], F32, tag="colsums")
            nc.vector.tensor_reduce(out=colsums,
                                    in_=xf_pad[:, 0:2 * XHW],
                                    axis=mybir.AxisListType.X,
                                    op=mybir.AluOpType.add)
            with tc.tile_pool(name="psA", bufs=1, space="PSUM") as psA:
                pm = psA.tile([1, 1], F32, tag="pm")
                nc.tensor.matmul(pm, colsums, sb["ONESC"], start=True, stop=True)
                ts2 = pp.tile([1, 1], F32, tag="ts2")
                nc.scalar.copy(out=ts2, in_=pm)
                pmb = psA.tile([128, 1], F32, tag="pmb")
                nc.tensor.matmul(pmb, sb["ONES2"][0:1, :], ts2,
                                 start=True, stop=True)
                mean_bc = pp.tile([128, 1], F32, tag="mean_bc")
                nc.scalar.activation(out=mean_bc, in_=pmb,
                                     func=mybir.ActivationFunctionType.Copy,
                                     scale=1.0 / HW)

                # ---- phase D: offsets, hats, K matmul ----
                dyx = pp.tile([128, 26], F32, tag="dyx")
                nc.vector.tensor_scalar_mul(out=dyx, in0=sb["WOFF"],
                                            scalar1=mean_bc[:, 0:1])
                nc.vector.tensor_add(out=dyx, in0=dyx, in1=sb["BOFF"])
                HH = pp.tile([128, 26 * NT], F32, tag="HH")
                HH3 = HH[:].rearrange("p (a b) -> p a b", a=26)
                nc.vector.tensor_tensor(
                    out=HH3[:, 0:13, :],
                    in0=dyx[:, 0:13].unsqueeze(2).to_broadcast([128, 13, NT]),
                    in1=sb["TAPSF"][:].unsqueeze(1).to_broadcast([128, 13, NT]),
                    op=mybir.AluOpType.subtract)
                nc.vector.tensor_tensor(
                    out=HH3[:, 13:26, :],
                    in0=dyx[:, 13:26].unsqueeze(2).to_broadcast([128, 13, NT]),
                    in1=sb["TAPSR"][:].unsqueeze(1).to_broadcast([128, 13, NT]),
                    op=mybir.AluOpType.subtract)
                nc.vector.tensor_scalar(out=HH, in0=HH, scalar1=0.0,
                                        scalar2=None,
                                        op0=mybir.AluOpType.abs_max)
                nc.scalar.activation(out=HH, in_=HH,
                                     func=mybir.ActivationFunctionType.Relu,
                                     scale=-1.0, bias=1.0)
                WHY = pp.tile([128, 13 * NT], F32, tag="WHY")
                nc.vector.tensor_tensor(
                    out=WHY[:].rearrange("p (a b) -> p a b", a=13),
                    in0=HH3[:, 0:13, :],
                    in1=sb["WF"][:].unsqueeze(2).to_broadcast([128, 13, NT]),
                    op=mybir.AluOpType.mult)
                WHY3 = WHY[:].rearrange("p (a b) -> p a b", a=13)
                pK = psA.tile([NT, NT], F32, tag="pK")
                for c in range(13):
                    nc.tensor.matmul(pK, WHY3[:, c, :], HH3[:, 13 + c, :],
                                     start=(c == 0), stop=(c == 12))
                Ksb = pp.tile([NT, NT], BF16, tag="Ksb")
                nc.scalar.copy(out=Ksb, in_=pK)

            # ---- phase E: reversed K write + contiguous staircase T tables
            # K_dram[C0 - d] = Ksb[d]  (bf16 cast during SWDGE write)
            nc.sync.dma_start(
                out=bass.AP(tensor=K_dram, offset=C0 * NT,
                            ap=[[-NT, NT], [1, NT]]),
                in_=Ksb)
            T_A = pp.tile([128, 128 * NT], BF16, tag="T_A")
            T_B = pp.tile([9, 128 * NT], BF16, tag="T_B")
            T_C = pp.tile([9, 128 * NT], BF16, tag="T_C")
            # T_A[p, a, e] = K[9+p-a] = K_dram[C0-9-p+a]
            nc.sync.dma_start(
                out=T_A[:].rearrange("p (a b) -> p a b", a=128),
                in_=bass.AP(tensor=K_dram, offset=(C0 - 9) * NT,
                            ap=[[-NT, 128], [NT, 128], [1, NT]]))
            # T_B[p, a, e] = K[p-a] = K_dram[C0-p+a]
            nc.gpsimd.dma_start(
                out=T_B[:].rearrange("p (a b) -> p a b", a=128),
                in_=bass.AP(tensor=K_dram, offset=C0 * NT,
                            ap=[[-NT, 9], [NT, 128], [1, NT]]))
            # T_C[p, a, e] = K[137+p-a] = K_dram[C0-137-p+a]
            nc.gpsimd.dma_start(
                out=T_C[:].rearrange("p (a b) -> p a b", a=128),
                in_=bass.AP(tensor=K_dram, offset=(C0 - 137) * NT,
                            ap=[[-NT, 9], [NT, 128], [1, NT]]))
            T_A3 = T_A[:].rearrange("p (a b) -> p a b", a=128)
            T_B3 = T_B[:].rearrange("p (a b) -> p a b", a=128)
            T_C3 = T_C[:].rearrange("p (a b) -> p a b", a=128)
            # matmul operands must start at partition 0/32/64: copy the 9
            # boundary rows of half 0 (119..127) into a base-0 tile
            xf_b0 = pp.tile([9, XHW], BF16, tag="xf_b0")
            nc.sync.dma_start(out=xf_b0, in_=xf_pad[119:128, 0:XHW])

            _heat(26, "b")
            # ---- phase F: stage-1 Toeplitz matmuls -> inp_dram ----
            with tc.tile_pool(name="psum1", bufs=2, space="PSUM") as p1p:
                for t in range(2):
                    pinp = p1p.tile([128, W], F32, tag="pinp")
                    nmm = NT * 2 + 2
                    i = 0
                    for kxp in range(NT):
                        sl = 18 - kxp
                        nc.tensor.matmul(pinp, T_A3[:, :, kxp],
                                         xf_pad[:, ds(t * XHW + sl, W)],
                                         start=(i == 0), stop=(i == nmm - 1)); i += 1
                        if t == 0:
                            nc.tensor.matmul(pinp, T_C3[0:9, :, kxp],
                                             xf_pad[0:9, ds(XHW + sl, W)],
                                             start=False, stop=(i == nmm - 1)); i += 1
                        else:
                            nc.tensor.matmul(pinp, T_B3[0:9, :, kxp],
                                             xf_b0[:, ds(sl, W)],
                                             start=False, stop=(i == nmm - 1)); i += 1
                    nc.tensor.matmul(pinp, sb["I128"],
                                     xf_pad[:, ds(t * XHW + R, W)],
                                     start=False, stop=False); i += 1
                    nc.tensor.matmul(pinp, sb["CVEC"], sb["ONESR"][0:1, 0:W],
                                     start=False, stop=True); i += 1
                    s1 = pp.tile([128, W], BF16, tag=f"s1_{t}", name=f"s1stage{t}")
                    nc.vector.tensor_copy(out=s1, in_=pinp)
                    # inp3[r, j, e] = inp_padded[r, j + e]; s1 covers padded
                    # cols 1..256 of rows 1+128t..128+128t
                    for j in range(3):
                        dst = bass.AP(
                            tensor=inp_dram,
                            offset=(1 + 128 * t) * 792 + j * 264 + (1 - j),
                            ap=[[792, 128], [1, W]])
                        eng = (nc.sync, nc.gpsimd)[t]
                        eng.dma_start(out=dst, in_=s1)

            _heat(30, "c")
            # ---- phase G: im2col + stage-2 + store ----
            with (
                tc.tile_pool(name="gpool", bufs=3) as gp,
                tc.tile_pool(name="psum2", bufs=4, space="PSUM") as p2p,
            ):
                for ch in range(8):               # h2-chunks of 16
                    im = gp.tile([18, 4096], BF16, tag="im", bufs=4)
                    for g in range(2):
                        srcp = bass.AP(
                            tensor=inp_dram,
                            offset=(g * 128 + ch * 16) * 792,
                            ap=[[264, 9], [792, 16], [1, W]])
                        nc.sync.dma_start(
                            out=im[ts(g, 9), :].rearrange(
                                "p (d e) -> p d e", d=16),
                            in_=srcp)
                    for pair in range(2):         # 8 rows per store
                        ysb = gp.tile([128, 2048], BF16, tag="ysb",
                                      name="ystage", bufs=3)
                        py = p2p.tile([128, 2048], F32, tag="py", bufs=2)
                        for j in range(4):
                            nc.tensor.matmul(
                                py[:, ts(j, 512)], sb["W2"],
                                im[:, ds(pair * 2048 + j * 512, 512)],
                                start=True, stop=True)
                        # single evac per store unit (+ b_conv bias, bf16)
                        u = ch * 2 + pair
                        if (u + u // 2) % 2 == 0:
                            nc.scalar.activation(
                                out=ysb, in_=py,
                                func=mybir.ActivationFunctionType.Identity,
                                bias=sb["BC"][:, 0:1], scale=1.0)
                        else:
                            nc.vector.tensor_scalar_add(
                                out=ysb, in0=py, scalar1=sb["BC"][:, 0:1])
                        dst = bass.AP(
                            tensor=y,
                            offset=(ch * 16 + pair * 8) * W,
                            ap=[[128 * W, 2], [HW, 64], [1, 2048]])
                        nc.gpsimd.dma_start(out=dst, in_=ysb[:])
    with tile.TileContext(nc) as tc:
        _graph(tc)
    nc.finalize()
    return nc


def kernel(**inputs):
    x = np.ascontiguousarray(inputs["x"], dtype=np.float32)
    params = {k: np.asarray(v) for k, v in inputs.items() if k != "x"}
    nc = build(params, num_devices=8)
    from concourse.bass_utils import run_bass_kernel_spmd
    in_maps = [{"xb": np.ascontiguousarray(x[b])} for b in range(B)]
    res = run_bass_kernel_spmd(nc, in_maps, core_ids=list(range(B)))
    return np.stack([np.asarray(res.results[b]["y"], dtype=np.float32)
                     for b in range(B)])


# revision 20
# speedup vs baseline: 1.2265x; 1.0631x over previous
"""Trainium2 Bass kernel for nn_DeformableConvLayer.

Math (validated vs reference in numpy):
  xf   = sum_c w_icfd[c] * x[:, c] + b_icfd                       (B,H,W)
  mean = mean(xf, (h,w));  dy/dx = mean*w_off + b_off             (per b, 1600 stencils)
  The whole translate+fuse stage is a dense 19x19 conv with a data-dependent
  per-b kernel K_b[ky,kx] = sum_s w_fus[g_s]*hat(dy_s-ky)*hat(dx_s-kx),
  hat(t) = max(0, 1-|t|)  (bilinear weights == hat at integer taps).
  inp  = conv2d(xf, K_b, zero-pad) + 64*b_fus + xf
  y    = conv2d(inp, w_conv 3x3, zero-pad) + b_conv               (B,64,H,W)

Sharding: data-parallel, one batch element per NeuronCore (B=8, 8 cores).
Wide data paths run in bf16: x is cast to bf16 during the SWDGE load DMA,
all conv matmuls are bf16 (fp32 PSUM accumulation), and y is stored as bf16
and widened to fp32 on the host.  Stage-1 runs as Toeplitz-banded matmuls on
the tensor engine; the banded lhsT tables are materialized from K_b via
row-reversed staircase reads of a padded DRAM buffer (contiguous 4.8KB
descriptors).  xf stays on-chip: stage-0 PSUM is evacuated (bias fused) to a
staging tile and scattered to the padded xf tile by SBUF->SBUF DMA.
"""
import numpy as np
import ml_dtypes

import concourse.bacc as bacc
import concourse.bass as bass
import concourse.tile as tile
from concourse import mybir
from concourse.bass import ds, ts

F32 = mybir.dt.float32
BF16 = mybir.dt.bfloat16
NPBF = np.dtype(ml_dtypes.bfloat16)

B, C, H, W = 8, 64, 256, 256
G, DFC = 25, 64
R = 9
NT = 2 * R + 1            # 19 taps
C0 = 145                  # reversed-K row anchor in K_dram
HW = H * W
XHW = 274                 # xf_pad per-half width: 9 | 256 | 9


def _consts(params):
    """Host-side constant tensors derived from the (small) param inputs."""
    w_icfd = params["w_icfd"].astype(np.float32)
    w_off = params["w_off"].astype(np.float32)
    b_off = params["b_off"].astype(np.float32)
    w_fus = params["w_fus"].astype(np.float32)
    b_fus = float(params["b_fus"])
    w_conv = params["w_conv"].astype(np.float32)
    b_conv = params["b_conv"].astype(np.float32)

    W0 = np.zeros((128, 2), np.float32)
    for half in range(2):
        W0[half * 64:(half + 1) * 64, half] = w_icfd

    W2 = np.zeros((18, 128), np.float32)
    for g in range(2):
        for ky2 in range(3):
            for kx2 in range(3):
                W2[g * 9 + ky2 * 3 + kx2, g * 64:(g + 1) * 64] = w_conv[:, 0, ky2, kx2]
    BC = np.zeros((128, 1), np.float32)
    BC[0:64, 0] = b_conv
    BC[64:128, 0] = b_conv

    taps_rev = (R - np.arange(NT)).astype(np.float32)     # [9, 8, ..., -9]
    taps_fwd = (np.arange(NT) - R).astype(np.float32)     # [-9, ..., 9]
    TAPSF = np.tile(taps_fwd[None, :], (128, 1))
    TAPSR = np.tile(taps_rev[None, :], (128, 1))

    # s-chunk layout: s = c*128 + p, 13 chunks; tail (s>=1600) padded with zeros
    WF = np.zeros((128, 13), np.float32)
    WOFF = np.zeros((128, 26), np.float32)    # cols 0..12 y, 13..25 x
    BOFF = np.zeros((128, 26), np.float32)
    for c in range(13):
        for p in range(128):
            s = c * 128 + p
            if s < 1600:
                WF[p, c] = w_fus[s // 64]
                WOFF[p, c] = w_off[2 * s]
                BOFF[p, c] = b_off[2 * s]
                WOFF[p, 13 + c] = w_off[2 * s + 1]
                BOFF[p, 13 + c] = b_off[2 * s + 1]

    C_total = DFC * b_fus
    return dict(
        W0=W0.astype(NPBF), W2=W2.astype(NPBF), BC=BC,
        TAPSF=TAPSF, TAPSR=TAPSR, WF=WF, WOFF=WOFF, BOFF=BOFF,
        I128=np.eye(128, dtype=np.float32).astype(NPBF),
        ONESR=np.ones((1, 256), np.float32),
        ONESC=np.ones((128, 1), np.float32),
        ONES2=np.ones((2, 128), np.float32),
        CVEC=np.full((1, 128), C_total, np.float32),
        b_icfd=float(params["b_icfd"]),
    )


def build(params, num_devices=8):
    cs = _consts(params)
    nc = bacc.Bacc("TRN2", target_bir_lowering=False, debug=False,
                   num_devices=num_devices)
    xb = nc.dram_tensor("xb", [C, H, W], F32, kind="ExternalInput")
    y = nc.dram_tensor("y", [64, H, W], BF16, kind="ExternalOutput")
    K_dram = nc.dram_tensor("k_scr", [280, NT], BF16, kind="Internal")
    inp_dram = nc.dram_tensor("inp_scr", [260, 792], BF16, kind="Internal")

    ct = {k: nc.inline_tensor(v, name=f"c_{k}") for k, v in cs.items()
          if isinstance(v, np.ndarray)}
    b_icfd = cs["b_icfd"]

    def _graph(tc):
        with (
            tc.tile_pool(name="consts", bufs=1) as cp,
            tc.tile_pool(name="persist", bufs=1) as pp,
        ):
            # ---- load constants ----
            sb = {}
            for i, k in enumerate(("W0", "W2", "BC", "TAPSF", "TAPSR", "WF",
                                   "WOFF", "BOFF", "I128", "ONESR", "ONESC",
                                   "ONES2", "CVEC")):
                dt = BF16 if cs[k].dtype == NPBF else F32
                t = cp.tile(list(cs[k].shape), dt, tag=k, name=f"sb_{k}")
                eng = (nc.sync, nc.scalar)[i % 2]
                eng.dma_start(out=t, in_=ct[k][:, :])
                sb[k] = t
            zbf = cp.tile([128, 792], BF16, tag="zbf")
            nc.vector.memset(zbf, 0.0)
            bic2 = cp.tile([2, 1], F32, tag="bic2")
            nc.vector.memset(bic2, b_icfd)

            # ---- zero scratch DRAM (early, off critical path) ----
            nc.sync.dma_start(
                out=bass.AP(tensor=K_dram, offset=0, ap=[[NT, 128], [1, NT]]),
                in_=zbf[:, 0:NT])
            nc.sync.dma_start(
                out=bass.AP(tensor=K_dram, offset=128 * NT,
                            ap=[[NT, 128], [1, NT]]),
                in_=zbf[:, 0:NT])
            nc.sync.dma_start(
                out=bass.AP(tensor=K_dram, offset=256 * NT,
                            ap=[[NT, 24], [1, NT]]),
                in_=zbf[0:24, 0:NT])
            nc.scalar.dma_start(out=inp_dram[0:128, :], in_=zbf[:, 0:792])
            nc.scalar.dma_start(out=inp_dram[128:256, :], in_=zbf[:, 0:792])
            nc.scalar.dma_start(out=inp_dram[256:260, :], in_=zbf[0:4, 0:792])

            # ---- persistent xf tile: [128, 274*2] bf16, halves side by side
            xf_pad = pp.tile([128, 2 * XHW], BF16, tag="xf_pad")
            nc.vector.memset(xf_pad, 0.0)

            def _heat(n, tag):
                # keep-warm matmuls into a scratch PSUM bank: fill PE idle
                # gaps so real matmuls dispatch into a warm p-state streak
                with tc.tile_pool(name=f"heat_{tag}", bufs=1,
                                  space="PSUM") as hp:
                    pdum = hp.tile([2, 512], F32, tag=f"pdum{tag}")
                    for _ in range(n):
                        nc.tensor.matmul(pdum, sb["W0"], zbf[:, 0:512],
                                         start=True, stop=True)

            _heat(10, "a")
            # ---- phase B: x cast-load + stage-0 matmul + evac + scatter ----
            NCH = 8                       # chunks of 16 rows
            with (
                tc.tile_pool(name="bpool", bufs=3) as bp,
                tc.tile_pool(name="psum0", bufs=4, space="PSUM") as p0p,
            ):
                for ch in range(NCH):
                    sbx = bp.tile([128, 4096], BF16, tag="sbx", bufs=6)
                    for half in range(2):   # partition = half*64 + c
                        srcp = bass.AP(tensor=xb,
                                       offset=(half * 128 + ch * 16) * W,
                                       ap=[[HW, 64], [1, 4096]])
                        nc.gpsimd.dma_start(out=sbx[ts(half, 64), :], in_=srcp)
                    s0b = bp.tile([2, 4096], BF16, tag="s0b", bufs=3)
                    for q in range(4):
                        p0 = p0p.tile([2, 1024], F32, tag="p0", name="p0t")
                        for j in range(2):
                            nc.tensor.matmul(
                                p0[:, ts(j, 512)],
                                sb["W0"],
                                sbx[:, ds(q * 1024 + j * 512, 512)],
                                start=True, stop=True)
                        # evac PSUM -> bf16 staging with b_icfd bias fused
                        if (ch * 4 + q) % 2 == 0:
                            nc.scalar.activation(
                                out=s0b[:, ts(q, 1024)], in_=p0,
                                func=mybir.ActivationFunctionType.Identity,
                                bias=bic2[:, 0:1], scale=1.0)
                        else:
                            nc.vector.tensor_scalar_add(
                                out=s0b[:, ts(q, 1024)], in0=p0,
                                scalar1=bic2[:, 0:1])
                    for half in range(2):   # scatter 16 rows into xf_pad
                        nc.sync.dma_start(
                            out=xf_pad[ch * 16:ch * 16 + 16,
                                       ds(half * XHW + R, 256)],
                            in_=s0b[half:half + 1, :])

            # ---- phase C: column sums -> mean ----
            colsums = pp.tile([128, 1], F32, tag="colsums")
            nc.vector.tensor_reduce(out=colsums,
                                    in_=xf_pad[:, 0:2 * XHW],
                                    axis=mybir.AxisListType.X,
                                    op=mybir.AluOpType.add)
            with tc.tile_pool(name="psA", bufs=1, space="PSUM") as psA:
                pm = psA.tile([1, 1], F32, tag="pm")
                nc.tensor.matmul(pm, colsums, sb["ONESC"], start=True, stop=True)
                ts2 = pp.tile([1, 1], F32, tag="ts2")
                nc.scalar.copy(out=ts2, in_=pm)
                pmb = psA.tile([128, 1], F32, tag="pmb")
                nc.tensor.matmul(pmb, sb["ONES2"][0:1, :], ts2,
                                 start=True, stop=True)
                mean_bc = pp.tile([128, 1], F32, tag="mean_bc")
                nc.scalar.activation(out=mean_bc, in_=pmb,
                                     func=mybir.ActivationFunctionType.Copy,
                                     scale=1.0 / HW)

                # ---- phase D: offsets, hats, K matmul ----
                dyx = pp.tile([128, 26], F32, tag="dyx")
                nc.vector.tensor_scalar_mul(out=dyx, in0=sb["WOFF"],
                                            scalar1=mean_bc[:, 0:1])
                nc.vector.tensor_add(out=dyx, in0=dyx, in1=sb["BOFF"])
                HH = pp.tile([128, 26 * NT], F32, tag="HH")
                HH3 = HH[:].rearrange("p (a b) -> p a b", a=26)
                nc.vector.tensor_tensor(
                    out=HH3[:, 0:13, :],
                    in0=dyx[:, 0:13].unsqueeze(2).to_broadcast([128, 13, NT]),
                    in1=sb["TAPSF"][:].unsqueeze(1).to_broadcast([128, 13, NT]),
                    op=mybir.AluOpType.subtract)
                nc.vector.tensor_tensor(
                    out=HH3[:, 13:26, :],
                    in0=dyx[:, 13:26].unsqueeze(2).to_broadcast([128, 13, NT]),
                    in1=sb["TAPSR"][:].unsqueeze(1).to_broadcast([128, 13, NT]),
                    op=mybir.AluOpType.subtract)
                nc.vector.tensor_scalar(out=HH, in0=HH, scalar1=0.0,
                                        scalar2=None,
                                        op0=mybir.AluOpType.abs_max)
                nc.scalar.activation(out=HH, in_=HH,
                                     func=mybir.ActivationFunctionType.Relu,
                                     scale=-1.0, bias=1.0)
                WHY = pp.tile([128, 13 * NT], F32, tag="WHY")
                nc.vector.tensor_tensor(
                    out=WHY[:].rearrange("p (a b) -> p a b", a=13),
                    in0=HH3[:, 0:13, :],
                    in1=sb["WF"][:].unsqueeze(2).to_broadcast([128, 13, NT]),
                    op=mybir.AluOpType.mult)
                WHY3 = WHY[:].rearrange("p (a b) -> p a b", a=13)
                pK = psA.tile([NT, NT], F32, tag="pK")
                for c in range(13):
                    nc.tensor.matmul(pK, WHY3[:, c, :], HH3[:, 13 + c, :],
                                     start=(c == 0), stop=(c == 12))
                Ksb = pp.tile([NT, NT], BF16, tag="Ksb")
                nc.scalar.copy(out=Ksb, in_=pK)

            # ---- phase E: reversed K write + contiguous staircase T tables
            # K_dram[C0 - d] = Ksb[d]  (bf16 cast during SWDGE write)
            nc.sync.dma_start(
                out=bass.AP(tensor=K_dram, offset=C0 * NT,
                            ap=[[-NT, NT], [1, NT]]),
                in_=Ksb)
            T_A = pp.tile([128, 128 * NT], BF16, tag="T_A")
            T_B = pp.tile([9, 128 * NT], BF16, tag="T_B")
            T_C = pp.tile([9, 128 * NT], BF16, tag="T_C")
            # T_A[p, a, e] = K[9+p-a] = K_dram[C0-9-p+a]
            nc.sync.dma_start(
                out=T_A[:].rearrange("p (a b) -> p a b", a=128),
                in_=bass.AP(tensor=K_dram, offset=(C0 - 9) * NT,
                            ap=[[-NT, 128], [NT, 128], [1, NT]]))
            # T_B[p, a, e] = K[p-a] = K_dram[C0-p+a]
            nc.gpsimd.dma_start(
                out=T_B[:].rearrange("p (a b) -> p a b", a=128),
                in_=bass.AP(tensor=K_dram, offset=C0 * NT,
                            ap=[[-NT, 9], [NT, 128], [1, NT]]))
            # T_C[p, a, e] = K[137+p-a] = K_dram[C0-137-p+a]
            nc.gpsimd.dma_start(
                out=T_C[:].rearrange("p (a b) -> p a b", a=128),
                in_=bass.AP(tensor=K_dram, offset=(C0 - 137) * NT,
                            ap=[[-NT, 9], [NT, 128], [1, NT]]))
            T_A3 = T_A[:].rearrange("p (a b) -> p a b", a=128)
            T_B3 = T_B[:].rearrange("p (a b) -> p a b", a=128)
            T_C3 = T_C[:].rearrange("p (a b) -> p a b", a=128)
            # matmul operands must start at partition 0/32/64: copy the 9
            # boundary rows of half 0 (119..127) into a base-0 tile
            xf_b0 = pp.tile([9, XHW], BF16, tag="xf_b0")
            nc.sync.dma_start(out=xf_b0, in_=xf_pad[119:128, 0:XHW])

            _heat(26, "b")
            # ---- phase F: stage-1 Toeplitz matmuls -> inp_dram ----
            with tc.tile_pool(name="psum1", bufs=2, space="PSUM") as p1p:
                for t in range(2):
                    pinp = p1p.tile([128, W], F32, tag="pinp")
                    nmm = NT * 2 + 2
                    i = 0
                    for kxp in range(NT):
                        sl = 18 - kxp
                        nc.tensor.matmul(pinp, T_A3[:, :, kxp],
                                         xf_pad[:, ds(t * XHW + sl, W)],
                                         start=(i == 0), stop=(i == nmm - 1)); i += 1
                        if t == 0:
                            nc.tensor.matmul(pinp, T_C3[0:9, :, kxp],
                                             xf_pad[0:9, ds(XHW + sl, W)],
                                             start=False, stop=(i == nmm - 1)); i += 1
                        else:
                            nc.tensor.matmul(pinp, T_B3[0:9, :, kxp],
                                             xf_b0[:, ds(sl, W)],
                                             start=False, stop=(i == nmm - 1)); i += 1
                    nc.tensor.matmul(pinp, sb["I128"],
                                     xf_pad[:, ds(t * XHW + R, W)],
                                     start=False, stop=False); i += 1
                    nc.tensor.matmul(pinp, sb["CVEC"], sb["ONESR"][0:1, 0:W],
                                     start=False, stop=True); i += 1
                    s1 = pp.tile([128, W], BF16, tag=f"s1_{t}", name=f"s1stage{t}")
                    nc.vector.tensor_copy(out=s1, in_=pinp)
                    # inp3[r, j, e] = inp_padded[r, j + e]; s1 covers padded
                    # cols 1..256 of rows 1+128t..128+128t
                    for j in range(3):
                        dst = bass.AP(
                            tensor=inp_dram,
                            offset=(1 + 128 * t) * 792 + j * 264 + (1 - j),
                            ap=[[792, 128], [1, W]])
                        eng = (nc.sync, nc.gpsimd)[t]
                        eng.dma_start(out=dst, in_=s1)

            _heat(30, "c")
            # ---- phase G: im2col + stage-2 + store ----
            with (
                tc.tile_pool(name="gpool", bufs=3) as gp,
                tc.tile_pool(name="psum2", bufs=4, space="PSUM") as p2p,
            ):
                for ch in range(8):               # h2-chunks of 16
                    im = gp.tile([18, 4096], BF16, tag="im", bufs=6)
                    for g in range(2):
                        srcp = bass.AP(
                            tensor=inp_dram,
                            offset=(g * 128 + ch * 16) * 792,
                            ap=[[264, 9], [792, 16], [1, W]])
                        nc.sync.dma_start(
                            out=im[ts(g, 9), :].rearrange(
                                "p (d e) -> p d e", d=16),
                            in_=srcp)
                    for pair in range(2):         # 8 rows per store
                        ysb = gp.tile([128, 2048], BF16, tag="ysb",
                                      name="ystage", bufs=4)
                        py = p2p.tile([128, 2048], F32, tag="py", bufs=2)
                        for j in range(4):
                            nc.tensor.matmul(
                                py[:, ts(j, 512)], sb["W2"],
                                im[:, ds(pair * 2048 + j * 512, 512)],
                                start=True, stop=True)
                        # single evac per store unit (+ b_conv bias, bf16)
                        u = ch * 2 + pair
                        if (u + u // 2) % 2 == 0:
                            nc.scalar.activation(
                                out=ysb, in_=py,
                                func=mybir.ActivationFunctionType.Identity,
                                bias=sb["BC"][:, 0:1], scale=1.0)
                        else:
                            nc.vector.tensor_scalar_add(
                                out=ysb, in0=py, scalar1=sb["BC"][:, 0:1])
                        dst = bass.AP(
                            tensor=y,
                            offset=(ch * 16 + pair * 8) * W,
                            ap=[[128 * W, 2], [HW, 64], [1, 2048]])
                        nc.gpsimd.dma_start(out=dst, in_=ysb[:])
    with tile.TileContext(nc) as tc:
        _graph(tc)
    nc.finalize()
    return nc


def kernel(**inputs):
    x = np.ascontiguousarray(inputs["x"], dtype=np.float32)
    params = {k: np.asarray(v) for k, v in inputs.items() if k != "x"}
    nc = build(params, num_devices=8)
    from concourse.bass_utils import run_bass_kernel_spmd
    in_maps = [{"xb": np.ascontiguousarray(x[b])} for b in range(B)]
    res = run_bass_kernel_spmd(nc, in_maps, core_ids=list(range(B)))
    return np.stack([np.asarray(res.results[b]["y"], dtype=np.float32)
                     for b in range(B)])


# revision 21
# speedup vs baseline: 1.2406x; 1.0115x over previous
"""Trainium2 Bass kernel for nn_DeformableConvLayer.

Math (validated vs reference in numpy):
  xf   = sum_c w_icfd[c] * x[:, c] + b_icfd                       (B,H,W)
  mean = mean(xf, (h,w));  dy/dx = mean*w_off + b_off             (per b, 1600 stencils)
  The whole translate+fuse stage is a dense 19x19 conv with a data-dependent
  per-b kernel K_b[ky,kx] = sum_s w_fus[g_s]*hat(dy_s-ky)*hat(dx_s-kx),
  hat(t) = max(0, 1-|t|)  (bilinear weights == hat at integer taps).
  inp  = conv2d(xf, K_b, zero-pad) + 64*b_fus + xf
  y    = conv2d(inp, w_conv 3x3, zero-pad) + b_conv               (B,64,H,W)

Sharding: data-parallel, one batch element per NeuronCore (B=8, 8 cores).
Wide data paths run in bf16: x is cast to bf16 during the SWDGE load DMA,
all conv matmuls are bf16 (fp32 PSUM accumulation), and y is stored as bf16
and widened to fp32 on the host.  Stage-1 runs as Toeplitz-banded matmuls on
the tensor engine; the banded lhsT tables are materialized from K_b via
row-reversed staircase reads of a padded DRAM buffer (contiguous 4.8KB
descriptors).  xf stays on-chip: stage-0 PSUM is evacuated (bias fused) to a
staging tile and scattered to the padded xf tile by SBUF->SBUF DMA.
"""
import numpy as np
import ml_dtypes

import concourse.bacc as bacc
import concourse.bass as bass
import concourse.tile as tile
from concourse import mybir
from concourse.bass import ds, ts

F32 = mybir.dt.float32
BF16 = mybir.dt.bfloat16
NPBF = np.dtype(ml_dtypes.bfloat16)

B, C, H, W = 8, 64, 256, 256
G, DFC = 25, 64
R = 9
NT = 2 * R + 1            # 19 taps
C0 = 145                  # reversed-K row anchor in K_dram
HW = H * W
XHW = 274                 # xf_pad per-half width: 9 | 256 | 9


def _consts(params):
    """Host-side constant tensors derived from the (small) param inputs."""
    w_icfd = params["w_icfd"].astype(np.float32)
    w_off = params["w_off"].astype(np.float32)
    b_off = params["b_off"].astype(np.float32)
    w_fus = params["w_fus"].astype(np.float32)
    b_fus = float(params["b_fus"])
    w_conv = params["w_conv"].astype(np.float32)
    b_conv = params["b_conv"].astype(np.float32)

    W0 = np.zeros((128, 2), np.float32)
    for half in range(2):
        W0[half * 64:(half + 1) * 64, half] = w_icfd

    W2 = np.zeros((18, 128), np.float32)
    for g in range(2):
        for ky2 in range(3):
            for kx2 in range(3):
                W2[g * 9 + ky2 * 3 + kx2, g * 64:(g + 1) * 64] = w_conv[:, 0, ky2, kx2]
    BC = np.zeros((128, 1), np.float32)
    BC[0:64, 0] = b_conv
    BC[64:128, 0] = b_conv

    taps_rev = (R - np.arange(NT)).astype(np.float32)     # [9, 8, ..., -9]
    taps_fwd = (np.arange(NT) - R).astype(np.float32)     # [-9, ..., 9]
    TAPSF = np.tile(taps_fwd[None, :], (128, 1))
    TAPSR = np.tile(taps_rev[None, :], (128, 1))

    # s-chunk layout: s = c*128 + p, 13 chunks; tail (s>=1600) padded with zeros
    WF = np.zeros((128, 13), np.float32)
    WOFF = np.zeros((128, 26), np.float32)    # cols 0..12 y, 13..25 x
    BOFF = np.zeros((128, 26), np.float32)
    for c in range(13):
        for p in range(128):
            s = c * 128 + p
            if s < 1600:
                WF[p, c] = w_fus[s // 64]
                WOFF[p, c] = w_off[2 * s]
                BOFF[p, c] = b_off[2 * s]
                WOFF[p, 13 + c] = w_off[2 * s + 1]
                BOFF[p, 13 + c] = b_off[2 * s + 1]

    C_total = DFC * b_fus
    return dict(
        W0=W0.astype(NPBF), W2=W2.astype(NPBF), BC=BC,
        TAPSF=TAPSF, TAPSR=TAPSR, WF=WF, WOFF=WOFF, BOFF=BOFF,
        I128=np.eye(128, dtype=np.float32).astype(NPBF),
        ONESR=np.ones((1, 256), np.float32),
        ONESC=np.ones((128, 1), np.float32),
        ONES2=np.ones((2, 128), np.float32),
        CVEC=np.full((1, 128), C_total, np.float32),
        b_icfd=float(params["b_icfd"]),
    )


def build(params, num_devices=8):
    cs = _consts(params)
    nc = bacc.Bacc("TRN2", target_bir_lowering=False, debug=False,
                   num_devices=num_devices)
    xb = nc.dram_tensor("xb", [C, H, W], F32, kind="ExternalInput")
    y = nc.dram_tensor("y", [64, H, W], BF16, kind="ExternalOutput")
    K_dram = nc.dram_tensor("k_scr", [280, NT], BF16, kind="Internal")
    inp_dram = nc.dram_tensor("inp_scr", [260, 792], BF16, kind="Internal")

    ct = {k: nc.inline_tensor(v, name=f"c_{k}") for k, v in cs.items()
          if isinstance(v, np.ndarray)}
    b_icfd = cs["b_icfd"]

    def _graph(tc):
        with (
            tc.tile_pool(name="consts", bufs=1) as cp,
            tc.tile_pool(name="persist", bufs=1) as pp,
        ):
            # ---- load constants ----
            sb = {}
            for i, k in enumerate(("W0", "W2", "BC", "TAPSF", "TAPSR", "WF",
                                   "WOFF", "BOFF", "I128", "ONESR", "ONESC",
                                   "ONES2", "CVEC")):
                dt = BF16 if cs[k].dtype == NPBF else F32
                t = cp.tile(list(cs[k].shape), dt, tag=k, name=f"sb_{k}")
                eng = (nc.sync, nc.scalar)[i % 2]
                eng.dma_start(out=t, in_=ct[k][:, :])
                sb[k] = t
            zbf = cp.tile([128, 792], BF16, tag="zbf")
            nc.vector.memset(zbf, 0.0)
            bic2 = cp.tile([2, 1], F32, tag="bic2")
            nc.vector.memset(bic2, b_icfd)

            # ---- zero scratch DRAM (early, off critical path) ----
            nc.sync.dma_start(
                out=bass.AP(tensor=K_dram, offset=0, ap=[[NT, 128], [1, NT]]),
                in_=zbf[:, 0:NT])
            nc.sync.dma_start(
                out=bass.AP(tensor=K_dram, offset=128 * NT,
                            ap=[[NT, 128], [1, NT]]),
                in_=zbf[:, 0:NT])
            nc.sync.dma_start(
                out=bass.AP(tensor=K_dram, offset=256 * NT,
                            ap=[[NT, 24], [1, NT]]),
                in_=zbf[0:24, 0:NT])
            nc.scalar.dma_start(out=inp_dram[0:128, :], in_=zbf[:, 0:792])
            nc.scalar.dma_start(out=inp_dram[128:256, :], in_=zbf[:, 0:792])
            nc.scalar.dma_start(out=inp_dram[256:260, :], in_=zbf[0:4, 0:792])

            # ---- persistent xf tile: [128, 274*2] bf16, halves side by side
            xf_pad = pp.tile([128, 2 * XHW], BF16, tag="xf_pad")
            nc.vector.memset(xf_pad, 0.0)

            def _heat(n, tag):
                # keep-warm matmuls into a scratch PSUM bank: fill PE idle
                # gaps so real matmuls dispatch into a warm p-state streak
                with tc.tile_pool(name=f"heat_{tag}", bufs=1,
                                  space="PSUM") as hp:
                    pdum = hp.tile([2, 512], F32, tag=f"pdum{tag}")
                    for _ in range(n):
                        nc.tensor.matmul(pdum, sb["W0"], zbf[:, 0:512],
                                         start=True, stop=True)

            _heat(10, "a")
            # ---- phase B: x cast-load + stage-0 matmul + evac + scatter ----
            NCH = 8                       # chunks of 16 rows
            with (
                tc.tile_pool(name="bpool", bufs=3) as bp,
                tc.tile_pool(name="psum0", bufs=4, space="PSUM") as p0p,
            ):
                for ch in range(NCH):
                    sbx = bp.tile([128, 4096], BF16, tag="sbx", bufs=6)
                    for half in range(2):   # partition = half*64 + c
                        srcp = bass.AP(tensor=xb,
                                       offset=(half * 128 + ch * 16) * W,
                                       ap=[[HW, 64], [1, 4096]])
                        nc.gpsimd.dma_start(out=sbx[ts(half, 64), :], in_=srcp)
                    s0b = bp.tile([2, 4096], BF16, tag="s0b", bufs=3)
                    for q in range(4):
                        p0 = p0p.tile([2, 1024], F32, tag="p0", name="p0t")
                        for j in range(2):
                            nc.tensor.matmul(
                                p0[:, ts(j, 512)],
                                sb["W0"],
                                sbx[:, ds(q * 1024 + j * 512, 512)],
                                start=True, stop=True)
                        # evac PSUM -> bf16 staging with b_icfd bias fused
                        if (ch * 4 + q) % 2 == 0:
                            nc.scalar.activation(
                                out=s0b[:, ts(q, 1024)], in_=p0,
                                func=mybir.ActivationFunctionType.Identity,
                                bias=bic2[:, 0:1], scale=1.0)
                        else:
                            nc.vector.tensor_scalar_add(
                                out=s0b[:, ts(q, 1024)], in0=p0,
                                scalar1=bic2[:, 0:1])
                    for half in range(2):   # scatter 16 rows into xf_pad
                        nc.sync.dma_start(
                            out=xf_pad[ch * 16:ch * 16 + 16,
                                       ds(half * XHW + R, 256)],
                            in_=s0b[half:half + 1, :])

            # ---- phase C: column sums -> mean ----
            colsums = pp.tile([128, 1], F32, tag="colsums")
            nc.vector.tensor_reduce(out=colsums,
                                    in_=xf_pad[:, 0:2 * XHW],
                                    axis=mybir.AxisListType.X,
                                    op=mybir.AluOpType.add)
            with tc.tile_pool(name="psA", bufs=1, space="PSUM") as psA:
                pm = psA.tile([1, 1], F32, tag="pm")
                nc.tensor.matmul(pm, colsums, sb["ONESC"], start=True, stop=True)
                ts2 = pp.tile([1, 1], F32, tag="ts2")
                nc.scalar.copy(out=ts2, in_=pm)
                pmb = psA.tile([128, 1], F32, tag="pmb")
                nc.tensor.matmul(pmb, sb["ONES2"][0:1, :], ts2,
                                 start=True, stop=True)
                mean_bc = pp.tile([128, 1], F32, tag="mean_bc")
                nc.scalar.activation(out=mean_bc, in_=pmb,
                                     func=mybir.ActivationFunctionType.Copy,
                                     scale=1.0 / HW)

                # ---- phase D: offsets, hats, K matmul ----
                dyx = pp.tile([128, 26], F32, tag="dyx")
                nc.vector.tensor_scalar_mul(out=dyx, in0=sb["WOFF"],
                                            scalar1=mean_bc[:, 0:1])
                nc.vector.tensor_add(out=dyx, in0=dyx, in1=sb["BOFF"])
                HH = pp.tile([128, 26 * NT], F32, tag="HH")
                HH3 = HH[:].rearrange("p (a b) -> p a b", a=26)
                nc.vector.tensor_tensor(
                    out=HH3[:, 0:13, :],
                    in0=dyx[:, 0:13].unsqueeze(2).to_broadcast([128, 13, NT]),
                    in1=sb["TAPSF"][:].unsqueeze(1).to_broadcast([128, 13, NT]),
                    op=mybir.AluOpType.subtract)
                nc.vector.tensor_tensor(
                    out=HH3[:, 13:26, :],
                    in0=dyx[:, 13:26].unsqueeze(2).to_broadcast([128, 13, NT]),
                    in1=sb["TAPSR"][:].unsqueeze(1).to_broadcast([128, 13, NT]),
                    op=mybir.AluOpType.subtract)
                nc.vector.tensor_scalar(out=HH, in0=HH, scalar1=0.0,
                                        scalar2=None,
                                        op0=mybir.AluOpType.abs_max)
                nc.scalar.activation(out=HH, in_=HH,
                                     func=mybir.ActivationFunctionType.Relu,
                                     scale=-1.0, bias=1.0)
                WHY = pp.tile([128, 13 * NT], F32, tag="WHY")
                nc.vector.tensor_tensor(
                    out=WHY[:].rearrange("p (a b) -> p a b", a=13),
                    in0=HH3[:, 0:13, :],
                    in1=sb["WF"][:].unsqueeze(2).to_broadcast([128, 13, NT]),
                    op=mybir.AluOpType.mult)
                WHY3 = WHY[:].rearrange("p (a b) -> p a b", a=13)
                pK = psA.tile([NT, NT], F32, tag="pK")
                for c in range(13):
                    nc.tensor.matmul(pK, WHY3[:, c, :], HH3[:, 13 + c, :],
                                     start=(c == 0), stop=(c == 12))
                Ksb = pp.tile([NT, NT], BF16, tag="Ksb")
                nc.scalar.copy(out=Ksb, in_=pK)

            # ---- phase E: reversed K write + contiguous staircase T tables
            # K_dram[C0 - d] = Ksb[d]  (bf16 cast during SWDGE write)
            nc.sync.dma_start(
                out=bass.AP(tensor=K_dram, offset=C0 * NT,
                            ap=[[-NT, NT], [1, NT]]),
                in_=Ksb)
            T_A = pp.tile([128, 128 * NT], BF16, tag="T_A")
            T_B = pp.tile([9, 128 * NT], BF16, tag="T_B")
            T_C = pp.tile([9, 128 * NT], BF16, tag="T_C")
            # T_A[p, a, e] = K[9+p-a] = K_dram[C0-9-p+a]
            nc.sync.dma_start(
                out=T_A[:].rearrange("p (a b) -> p a b", a=128),
                in_=bass.AP(tensor=K_dram, offset=(C0 - 9) * NT,
                            ap=[[-NT, 128], [NT, 128], [1, NT]]))
            # T_B[p, a, e] = K[p-a] = K_dram[C0-p+a]
            nc.gpsimd.dma_start(
                out=T_B[:].rearrange("p (a b) -> p a b", a=128),
                in_=bass.AP(tensor=K_dram, offset=C0 * NT,
                            ap=[[-NT, 9], [NT, 128], [1, NT]]))
            # T_C[p, a, e] = K[137+p-a] = K_dram[C0-137-p+a]
            nc.gpsimd.dma_start(
                out=T_C[:].rearrange("p (a b) -> p a b", a=128),
                in_=bass.AP(tensor=K_dram, offset=(C0 - 137) * NT,
                            ap=[[-NT, 9], [NT, 128], [1, NT]]))
            T_A3 = T_A[:].rearrange("p (a b) -> p a b", a=128)
            T_B3 = T_B[:].rearrange("p (a b) -> p a b", a=128)
            T_C3 = T_C[:].rearrange("p (a b) -> p a b", a=128)
            # matmul operands must start at partition 0/32/64: copy the 9
            # boundary rows of half 0 (119..127) into a base-0 tile
            xf_b0 = pp.tile([9, XHW], BF16, tag="xf_b0")
            nc.sync.dma_start(out=xf_b0, in_=xf_pad[119:128, 0:XHW])

            _heat(26, "b")
            # ---- phase F: stage-1 Toeplitz matmuls -> inp_dram ----
            with tc.tile_pool(name="psum1", bufs=2, space="PSUM") as p1p:
                for t in range(2):
                    pinp = p1p.tile([128, W], F32, tag="pinp")
                    nmm = NT * 2 + 2
                    i = 0
                    for kxp in range(NT):
                        sl = 18 - kxp
                        nc.tensor.matmul(pinp, T_A3[:, :, kxp],
                                         xf_pad[:, ds(t * XHW + sl, W)],
                                         start=(i == 0), stop=(i == nmm - 1)); i += 1
                        if t == 0:
                            nc.tensor.matmul(pinp, T_C3[0:9, :, kxp],
                                             xf_pad[0:9, ds(XHW + sl, W)],
                                             start=False, stop=(i == nmm - 1)); i += 1
                        else:
                            nc.tensor.matmul(pinp, T_B3[0:9, :, kxp],
                                             xf_b0[:, ds(sl, W)],
                                             start=False, stop=(i == nmm - 1)); i += 1
                    nc.tensor.matmul(pinp, sb["I128"],
                                     xf_pad[:, ds(t * XHW + R, W)],
                                     start=False, stop=False); i += 1
                    nc.tensor.matmul(pinp, sb["CVEC"], sb["ONESR"][0:1, 0:W],
                                     start=False, stop=True); i += 1
                    s1 = pp.tile([128, W], BF16, tag=f"s1_{t}", name=f"s1stage{t}")
                    nc.vector.tensor_copy(out=s1, in_=pinp)
                    # inp3[r, j, e] = inp_padded[r, j + e]; s1 covers padded
                    # cols 1..256 of rows 1+128t..128+128t
                    for j in range(3):
                        dst = bass.AP(
                            tensor=inp_dram,
                            offset=(1 + 128 * t) * 792 + j * 264 + (1 - j),
                            ap=[[792, 128], [1, W]])
                        eng = (nc.sync, nc.gpsimd)[t]
                        eng.dma_start(out=dst, in_=s1)

            _heat(30, "c")
            # ---- phase G: im2col + stage-2 + store ----
            with (
                tc.tile_pool(name="gpool", bufs=3) as gp,
                tc.tile_pool(name="psum2", bufs=4, space="PSUM") as p2p,
            ):
                for ch in range(8):               # h2-chunks of 16
                    im = gp.tile([18, 4096], BF16, tag="im", bufs=8)
                    for g in range(2):
                        srcp = bass.AP(
                            tensor=inp_dram,
                            offset=(g * 128 + ch * 16) * 792,
                            ap=[[264, 9], [792, 16], [1, W]])
                        nc.sync.dma_start(
                            out=im[ts(g, 9), :].rearrange(
                                "p (d e) -> p d e", d=16),
                            in_=srcp)
                    for pair in range(2):         # 8 rows per store
                        ysb = gp.tile([128, 2048], BF16, tag="ysb",
                                      name="ystage", bufs=6)
                        py = p2p.tile([128, 2048], F32, tag="py", bufs=2)
                        for j in range(4):
                            nc.tensor.matmul(
                                py[:, ts(j, 512)], sb["W2"],
                                im[:, ds(pair * 2048 + j * 512, 512)],
                                start=True, stop=True)
                        # single evac per store unit (+ b_conv bias, bf16)
                        u = ch * 2 + pair
                        if (u + u // 2) % 2 == 0:
                            nc.scalar.activation(
                                out=ysb, in_=py,
                                func=mybir.ActivationFunctionType.Identity,
                                bias=sb["BC"][:, 0:1], scale=1.0)
                        else:
                            nc.vector.tensor_scalar_add(
                                out=ysb, in0=py, scalar1=sb["BC"][:, 0:1])
                        dst = bass.AP(
                            tensor=y,
                            offset=(ch * 16 + pair * 8) * W,
                            ap=[[128 * W, 2], [HW, 64], [1, 2048]])
                        nc.gpsimd.dma_start(out=dst, in_=ysb[:])
    with tile.TileContext(nc) as tc:
        _graph(tc)
    nc.finalize()
    return nc


def kernel(**inputs):
    x = np.ascontiguousarray(inputs["x"], dtype=np.float32)
    params = {k: np.asarray(v) for k, v in inputs.items() if k != "x"}
    nc = build(params, num_devices=8)
    from concourse.bass_utils import run_bass_kernel_spmd
    in_maps = [{"xb": np.ascontiguousarray(x[b])} for b in range(B)]
    res = run_bass_kernel_spmd(nc, in_maps, core_ids=list(range(B)))
    return np.stack([np.asarray(res.results[b]["y"], dtype=np.float32)
                     for b in range(B)])


# revision 22
# speedup vs baseline: 1.3131x; 1.0585x over previous
"""Trainium2 Bass kernel for nn_DeformableConvLayer.

Math (validated vs reference in numpy):
  xf   = sum_c w_icfd[c] * x[:, c] + b_icfd                       (B,H,W)
  mean = mean(xf, (h,w));  dy/dx = mean*w_off + b_off             (per b, 1600 stencils)
  The whole translate+fuse stage is a dense 19x19 conv with a data-dependent
  per-b kernel K_b[ky,kx] = sum_s w_fus[g_s]*hat(dy_s-ky)*hat(dx_s-kx),
  hat(t) = max(0, 1-|t|)  (bilinear weights == hat at integer taps).
  inp  = conv2d(xf, K_b, zero-pad) + 64*b_fus + xf
  y    = conv2d(inp, w_conv 3x3, zero-pad) + b_conv               (B,64,H,W)

Sharding: data-parallel, one batch element per NeuronCore (B=8, 8 cores).
Wide data paths run in bf16: x is cast to bf16 during the SWDGE load DMA,
all conv matmuls are bf16 (fp32 PSUM accumulation), and y is stored as bf16
and widened to fp32 on the host.  Stage-1 runs as Toeplitz-banded matmuls on
the tensor engine; the banded lhsT tables are materialized from K_b via
row-reversed staircase reads of a padded DRAM buffer (contiguous 4.8KB
descriptors).  xf stays on-chip: stage-0 PSUM is evacuated (bias fused) to a
staging tile and scattered to the padded xf tile by SBUF->SBUF DMA.
"""
import numpy as np
import ml_dtypes

import concourse.bacc as bacc
import concourse.bass as bass
import concourse.tile as tile
from concourse import mybir
from concourse.bass import ds, ts

F32 = mybir.dt.float32
BF16 = mybir.dt.bfloat16
NPBF = np.dtype(ml_dtypes.bfloat16)

B, C, H, W = 8, 64, 256, 256
G, DFC = 25, 64
R = 9
NT = 2 * R + 1            # 19 taps
C0 = 145                  # reversed-K row anchor in K_dram
HW = H * W
XHW = 274                 # xf_pad per-half width: 9 | 256 | 9


def _consts(params):
    """Host-side constant tensors derived from the (small) param inputs."""
    w_icfd = params["w_icfd"].astype(np.float32)
    w_off = params["w_off"].astype(np.float32)
    b_off = params["b_off"].astype(np.float32)
    w_fus = params["w_fus"].astype(np.float32)
    b_fus = float(params["b_fus"])
    w_conv = params["w_conv"].astype(np.float32)
    b_conv = params["b_conv"].astype(np.float32)

    W0 = np.zeros((128, 2), np.float32)
    for half in range(2):
        W0[half * 64:(half + 1) * 64, half] = w_icfd

    W2 = np.zeros((18, 128), np.float32)
    for g in range(2):
        for ky2 in range(3):
            for kx2 in range(3):
                W2[g * 9 + ky2 * 3 + kx2, g * 64:(g + 1) * 64] = w_conv[:, 0, ky2, kx2]
    BC = np.zeros((128, 1), np.float32)
    BC[0:64, 0] = b_conv
    BC[64:128, 0] = b_conv

    taps_rev = (R - np.arange(NT)).astype(np.float32)     # [9, 8, ..., -9]
    taps_fwd = (np.arange(NT) - R).astype(np.float32)     # [-9, ..., 9]
    TAPSF = np.tile(taps_fwd[None, :], (128, 1))
    TAPSR = np.tile(taps_rev[None, :], (128, 1))

    # s-chunk layout: s = c*128 + p, 13 chunks; tail (s>=1600) padded with zeros
    WF = np.zeros((128, 13), np.float32)
    WOFF = np.zeros((128, 26), np.float32)    # cols 0..12 y, 13..25 x
    BOFF = np.zeros((128, 26), np.float32)
    for c in range(13):
        for p in range(128):
            s = c * 128 + p
            if s < 1600:
                WF[p, c] = w_fus[s // 64]
                WOFF[p, c] = w_off[2 * s]
                BOFF[p, c] = b_off[2 * s]
                WOFF[p, 13 + c] = w_off[2 * s + 1]
                BOFF[p, 13 + c] = b_off[2 * s + 1]

    C_total = DFC * b_fus
    return dict(
        W0=W0.astype(NPBF), W2=W2.astype(NPBF), BC=BC,
        TAPSF=TAPSF, TAPSR=TAPSR, WF=WF, WOFF=WOFF, BOFF=BOFF,
        I128=np.eye(128, dtype=np.float32).astype(NPBF),
        ONESR=np.ones((1, 256), np.float32),
        ONESC=np.ones((128, 1), np.float32),
        ONES2=np.ones((2, 128), np.float32),
        CVEC=np.full((1, 128), C_total, np.float32),
        b_icfd=float(params["b_icfd"]),
    )


def build(params, num_devices=8):
    cs = _consts(params)
    nc = bacc.Bacc("TRN2", target_bir_lowering=False, debug=False,
                   num_devices=num_devices)
    xb = nc.dram_tensor("xb", [C, H, W], F32, kind="ExternalInput")
    y = nc.dram_tensor("y", [64, H, W], BF16, kind="ExternalOutput")
    K_dram = nc.dram_tensor("k_scr", [280, NT], BF16, kind="Internal")
    inp_dram = nc.dram_tensor("inp_scr", [260, 792], BF16, kind="Internal")

    ct = {k: nc.inline_tensor(v, name=f"c_{k}") for k, v in cs.items()
          if isinstance(v, np.ndarray)}
    b_icfd = cs["b_icfd"]

    def _graph(tc):
        with (
            tc.tile_pool(name="consts", bufs=1) as cp,
            tc.tile_pool(name="persist", bufs=1) as pp,
        ):
            # ---- load constants ----
            sb = {}
            for i, k in enumerate(("W0", "W2", "BC", "TAPSF", "TAPSR", "WF",
                                   "WOFF", "BOFF", "I128", "ONESR", "ONESC",
                                   "ONES2", "CVEC")):
                dt = BF16 if cs[k].dtype == NPBF else F32
                t = cp.tile(list(cs[k].shape), dt, tag=k, name=f"sb_{k}")
                eng = (nc.sync, nc.scalar)[i % 2]
                eng.dma_start(out=t, in_=ct[k][:, :])
                sb[k] = t
            zbf = cp.tile([128, 792], BF16, tag="zbf")
            nc.vector.memset(zbf, 0.0)
            bic2 = cp.tile([2, 1], F32, tag="bic2")
            nc.vector.memset(bic2, b_icfd)

            # ---- zero scratch DRAM (early, off critical path) ----
            nc.sync.dma_start(
                out=bass.AP(tensor=K_dram, offset=0, ap=[[NT, 128], [1, NT]]),
                in_=zbf[:, 0:NT])
            nc.sync.dma_start(
                out=bass.AP(tensor=K_dram, offset=128 * NT,
                            ap=[[NT, 128], [1, NT]]),
                in_=zbf[:, 0:NT])
            nc.sync.dma_start(
                out=bass.AP(tensor=K_dram, offset=256 * NT,
                            ap=[[NT, 24], [1, NT]]),
                in_=zbf[0:24, 0:NT])
            nc.scalar.dma_start(out=inp_dram[0:128, :], in_=zbf[:, 0:792])
            nc.scalar.dma_start(out=inp_dram[128:256, :], in_=zbf[:, 0:792])
            nc.scalar.dma_start(out=inp_dram[256:260, :], in_=zbf[0:4, 0:792])

            # ---- persistent xf tile: [128, 274*2] bf16, halves side by side
            xf_pad = pp.tile([128, 2 * XHW], BF16, tag="xf_pad")
            nc.vector.memset(xf_pad, 0.0)

            def _heat(n, tag):
                # keep-warm matmuls into a scratch PSUM bank: fill PE idle
                # gaps so real matmuls dispatch into a warm p-state streak
                with tc.tile_pool(name=f"heat_{tag}", bufs=1,
                                  space="PSUM") as hp:
                    pdum = hp.tile([2, 512], F32, tag=f"pdum{tag}")
                    for _ in range(n):
                        nc.tensor.matmul(pdum, sb["W0"], zbf[:, 0:512],
                                         start=True, stop=True)

            _heat(10, "a")
            # ---- phase B: x cast-load + stage-0 matmul + evac + scatter ----
            NCH = 8                       # chunks of 16 rows
            with (
                tc.tile_pool(name="bpool", bufs=3) as bp,
                tc.tile_pool(name="psum0", bufs=4, space="PSUM") as p0p,
            ):
                for ch in range(NCH):
                    sbx = bp.tile([128, 4096], BF16, tag="sbx", bufs=6)
                    for half in range(2):   # partition = half*64 + c
                        srcp = bass.AP(tensor=xb,
                                       offset=(half * 128 + ch * 16) * W,
                                       ap=[[HW, 64], [1, 4096]])
                        nc.gpsimd.dma_start(out=sbx[ts(half, 64), :], in_=srcp)
                    s0b = bp.tile([2, 4096], BF16, tag="s0b", bufs=5)
                    for q in range(4):
                        p0 = p0p.tile([2, 1024], F32, tag="p0", name="p0t")
                        for j in range(2):
                            nc.tensor.matmul(
                                p0[:, ts(j, 512)],
                                sb["W0"],
                                sbx[:, ds(q * 1024 + j * 512, 512)],
                                start=True, stop=True)
                        # evac PSUM -> bf16 staging with b_icfd bias fused
                        if (ch * 4 + q) % 2 == 0:
                            nc.scalar.activation(
                                out=s0b[:, ts(q, 1024)], in_=p0,
                                func=mybir.ActivationFunctionType.Identity,
                                bias=bic2[:, 0:1], scale=1.0)
                        else:
                            nc.vector.tensor_scalar_add(
                                out=s0b[:, ts(q, 1024)], in0=p0,
                                scalar1=bic2[:, 0:1])
                    for half in range(2):   # scatter 16 rows into xf_pad
                        nc.sync.dma_start(
                            out=xf_pad[ch * 16:ch * 16 + 16,
                                       ds(half * XHW + R, 256)],
                            in_=s0b[half:half + 1, :])

            # ---- phase C: column sums -> mean ----
            colsums = pp.tile([128, 1], F32, tag="colsums")
            nc.vector.tensor_reduce(out=colsums,
                                    in_=xf_pad[:, 0:2 * XHW],
                                    axis=mybir.AxisListType.X,
                                    op=mybir.AluOpType.add)
            with tc.tile_pool(name="psA", bufs=1, space="PSUM") as psA:
                pm = psA.tile([1, 1], F32, tag="pm")
                nc.tensor.matmul(pm, colsums, sb["ONESC"], start=True, stop=True)
                ts2 = pp.tile([1, 1], F32, tag="ts2")
                nc.scalar.copy(out=ts2, in_=pm)
                pmb = psA.tile([128, 1], F32, tag="pmb")
                nc.tensor.matmul(pmb, sb["ONES2"][0:1, :], ts2,
                                 start=True, stop=True)
                mean_bc = pp.tile([128, 1], F32, tag="mean_bc")
                nc.scalar.activation(out=mean_bc, in_=pmb,
                                     func=mybir.ActivationFunctionType.Copy,
                                     scale=1.0 / HW)

                # ---- phase D: offsets, hats, K matmul ----
                dyx = pp.tile([128, 26], F32, tag="dyx")
                nc.vector.tensor_scalar_mul(out=dyx, in0=sb["WOFF"],
                                            scalar1=mean_bc[:, 0:1])
                nc.vector.tensor_add(out=dyx, in0=dyx, in1=sb["BOFF"])
                HH = pp.tile([128, 26 * NT], F32, tag="HH")
                HH3 = HH[:].rearrange("p (a b) -> p a b", a=26)
                nc.vector.tensor_tensor(
                    out=HH3[:, 0:13, :],
                    in0=dyx[:, 0:13].unsqueeze(2).to_broadcast([128, 13, NT]),
                    in1=sb["TAPSF"][:].unsqueeze(1).to_broadcast([128, 13, NT]),
                    op=mybir.AluOpType.subtract)
                nc.vector.tensor_tensor(
                    out=HH3[:, 13:26, :],
                    in0=dyx[:, 13:26].unsqueeze(2).to_broadcast([128, 13, NT]),
                    in1=sb["TAPSR"][:].unsqueeze(1).to_broadcast([128, 13, NT]),
                    op=mybir.AluOpType.subtract)
                nc.vector.tensor_scalar(out=HH, in0=HH, scalar1=0.0,
                                        scalar2=None,
                                        op0=mybir.AluOpType.abs_max)
                nc.scalar.activation(out=HH, in_=HH,
                                     func=mybir.ActivationFunctionType.Relu,
                                     scale=-1.0, bias=1.0)
                WHY = pp.tile([128, 13 * NT], F32, tag="WHY")
                nc.vector.tensor_tensor(
                    out=WHY[:].rearrange("p (a b) -> p a b", a=13),
                    in0=HH3[:, 0:13, :],
                    in1=sb["WF"][:].unsqueeze(2).to_broadcast([128, 13, NT]),
                    op=mybir.AluOpType.mult)
                WHY3 = WHY[:].rearrange("p (a b) -> p a b", a=13)
                pK = psA.tile([NT, NT], F32, tag="pK")
                for c in range(13):
                    nc.tensor.matmul(pK, WHY3[:, c, :], HH3[:, 13 + c, :],
                                     start=(c == 0), stop=(c == 12))
                Ksb = pp.tile([NT, NT], BF16, tag="Ksb")
                nc.scalar.copy(out=Ksb, in_=pK)

            # ---- phase E: reversed K write + contiguous staircase T tables
            # K_dram[C0 - d] = Ksb[d]  (bf16 cast during SWDGE write)
            nc.sync.dma_start(
                out=bass.AP(tensor=K_dram, offset=C0 * NT,
                            ap=[[-NT, NT], [1, NT]]),
                in_=Ksb)
            T_A = pp.tile([128, 128 * NT], BF16, tag="T_A")
            T_B = pp.tile([9, 128 * NT], BF16, tag="T_B")
            T_C = pp.tile([9, 128 * NT], BF16, tag="T_C")
            # T_A[p, a, e] = K[9+p-a] = K_dram[C0-9-p+a]
            nc.sync.dma_start(
                out=T_A[:].rearrange("p (a b) -> p a b", a=128),
                in_=bass.AP(tensor=K_dram, offset=(C0 - 9) * NT,
                            ap=[[-NT, 128], [NT, 128], [1, NT]]))
            # T_B[p, a, e] = K[p-a] = K_dram[C0-p+a]
            nc.gpsimd.dma_start(
                out=T_B[:].rearrange("p (a b) -> p a b", a=128),
                in_=bass.AP(tensor=K_dram, offset=C0 * NT,
                            ap=[[-NT, 9], [NT, 128], [1, NT]]))
            # T_C[p, a, e] = K[137+p-a] = K_dram[C0-137-p+a]
            nc.gpsimd.dma_start(
                out=T_C[:].rearrange("p (a b) -> p a b", a=128),
                in_=bass.AP(tensor=K_dram, offset=(C0 - 137) * NT,
                            ap=[[-NT, 9], [NT, 128], [1, NT]]))
            T_A3 = T_A[:].rearrange("p (a b) -> p a b", a=128)
            T_B3 = T_B[:].rearrange("p (a b) -> p a b", a=128)
            T_C3 = T_C[:].rearrange("p (a b) -> p a b", a=128)
            # matmul operands must start at partition 0/32/64: copy the 9
            # boundary rows of half 0 (119..127) into a base-0 tile
            xf_b0 = pp.tile([9, XHW], BF16, tag="xf_b0")
            nc.sync.dma_start(out=xf_b0, in_=xf_pad[119:128, 0:XHW])

            _heat(26, "b")
            # ---- phase F: stage-1 Toeplitz matmuls -> inp_dram ----
            with tc.tile_pool(name="psum1", bufs=2, space="PSUM") as p1p:
                for t in range(2):
                    pinp = p1p.tile([128, W], F32, tag="pinp")
                    nmm = NT * 2 + 2
                    i = 0
                    for kxp in range(NT):
                        sl = 18 - kxp
                        nc.tensor.matmul(pinp, T_A3[:, :, kxp],
                                         xf_pad[:, ds(t * XHW + sl, W)],
                                         start=(i == 0), stop=(i == nmm - 1)); i += 1
                        if t == 0:
                            nc.tensor.matmul(pinp, T_C3[0:9, :, kxp],
                                             xf_pad[0:9, ds(XHW + sl, W)],
                                             start=False, stop=(i == nmm - 1)); i += 1
                        else:
                            nc.tensor.matmul(pinp, T_B3[0:9, :, kxp],
                                             xf_b0[:, ds(sl, W)],
                                             start=False, stop=(i == nmm - 1)); i += 1
                    nc.tensor.matmul(pinp, sb["I128"],
                                     xf_pad[:, ds(t * XHW + R, W)],
                                     start=False, stop=False); i += 1
                    nc.tensor.matmul(pinp, sb["CVEC"], sb["ONESR"][0:1, 0:W],
                                     start=False, stop=True); i += 1
                    s1 = pp.tile([128, W], BF16, tag=f"s1_{t}", name=f"s1stage{t}")
                    nc.vector.tensor_copy(out=s1, in_=pinp)
                    # inp3[r, j, e] = inp_padded[r, j + e]; s1 covers padded
                    # cols 1..256 of rows 1+128t..128+128t
                    for j in range(3):
                        dst = bass.AP(
                            tensor=inp_dram,
                            offset=(1 + 128 * t) * 792 + j * 264 + (1 - j),
                            ap=[[792, 128], [1, W]])
                        eng = (nc.sync, nc.gpsimd)[t]
                        eng.dma_start(out=dst, in_=s1)

            _heat(30, "c")
            # ---- phase G: im2col + stage-2 + store ----
            with (
                tc.tile_pool(name="gpool", bufs=3) as gp,
                tc.tile_pool(name="psum2", bufs=4, space="PSUM") as p2p,
            ):
                for ch in range(8):               # h2-chunks of 16
                    im = gp.tile([18, 4096], BF16, tag="im", bufs=8)
                    for g in range(2):
                        srcp = bass.AP(
                            tensor=inp_dram,
                            offset=(g * 128 + ch * 16) * 792,
                            ap=[[264, 9], [792, 16], [1, W]])
                        nc.sync.dma_start(
                            out=im[ts(g, 9), :].rearrange(
                                "p (d e) -> p d e", d=16),
                            in_=srcp)
                    for pair in range(2):         # 8 rows per store
                        ysb = gp.tile([128, 2048], BF16, tag="ysb",
                                      name="ystage", bufs=6)
                        py = p2p.tile([128, 2048], F32, tag="py", bufs=2)
                        for j in range(4):
                            nc.tensor.matmul(
                                py[:, ts(j, 512)], sb["W2"],
                                im[:, ds(pair * 2048 + j * 512, 512)],
                                start=True, stop=True)
                        # single evac per store unit (+ b_conv bias, bf16)
                        u = ch * 2 + pair
                        if (u + u // 2) % 2 == 0:
                            nc.scalar.activation(
                                out=ysb, in_=py,
                                func=mybir.ActivationFunctionType.Identity,
                                bias=sb["BC"][:, 0:1], scale=1.0)
                        else:
                            nc.vector.tensor_scalar_add(
                                out=ysb, in0=py, scalar1=sb["BC"][:, 0:1])
                        dst = bass.AP(
                            tensor=y,
                            offset=(ch * 16 + pair * 8) * W,
                            ap=[[128 * W, 2], [HW, 64], [1, 2048]])
                        nc.gpsimd.dma_start(out=dst, in_=ysb[:])
    with tile.TileContext(nc) as tc:
        _graph(tc)
    nc.finalize()
    return nc


def kernel(**inputs):
    x = np.ascontiguousarray(inputs["x"], dtype=np.float32)
    params = {k: np.asarray(v) for k, v in inputs.items() if k != "x"}
    nc = build(params, num_devices=8)
    from concourse.bass_utils import run_bass_kernel_spmd
    in_maps = [{"xb": np.ascontiguousarray(x[b])} for b in range(B)]
    res = run_bass_kernel_spmd(nc, in_maps, core_ids=list(range(B)))
    return np.stack([np.asarray(res.results[b]["y"], dtype=np.float32)
                     for b in range(B)])


# revision 23
# speedup vs baseline: 1.3183x; 1.0039x over previous
"""Trainium2 Bass kernel for nn_DeformableConvLayer.

Math (validated vs reference in numpy):
  xf   = sum_c w_icfd[c] * x[:, c] + b_icfd                       (B,H,W)
  mean = mean(xf, (h,w));  dy/dx = mean*w_off + b_off             (per b, 1600 stencils)
  The whole translate+fuse stage is a dense 19x19 conv with a data-dependent
  per-b kernel K_b[ky,kx] = sum_s w_fus[g_s]*hat(dy_s-ky)*hat(dx_s-kx),
  hat(t) = max(0, 1-|t|)  (bilinear weights == hat at integer taps).
  inp  = conv2d(xf, K_b, zero-pad) + 64*b_fus + xf
  y    = conv2d(inp, w_conv 3x3, zero-pad) + b_conv               (B,64,H,W)

Sharding: data-parallel, one batch element per NeuronCore (B=8, 8 cores).
Wide data paths run in bf16: x is cast to bf16 during the SWDGE load DMA,
all conv matmuls are bf16 (fp32 PSUM accumulation), and y is stored as bf16
and widened to fp32 on the host.  Stage-1 runs as Toeplitz-banded matmuls on
the tensor engine; the banded lhsT tables are materialized from K_b via
row-reversed staircase reads of a padded DRAM buffer (contiguous 4.8KB
descriptors).  xf stays on-chip: stage-0 PSUM is evacuated (bias fused) to a
staging tile and scattered to the padded xf tile by SBUF->SBUF DMA.
"""
import numpy as np
import ml_dtypes

import concourse.bacc as bacc
import concourse.bass as bass
import concourse.tile as tile
from concourse import mybir
from concourse.bass import ds, ts

F32 = mybir.dt.float32
BF16 = mybir.dt.bfloat16
NPBF = np.dtype(ml_dtypes.bfloat16)

B, C, H, W = 8, 64, 256, 256
G, DFC = 25, 64
R = 9
NT = 2 * R + 1            # 19 taps
C0 = 145                  # reversed-K row anchor in K_dram
HW = H * W
XHW = 274                 # xf_pad per-half width: 9 | 256 | 9


def _consts(params):
    """Host-side constant tensors derived from the (small) param inputs."""
    w_icfd = params["w_icfd"].astype(np.float32)
    w_off = params["w_off"].astype(np.float32)
    b_off = params["b_off"].astype(np.float32)
    w_fus = params["w_fus"].astype(np.float32)
    b_fus = float(params["b_fus"])
    w_conv = params["w_conv"].astype(np.float32)
    b_conv = params["b_conv"].astype(np.float32)

    W0 = np.zeros((128, 2), np.float32)
    for half in range(2):
        W0[half * 64:(half + 1) * 64, half] = w_icfd

    W2 = np.zeros((18, 128), np.float32)
    for g in range(2):
        for ky2 in range(3):
            for kx2 in range(3):
                W2[g * 9 + ky2 * 3 + kx2, g * 64:(g + 1) * 64] = w_conv[:, 0, ky2, kx2]
    BC = np.zeros((128, 1), np.float32)
    BC[0:64, 0] = b_conv
    BC[64:128, 0] = b_conv

    taps_rev = (R - np.arange(NT)).astype(np.float32)     # [9, 8, ..., -9]
    taps_fwd = (np.arange(NT) - R).astype(np.float32)     # [-9, ..., 9]
    TAPSF = np.tile(taps_fwd[None, :], (128, 1))
    TAPSR = np.tile(taps_rev[None, :], (128, 1))

    # s-chunk layout: s = c*128 + p, 13 chunks; tail (s>=1600) padded with zeros
    WF = np.zeros((128, 13), np.float32)
    WOFF = np.zeros((128, 26), np.float32)    # cols 0..12 y, 13..25 x
    BOFF = np.zeros((128, 26), np.float32)
    for c in range(13):
        for p in range(128):
            s = c * 128 + p
            if s < 1600:
                WF[p, c] = w_fus[s // 64]
                WOFF[p, c] = w_off[2 * s]
                BOFF[p, c] = b_off[2 * s]
                WOFF[p, 13 + c] = w_off[2 * s + 1]
                BOFF[p, 13 + c] = b_off[2 * s + 1]

    C_total = DFC * b_fus
    return dict(
        W0=W0.astype(NPBF), W2=W2.astype(NPBF), BC=BC,
        TAPSF=TAPSF, TAPSR=TAPSR, WF=WF, WOFF=WOFF, BOFF=BOFF,
        I128=np.eye(128, dtype=np.float32).astype(NPBF),
        ONESR=np.ones((1, 256), np.float32),
        ONESC=np.ones((128, 1), np.float32),
        ONES2=np.ones((2, 128), np.float32),
        CVEC=np.full((1, 128), C_total, np.float32),
        b_icfd=float(params["b_icfd"]),
    )


def build(params, num_devices=8):
    cs = _consts(params)
    nc = bacc.Bacc("TRN2", target_bir_lowering=False, debug=False,
                   num_devices=num_devices)
    xb = nc.dram_tensor("xb", [C, H, W], F32, kind="ExternalInput")
    y = nc.dram_tensor("y", [64, H, W], BF16, kind="ExternalOutput")
    K_dram = nc.dram_tensor("k_scr", [280, NT], BF16, kind="Internal")
    inp_dram = nc.dram_tensor("inp_scr", [260, 792], BF16, kind="Internal")

    ct = {k: nc.inline_tensor(v, name=f"c_{k}") for k, v in cs.items()
          if isinstance(v, np.ndarray)}
    b_icfd = cs["b_icfd"]

    def _graph(tc):
        with (
            tc.tile_pool(name="consts", bufs=1) as cp,
            tc.tile_pool(name="persist", bufs=1) as pp,
        ):
            # ---- load constants ----
            sb = {}
            for i, k in enumerate(("W0", "W2", "BC", "TAPSF", "TAPSR", "WF",
                                   "WOFF", "BOFF", "I128", "ONESR", "ONESC",
                                   "ONES2", "CVEC")):
                dt = BF16 if cs[k].dtype == NPBF else F32
                t = cp.tile(list(cs[k].shape), dt, tag=k, name=f"sb_{k}")
                eng = (nc.sync, nc.scalar)[i % 2]
                eng.dma_start(out=t, in_=ct[k][:, :])
                sb[k] = t
            zbf = cp.tile([128, 792], BF16, tag="zbf")
            nc.vector.memset(zbf, 0.0)
            bic2 = cp.tile([2, 1], F32, tag="bic2")
            nc.vector.memset(bic2, b_icfd)

            # ---- zero scratch DRAM (early, off critical path) ----
            nc.sync.dma_start(
                out=bass.AP(tensor=K_dram, offset=0, ap=[[NT, 128], [1, NT]]),
                in_=zbf[:, 0:NT])
            nc.sync.dma_start(
                out=bass.AP(tensor=K_dram, offset=128 * NT,
                            ap=[[NT, 128], [1, NT]]),
                in_=zbf[:, 0:NT])
            nc.sync.dma_start(
                out=bass.AP(tensor=K_dram, offset=256 * NT,
                            ap=[[NT, 24], [1, NT]]),
                in_=zbf[0:24, 0:NT])
            nc.scalar.dma_start(out=inp_dram[0:128, :], in_=zbf[:, 0:792])
            nc.scalar.dma_start(out=inp_dram[128:256, :], in_=zbf[:, 0:792])
            nc.scalar.dma_start(out=inp_dram[256:260, :], in_=zbf[0:4, 0:792])

            # ---- persistent xf tile: [128, 274*2] bf16, halves side by side
            xf_pad = pp.tile([128, 2 * XHW], BF16, tag="xf_pad")
            nc.vector.memset(xf_pad, 0.0)

            def _heat(n, tag):
                # keep-warm matmuls into a scratch PSUM bank: fill PE idle
                # gaps so real matmuls dispatch into a warm p-state streak
                with tc.tile_pool(name=f"heat_{tag}", bufs=1,
                                  space="PSUM") as hp:
                    pdum = hp.tile([2, 512], F32, tag=f"pdum{tag}")
                    for _ in range(n):
                        nc.tensor.matmul(pdum, sb["W0"], zbf[:, 0:512],
                                         start=True, stop=True)

            _heat(10, "a")
            # ---- phase B: x cast-load + stage-0 matmul + evac + scatter ----
            NCH = 8                       # chunks of 16 rows
            with (
                tc.tile_pool(name="bpool", bufs=3) as bp,
                tc.tile_pool(name="psum0", bufs=4, space="PSUM") as p0p,
            ):
                for ch in range(NCH):
                    sbx = bp.tile([128, 4096], BF16, tag="sbx", bufs=8)
                    for half in range(2):   # partition = half*64 + c
                        srcp = bass.AP(tensor=xb,
                                       offset=(half * 128 + ch * 16) * W,
                                       ap=[[HW, 64], [1, 4096]])
                        nc.gpsimd.dma_start(out=sbx[ts(half, 64), :], in_=srcp)
                    s0b = bp.tile([2, 4096], BF16, tag="s0b", bufs=8)
                    for q in range(4):
                        p0 = p0p.tile([2, 1024], F32, tag="p0", name="p0t")
                        for j in range(2):
                            nc.tensor.matmul(
                                p0[:, ts(j, 512)],
                                sb["W0"],
                                sbx[:, ds(q * 1024 + j * 512, 512)],
                                start=True, stop=True)
                        # evac PSUM -> bf16 staging with b_icfd bias fused
                        if (ch * 4 + q) % 2 == 0:
                            nc.scalar.activation(
                                out=s0b[:, ts(q, 1024)], in_=p0,
                                func=mybir.ActivationFunctionType.Identity,
                                bias=bic2[:, 0:1], scale=1.0)
                        else:
                            nc.vector.tensor_scalar_add(
                                out=s0b[:, ts(q, 1024)], in0=p0,
                                scalar1=bic2[:, 0:1])
                    for half in range(2):   # scatter 16 rows into xf_pad
                        nc.sync.dma_start(
                            out=xf_pad[ch * 16:ch * 16 + 16,
                                       ds(half * XHW + R, 256)],
                            in_=s0b[half:half + 1, :])

            # ---- phase C: column sums -> mean ----
            colsums = pp.tile([128, 1], F32, tag="colsums")
            nc.vector.tensor_reduce(out=colsums,
                                    in_=xf_pad[:, 0:2 * XHW],
                                    axis=mybir.AxisListType.X,
                                    op=mybir.AluOpType.add)
            with tc.tile_pool(name="psA", bufs=1, space="PSUM") as psA:
                pm = psA.tile([1, 1], F32, tag="pm")
                nc.tensor.matmul(pm, colsums, sb["ONESC"], start=True, stop=True)
                ts2 = pp.tile([1, 1], F32, tag="ts2")
                nc.scalar.copy(out=ts2, in_=pm)
                pmb = psA.tile([128, 1], F32, tag="pmb")
                nc.tensor.matmul(pmb, sb["ONES2"][0:1, :], ts2,
                                 start=True, stop=True)
                mean_bc = pp.tile([128, 1], F32, tag="mean_bc")
                nc.scalar.activation(out=mean_bc, in_=pmb,
                                     func=mybir.ActivationFunctionType.Copy,
                                     scale=1.0 / HW)

                # ---- phase D: offsets, hats, K matmul ----
                dyx = pp.tile([128, 26], F32, tag="dyx")
                nc.vector.tensor_scalar_mul(out=dyx, in0=sb["WOFF"],
                                            scalar1=mean_bc[:, 0:1])
                nc.vector.tensor_add(out=dyx, in0=dyx, in1=sb["BOFF"])
                HH = pp.tile([128, 26 * NT], F32, tag="HH")
                HH3 = HH[:].rearrange("p (a b) -> p a b", a=26)
                nc.vector.tensor_tensor(
                    out=HH3[:, 0:13, :],
                    in0=dyx[:, 0:13].unsqueeze(2).to_broadcast([128, 13, NT]),
                    in1=sb["TAPSF"][:].unsqueeze(1).to_broadcast([128, 13, NT]),
                    op=mybir.AluOpType.subtract)
                nc.vector.tensor_tensor(
                    out=HH3[:, 13:26, :],
                    in0=dyx[:, 13:26].unsqueeze(2).to_broadcast([128, 13, NT]),
                    in1=sb["TAPSR"][:].unsqueeze(1).to_broadcast([128, 13, NT]),
                    op=mybir.AluOpType.subtract)
                nc.vector.tensor_scalar(out=HH, in0=HH, scalar1=0.0,
                                        scalar2=None,
                                        op0=mybir.AluOpType.abs_max)
                nc.scalar.activation(out=HH, in_=HH,
                                     func=mybir.ActivationFunctionType.Relu,
                                     scale=-1.0, bias=1.0)
                WHY = pp.tile([128, 13 * NT], F32, tag="WHY")
                nc.vector.tensor_tensor(
                    out=WHY[:].rearrange("p (a b) -> p a b", a=13),
                    in0=HH3[:, 0:13, :],
                    in1=sb["WF"][:].unsqueeze(2).to_broadcast([128, 13, NT]),
                    op=mybir.AluOpType.mult)
                WHY3 = WHY[:].rearrange("p (a b) -> p a b", a=13)
                pK = psA.tile([NT, NT], F32, tag="pK")
                for c in range(13):
                    nc.tensor.matmul(pK, WHY3[:, c, :], HH3[:, 13 + c, :],
                                     start=(c == 0), stop=(c == 12))
                Ksb = pp.tile([NT, NT], BF16, tag="Ksb")
                nc.scalar.copy(out=Ksb, in_=pK)

            # ---- phase E: reversed K write + contiguous staircase T tables
            # K_dram[C0 - d] = Ksb[d]  (bf16 cast during SWDGE write)
            nc.sync.dma_start(
                out=bass.AP(tensor=K_dram, offset=C0 * NT,
                            ap=[[-NT, NT], [1, NT]]),
                in_=Ksb)
            T_A = pp.tile([128, 128 * NT], BF16, tag="T_A")
            T_B = pp.tile([9, 128 * NT], BF16, tag="T_B")
            T_C = pp.tile([9, 128 * NT], BF16, tag="T_C")
            # T_A[p, a, e] = K[9+p-a] = K_dram[C0-9-p+a]
            nc.sync.dma_start(
                out=T_A[:].rearrange("p (a b) -> p a b", a=128),
                in_=bass.AP(tensor=K_dram, offset=(C0 - 9) * NT,
                            ap=[[-NT, 128], [NT, 128], [1, NT]]))
            # T_B[p, a, e] = K[p-a] = K_dram[C0-p+a]
            nc.gpsimd.dma_start(
                out=T_B[:].rearrange("p (a b) -> p a b", a=128),
                in_=bass.AP(tensor=K_dram, offset=C0 * NT,
                            ap=[[-NT, 9], [NT, 128], [1, NT]]))
            # T_C[p, a, e] = K[137+p-a] = K_dram[C0-137-p+a]
            nc.gpsimd.dma_start(
                out=T_C[:].rearrange("p (a b) -> p a b", a=128),
                in_=bass.AP(tensor=K_dram, offset=(C0 - 137) * NT,
                            ap=[[-NT, 9], [NT, 128], [1, NT]]))
            T_A3 = T_A[:].rearrange("p (a b) -> p a b", a=128)
            T_B3 = T_B[:].rearrange("p (a b) -> p a b", a=128)
            T_C3 = T_C[:].rearrange("p (a b) -> p a b", a=128)
            # matmul operands must start at partition 0/32/64: copy the 9
            # boundary rows of half 0 (119..127) into a base-0 tile
            xf_b0 = pp.tile([9, XHW], BF16, tag="xf_b0")
            nc.sync.dma_start(out=xf_b0, in_=xf_pad[119:128, 0:XHW])

            _heat(26, "b")
            # ---- phase F: stage-1 Toeplitz matmuls -> inp_dram ----
            with tc.tile_pool(name="psum1", bufs=2, space="PSUM") as p1p:
                for t in range(2):
                    pinp = p1p.tile([128, W], F32, tag="pinp")
                    nmm = NT * 2 + 2
                    i = 0
                    for kxp in range(NT):
                        sl = 18 - kxp
                        nc.tensor.matmul(pinp, T_A3[:, :, kxp],
                                         xf_pad[:, ds(t * XHW + sl, W)],
                                         start=(i == 0), stop=(i == nmm - 1)); i += 1
                        if t == 0:
                            nc.tensor.matmul(pinp, T_C3[0:9, :, kxp],
                                             xf_pad[0:9, ds(XHW + sl, W)],
                                             start=False, stop=(i == nmm - 1)); i += 1
                        else:
                            nc.tensor.matmul(pinp, T_B3[0:9, :, kxp],
                                             xf_b0[:, ds(sl, W)],
                                             start=False, stop=(i == nmm - 1)); i += 1
                    nc.tensor.matmul(pinp, sb["I128"],
                                     xf_pad[:, ds(t * XHW + R, W)],
                                     start=False, stop=False); i += 1
                    nc.tensor.matmul(pinp, sb["CVEC"], sb["ONESR"][0:1, 0:W],
                                     start=False, stop=True); i += 1
                    s1 = pp.tile([128, W], BF16, tag=f"s1_{t}", name=f"s1stage{t}")
                    nc.vector.tensor_copy(out=s1, in_=pinp)
                    # inp3[r, j, e] = inp_padded[r, j + e]; s1 covers padded
                    # cols 1..256 of rows 1+128t..128+128t
                    for j in range(3):
                        dst = bass.AP(
                            tensor=inp_dram,
                            offset=(1 + 128 * t) * 792 + j * 264 + (1 - j),
                            ap=[[792, 128], [1, W]])
                        eng = (nc.sync, nc.gpsimd)[t]
                        eng.dma_start(out=dst, in_=s1)

            _heat(30, "c")
            # ---- phase G: im2col + stage-2 + store ----
            with (
                tc.tile_pool(name="gpool", bufs=3) as gp,
                tc.tile_pool(name="psum2", bufs=4, space="PSUM") as p2p,
            ):
                for ch in range(8):               # h2-chunks of 16
                    im = gp.tile([18, 4096], BF16, tag="im", bufs=8)
                    for g in range(2):
                        srcp = bass.AP(
                            tensor=inp_dram,
                            offset=(g * 128 + ch * 16) * 792,
                            ap=[[264, 9], [792, 16], [1, W]])
                        nc.sync.dma_start(
                            out=im[ts(g, 9), :].rearrange(
                                "p (d e) -> p d e", d=16),
                            in_=srcp)
                    for pair in range(2):         # 8 rows per store
                        ysb = gp.tile([128, 2048], BF16, tag="ysb",
                                      name="ystage", bufs=6)
                        py = p2p.tile([128, 2048], F32, tag="py", bufs=2)
                        for j in range(4):
                            nc.tensor.matmul(
                                py[:, ts(j, 512)], sb["W2"],
                                im[:, ds(pair * 2048 + j * 512, 512)],
                                start=True, stop=True)
                        # single evac per store unit (+ b_conv bias, bf16)
                        u = ch * 2 + pair
                        if (u + u // 2) % 2 == 0:
                            nc.scalar.activation(
                                out=ysb, in_=py,
                                func=mybir.ActivationFunctionType.Identity,
                                bias=sb["BC"][:, 0:1], scale=1.0)
                        else:
                            nc.vector.tensor_scalar_add(
                                out=ysb, in0=py, scalar1=sb["BC"][:, 0:1])
                        dst = bass.AP(
                            tensor=y,
                            offset=(ch * 16 + pair * 8) * W,
                            ap=[[128 * W, 2], [HW, 64], [1, 2048]])
                        nc.gpsimd.dma_start(out=dst, in_=ysb[:])
    with tile.TileContext(nc) as tc:
        _graph(tc)
    nc.finalize()
    return nc


def kernel(**inputs):
    x = np.ascontiguousarray(inputs["x"], dtype=np.float32)
    params = {k: np.asarray(v) for k, v in inputs.items() if k != "x"}
    nc = build(params, num_devices=8)
    from concourse.bass_utils import run_bass_kernel_spmd
    in_maps = [{"xb": np.ascontiguousarray(x[b])} for b in range(B)]
    res = run_bass_kernel_spmd(nc, in_maps, core_ids=list(range(B)))
    return np.stack([np.asarray(res.results[b]["y"], dtype=np.float32)
                     for b in range(B)])


# revision 24
# speedup vs baseline: 1.3211x; 1.0022x over previous
"""Trainium2 Bass kernel for nn_DeformableConvLayer.

Math (validated vs reference in numpy):
  xf   = sum_c w_icfd[c] * x[:, c] + b_icfd                       (B,H,W)
  mean = mean(xf, (h,w));  dy/dx = mean*w_off + b_off             (per b, 1600 stencils)
  The whole translate+fuse stage is a dense 19x19 conv with a data-dependent
  per-b kernel K_b[ky,kx] = sum_s w_fus[g_s]*hat(dy_s-ky)*hat(dx_s-kx),
  hat(t) = max(0, 1-|t|)  (bilinear weights == hat at integer taps).
  inp  = conv2d(xf, K_b, zero-pad) + 64*b_fus + xf
  y    = conv2d(inp, w_conv 3x3, zero-pad) + b_conv               (B,64,H,W)

Sharding: data-parallel, one batch element per NeuronCore (B=8, 8 cores).
Wide data paths run in bf16: x is cast to bf16 during the SWDGE load DMA,
all conv matmuls are bf16 (fp32 PSUM accumulation), and y is stored as bf16
and widened to fp32 on the host.  Stage-1 runs as Toeplitz-banded matmuls on
the tensor engine; the banded lhsT tables are materialized from K_b via
row-reversed staircase reads of a padded DRAM buffer (contiguous 4.8KB
descriptors).  xf stays on-chip: stage-0 PSUM is evacuated (bias fused) to a
staging tile and scattered to the padded xf tile by SBUF->SBUF DMA.
"""
import numpy as np
import ml_dtypes

import concourse.bacc as bacc
import concourse.bass as bass
import concourse.tile as tile
from concourse import mybir
from concourse.bass import ds, ts

F32 = mybir.dt.float32
BF16 = mybir.dt.bfloat16
NPBF = np.dtype(ml_dtypes.bfloat16)

B, C, H, W = 8, 64, 256, 256
G, DFC = 25, 64
R = 9
NT = 2 * R + 1            # 19 taps
C0 = 145                  # reversed-K row anchor in K_dram
HW = H * W
XHW = 274                 # xf_pad per-half width: 9 | 256 | 9


def _consts(params):
    """Host-side constant tensors derived from the (small) param inputs."""
    w_icfd = params["w_icfd"].astype(np.float32)
    w_off = params["w_off"].astype(np.float32)
    b_off = params["b_off"].astype(np.float32)
    w_fus = params["w_fus"].astype(np.float32)
    b_fus = float(params["b_fus"])
    w_conv = params["w_conv"].astype(np.float32)
    b_conv = params["b_conv"].astype(np.float32)

    W0 = np.zeros((128, 2), np.float32)
    for half in range(2):
        W0[half * 64:(half + 1) * 64, half] = w_icfd

    W2 = np.zeros((18, 128), np.float32)
    for g in range(2):
        for ky2 in range(3):
            for kx2 in range(3):
                W2[g * 9 + ky2 * 3 + kx2, g * 64:(g + 1) * 64] = w_conv[:, 0, ky2, kx2]
    BC = np.zeros((128, 1), np.float32)
    BC[0:64, 0] = b_conv
    BC[64:128, 0] = b_conv

    taps_rev = (R - np.arange(NT)).astype(np.float32)     # [9, 8, ..., -9]
    taps_fwd = (np.arange(NT) - R).astype(np.float32)     # [-9, ..., 9]
    TAPSF = np.tile(taps_fwd[None, :], (128, 1))
    TAPSR = np.tile(taps_rev[None, :], (128, 1))

    # s-chunk layout: s = c*128 + p, 13 chunks; tail (s>=1600) padded with zeros
    WF = np.zeros((128, 13), np.float32)
    WOFF = np.zeros((128, 26), np.float32)    # cols 0..12 y, 13..25 x
    BOFF = np.zeros((128, 26), np.float32)
    for c in range(13):
        for p in range(128):
            s = c * 128 + p
            if s < 1600:
                WF[p, c] = w_fus[s // 64]
                WOFF[p, c] = w_off[2 * s]
                BOFF[p, c] = b_off[2 * s]
                WOFF[p, 13 + c] = w_off[2 * s + 1]
                BOFF[p, 13 + c] = b_off[2 * s + 1]

    C_total = DFC * b_fus
    return dict(
        W0=W0.astype(NPBF), W2=W2.astype(NPBF), BC=BC,
        TAPSF=TAPSF, TAPSR=TAPSR, WF=WF, WOFF=WOFF, BOFF=BOFF,
        I128=np.eye(128, dtype=np.float32).astype(NPBF),
        ONESR=np.ones((1, 256), np.float32),
        ONESC=np.ones((128, 1), np.float32),
        ONES2=np.ones((2, 128), np.float32),
        CVEC=np.full((1, 128), C_total, np.float32),
        b_icfd=float(params["b_icfd"]),
    )


def build(params, num_devices=8):
    cs = _consts(params)
    nc = bacc.Bacc("TRN2", target_bir_lowering=False, debug=False,
                   num_devices=num_devices)
    xb = nc.dram_tensor("xb", [C, H, W], F32, kind="ExternalInput")
    y = nc.dram_tensor("y", [64, H, W], BF16, kind="ExternalOutput")
    K_dram = nc.dram_tensor("k_scr", [280, NT], BF16, kind="Internal")
    inp_dram = nc.dram_tensor("inp_scr", [260, 792], BF16, kind="Internal")

    ct = {k: nc.inline_tensor(v, name=f"c_{k}") for k, v in cs.items()
          if isinstance(v, np.ndarray)}
    b_icfd = cs["b_icfd"]

    def _graph(tc):
        with (
            tc.tile_pool(name="consts", bufs=1) as cp,
            tc.tile_pool(name="persist", bufs=1) as pp,
        ):
            # ---- load constants ----
            sb = {}
            for i, k in enumerate(("W0", "W2", "BC", "TAPSF", "TAPSR", "WF",
                                   "WOFF", "BOFF", "I128", "ONESR", "ONESC",
                                   "ONES2", "CVEC")):
                dt = BF16 if cs[k].dtype == NPBF else F32
                t = cp.tile(list(cs[k].shape), dt, tag=k, name=f"sb_{k}")
                eng = (nc.sync, nc.scalar)[i % 2]
                eng.dma_start(out=t, in_=ct[k][:, :])
                sb[k] = t
            zbf = cp.tile([128, 792], BF16, tag="zbf")
            nc.vector.memset(zbf, 0.0)
            bic2 = cp.tile([2, 1], F32, tag="bic2")
            nc.vector.memset(bic2, b_icfd)

            # ---- zero scratch DRAM (early, off critical path) ----
            nc.sync.dma_start(
                out=bass.AP(tensor=K_dram, offset=0, ap=[[NT, 128], [1, NT]]),
                in_=zbf[:, 0:NT])
            nc.sync.dma_start(
                out=bass.AP(tensor=K_dram, offset=128 * NT,
                            ap=[[NT, 128], [1, NT]]),
                in_=zbf[:, 0:NT])
            nc.sync.dma_start(
                out=bass.AP(tensor=K_dram, offset=256 * NT,
                            ap=[[NT, 24], [1, NT]]),
                in_=zbf[0:24, 0:NT])
            nc.scalar.dma_start(out=inp_dram[0:128, :], in_=zbf[:, 0:792])
            nc.scalar.dma_start(out=inp_dram[128:256, :], in_=zbf[:, 0:792])
            nc.scalar.dma_start(out=inp_dram[256:260, :], in_=zbf[0:4, 0:792])

            # ---- persistent xf tile: [128, 274*2] bf16, halves side by side
            xf_pad = pp.tile([128, 2 * XHW], BF16, tag="xf_pad")
            nc.vector.memset(xf_pad, 0.0)

            def _heat(n, tag):
                # keep-warm matmuls into a scratch PSUM bank: fill PE idle
                # gaps so real matmuls dispatch into a warm p-state streak
                with tc.tile_pool(name=f"heat_{tag}", bufs=1,
                                  space="PSUM") as hp:
                    pdum = hp.tile([2, 512], F32, tag=f"pdum{tag}")
                    for _ in range(n):
                        nc.tensor.matmul(pdum, sb["W0"], zbf[:, 0:512],
                                         start=True, stop=True)

            _heat(10, "a")
            # ---- phase B: x cast-load + stage-0 matmul + evac + scatter ----
            NCH = 8                       # chunks of 16 rows
            with (
                tc.tile_pool(name="bpool", bufs=3) as bp,
                tc.tile_pool(name="psum0", bufs=4, space="PSUM") as p0p,
            ):
                for ch in range(NCH):
                    sbx = bp.tile([128, 4096], BF16, tag="sbx", bufs=8)
                    for half in range(2):   # partition = half*64 + c
                        srcp = bass.AP(tensor=xb,
                                       offset=(half * 128 + ch * 16) * W,
                                       ap=[[HW, 64], [1, 4096]])
                        nc.gpsimd.dma_start(out=sbx[ts(half, 64), :], in_=srcp)
                    s0b = bp.tile([2, 4096], BF16, tag="s0b", bufs=8)
                    for q in range(4):
                        p0 = p0p.tile([2, 1024], F32, tag="p0", name="p0t")
                        for j in range(2):
                            nc.tensor.matmul(
                                p0[:, ts(j, 512)],
                                sb["W0"],
                                sbx[:, ds(q * 1024 + j * 512, 512)],
                                start=True, stop=True)
                        # evac PSUM -> bf16 staging with b_icfd bias fused
                        if (ch * 4 + q) % 2 == 0:
                            nc.scalar.activation(
                                out=s0b[:, ts(q, 1024)], in_=p0,
                                func=mybir.ActivationFunctionType.Identity,
                                bias=bic2[:, 0:1], scale=1.0)
                        else:
                            nc.vector.tensor_scalar_add(
                                out=s0b[:, ts(q, 1024)], in0=p0,
                                scalar1=bic2[:, 0:1])
                    for half in range(2):   # scatter 16 rows into xf_pad
                        nc.sync.dma_start(
                            out=xf_pad[ch * 16:ch * 16 + 16,
                                       ds(half * XHW + R, 256)],
                            in_=s0b[half:half + 1, :])

            # ---- phase C: column sums -> mean ----
            colsums = pp.tile([128, 1], F32, tag="colsums")
            nc.vector.tensor_reduce(out=colsums,
                                    in_=xf_pad[:, 0:2 * XHW],
                                    axis=mybir.AxisListType.X,
                                    op=mybir.AluOpType.add)
            with tc.tile_pool(name="psA", bufs=1, space="PSUM") as psA:
                pm = psA.tile([1, 1], F32, tag="pm")
                nc.tensor.matmul(pm, colsums, sb["ONESC"], start=True, stop=True)
                ts2 = pp.tile([1, 1], F32, tag="ts2")
                nc.scalar.copy(out=ts2, in_=pm)
                pmb = psA.tile([128, 1], F32, tag="pmb")
                nc.tensor.matmul(pmb, sb["ONES2"][0:1, :], ts2,
                                 start=True, stop=True)
                mean_bc = pp.tile([128, 1], F32, tag="mean_bc")
                nc.scalar.activation(out=mean_bc, in_=pmb,
                                     func=mybir.ActivationFunctionType.Copy,
                                     scale=1.0 / HW)

                # ---- phase D: offsets, hats, K matmul ----
                dyx = pp.tile([128, 26], F32, tag="dyx")
                nc.vector.tensor_scalar_mul(out=dyx, in0=sb["WOFF"],
                                            scalar1=mean_bc[:, 0:1])
                nc.vector.tensor_add(out=dyx, in0=dyx, in1=sb["BOFF"])
                HH = pp.tile([128, 26 * NT], F32, tag="HH")
                HH3 = HH[:].rearrange("p (a b) -> p a b", a=26)
                nc.vector.tensor_tensor(
                    out=HH3[:, 0:13, :],
                    in0=dyx[:, 0:13].unsqueeze(2).to_broadcast([128, 13, NT]),
                    in1=sb["TAPSF"][:].unsqueeze(1).to_broadcast([128, 13, NT]),
                    op=mybir.AluOpType.subtract)
                nc.vector.tensor_tensor(
                    out=HH3[:, 13:26, :],
                    in0=dyx[:, 13:26].unsqueeze(2).to_broadcast([128, 13, NT]),
                    in1=sb["TAPSR"][:].unsqueeze(1).to_broadcast([128, 13, NT]),
                    op=mybir.AluOpType.subtract)
                nc.vector.tensor_scalar(out=HH, in0=HH, scalar1=0.0,
                                        scalar2=None,
                                        op0=mybir.AluOpType.abs_max)
                nc.scalar.activation(out=HH, in_=HH,
                                     func=mybir.ActivationFunctionType.Relu,
                                     scale=-1.0, bias=1.0)
                WHY = pp.tile([128, 13 * NT], F32, tag="WHY")
                nc.vector.tensor_tensor(
                    out=WHY[:].rearrange("p (a b) -> p a b", a=13),
                    in0=HH3[:, 0:13, :],
                    in1=sb["WF"][:].unsqueeze(2).to_broadcast([128, 13, NT]),
                    op=mybir.AluOpType.mult)
                WHY3 = WHY[:].rearrange("p (a b) -> p a b", a=13)
                pK = psA.tile([NT, NT], F32, tag="pK")
                for c in range(13):
                    nc.tensor.matmul(pK, WHY3[:, c, :], HH3[:, 13 + c, :],
                                     start=(c == 0), stop=(c == 12))
                Ksb = pp.tile([NT, NT], BF16, tag="Ksb")
                nc.scalar.copy(out=Ksb, in_=pK)

            # ---- phase E: reversed K write + contiguous staircase T tables
            # K_dram[C0 - d] = Ksb[d]  (bf16 cast during SWDGE write)
            nc.sync.dma_start(
                out=bass.AP(tensor=K_dram, offset=C0 * NT,
                            ap=[[-NT, NT], [1, NT]]),
                in_=Ksb)
            T_A = pp.tile([128, 128 * NT], BF16, tag="T_A")
            T_B = pp.tile([9, 128 * NT], BF16, tag="T_B")
            T_C = pp.tile([9, 128 * NT], BF16, tag="T_C")
            # T_A[p, a, e] = K[9+p-a] = K_dram[C0-9-p+a]
            nc.sync.dma_start(
                out=T_A[:].rearrange("p (a b) -> p a b", a=128),
                in_=bass.AP(tensor=K_dram, offset=(C0 - 9) * NT,
                            ap=[[-NT, 128], [NT, 128], [1, NT]]))
            # T_B[p, a, e] = K[p-a] = K_dram[C0-p+a]
            nc.gpsimd.dma_start(
                out=T_B[:].rearrange("p (a b) -> p a b", a=128),
                in_=bass.AP(tensor=K_dram, offset=C0 * NT,
                            ap=[[-NT, 9], [NT, 128], [1, NT]]))
            # T_C[p, a, e] = K[137+p-a] = K_dram[C0-137-p+a]
            nc.gpsimd.dma_start(
                out=T_C[:].rearrange("p (a b) -> p a b", a=128),
                in_=bass.AP(tensor=K_dram, offset=(C0 - 137) * NT,
                            ap=[[-NT, 9], [NT, 128], [1, NT]]))
            T_A3 = T_A[:].rearrange("p (a b) -> p a b", a=128)
            T_B3 = T_B[:].rearrange("p (a b) -> p a b", a=128)
            T_C3 = T_C[:].rearrange("p (a b) -> p a b", a=128)
            # matmul operands must start at partition 0/32/64: copy the 9
            # boundary rows of half 0 (119..127) into a base-0 tile
            xf_b0 = pp.tile([9, XHW], BF16, tag="xf_b0")
            nc.sync.dma_start(out=xf_b0, in_=xf_pad[119:128, 0:XHW])

            _heat(26, "b")
            # ---- phase F: stage-1 Toeplitz matmuls -> inp_dram ----
            with tc.tile_pool(name="psum1", bufs=2, space="PSUM") as p1p:
                for t in range(2):
                    pinp = p1p.tile([128, W], F32, tag="pinp")
                    nmm = NT * 2 + 2
                    i = 0
                    for kxp in range(NT):
                        sl = 18 - kxp
                        nc.tensor.matmul(pinp, T_A3[:, :, kxp],
                                         xf_pad[:, ds(t * XHW + sl, W)],
                                         start=(i == 0), stop=(i == nmm - 1)); i += 1
                        if t == 0:
                            nc.tensor.matmul(pinp, T_C3[0:9, :, kxp],
                                             xf_pad[0:9, ds(XHW + sl, W)],
                                             start=False, stop=(i == nmm - 1)); i += 1
                        else:
                            nc.tensor.matmul(pinp, T_B3[0:9, :, kxp],
                                             xf_b0[:, ds(sl, W)],
                                             start=False, stop=(i == nmm - 1)); i += 1
                    nc.tensor.matmul(pinp, sb["I128"],
                                     xf_pad[:, ds(t * XHW + R, W)],
                                     start=False, stop=False); i += 1
                    nc.tensor.matmul(pinp, sb["CVEC"], sb["ONESR"][0:1, 0:W],
                                     start=False, stop=True); i += 1
                    s1 = pp.tile([128, W], BF16, tag=f"s1_{t}", name=f"s1stage{t}")
                    nc.vector.tensor_copy(out=s1, in_=pinp)
                    # inp3[r, j, e] = inp_padded[r, j + e]; s1 covers padded
                    # cols 1..256 of rows 1+128t..128+128t
                    for j in range(3):
                        dst = bass.AP(
                            tensor=inp_dram,
                            offset=(1 + 128 * t) * 792 + j * 264 + (1 - j),
                            ap=[[792, 128], [1, W]])
                        eng = (nc.sync, nc.gpsimd)[t]
                        eng.dma_start(out=dst, in_=s1)

            _heat(30, "c")
            # ---- phase G: im2col + stage-2 + store ----
            with (
                tc.tile_pool(name="gpool", bufs=3) as gp,
                tc.tile_pool(name="psum2", bufs=4, space="PSUM") as p2p,
            ):
                for ch in range(8):               # h2-chunks of 16
                    im = gp.tile([18, 4096], BF16, tag="im", bufs=8)
                    for g in range(2):
                        srcp = bass.AP(
                            tensor=inp_dram,
                            offset=(g * 128 + ch * 16) * 792,
                            ap=[[264, 9], [792, 16], [1, W]])
                        nc.sync.dma_start(
                            out=im[ts(g, 9), :].rearrange(
                                "p (d e) -> p d e", d=16),
                            in_=srcp)
                    for pair in range(2):         # 8 rows per store
                        ysb = gp.tile([128, 2048], BF16, tag="ysb",
                                      name="ystage", bufs=8)
                        py = p2p.tile([128, 2048], F32, tag="py", bufs=2)
                        for j in range(4):
                            nc.tensor.matmul(
                                py[:, ts(j, 512)], sb["W2"],
                                im[:, ds(pair * 2048 + j * 512, 512)],
                                start=True, stop=True)
                        # single evac per store unit (+ b_conv bias, bf16)
                        u = ch * 2 + pair
                        if (u + u // 2) % 2 == 0:
                            nc.scalar.activation(
                                out=ysb, in_=py,
                                func=mybir.ActivationFunctionType.Identity,
                                bias=sb["BC"][:, 0:1], scale=1.0)
                        else:
                            nc.vector.tensor_scalar_add(
                                out=ysb, in0=py, scalar1=sb["BC"][:, 0:1])
                        dst = bass.AP(
                            tensor=y,
                            offset=(ch * 16 + pair * 8) * W,
                            ap=[[128 * W, 2], [HW, 64], [1, 2048]])
                        nc.gpsimd.dma_start(out=dst, in_=ysb[:])
    with tile.TileContext(nc) as tc:
        _graph(tc)
    nc.finalize()
    return nc


def kernel(**inputs):
    x = np.ascontiguousarray(inputs["x"], dtype=np.float32)
    params = {k: np.asarray(v) for k, v in inputs.items() if k != "x"}
    nc = build(params, num_devices=8)
    from concourse.bass_utils import run_bass_kernel_spmd
    in_maps = [{"xb": np.ascontiguousarray(x[b])} for b in range(B)]
    res = run_bass_kernel_spmd(nc, in_maps, core_ids=list(range(B)))
    return np.stack([np.asarray(res.results[b]["y"], dtype=np.float32)
                     for b in range(B)])


# revision 25
# speedup vs baseline: 1.3456x; 1.0185x over previous
"""Trainium2 Bass kernel for nn_DeformableConvLayer.

Math (validated vs reference in numpy):
  xf   = sum_c w_icfd[c] * x[:, c] + b_icfd                       (B,H,W)
  mean = mean(xf, (h,w));  dy/dx = mean*w_off + b_off             (per b, 1600 stencils)
  The whole translate+fuse stage is a dense 19x19 conv with a data-dependent
  per-b kernel K_b[ky,kx] = sum_s w_fus[g_s]*hat(dy_s-ky)*hat(dx_s-kx),
  hat(t) = max(0, 1-|t|)  (bilinear weights == hat at integer taps).
  inp  = conv2d(xf, K_b, zero-pad) + 64*b_fus + xf
  y    = conv2d(inp, w_conv 3x3, zero-pad) + b_conv               (B,64,H,W)

Sharding: data-parallel, one batch element per NeuronCore (B=8, 8 cores).
Wide data paths run in bf16: x is cast to bf16 during the SWDGE load DMA,
all conv matmuls are bf16 (fp32 PSUM accumulation), and y is stored as bf16
and widened to fp32 on the host.  Stage-1 runs as Toeplitz-banded matmuls on
the tensor engine; the banded lhsT tables are materialized from K_b via
row-reversed staircase reads of a padded DRAM buffer (contiguous 4.8KB
descriptors).  xf stays on-chip: stage-0 PSUM is evacuated (bias fused) to a
staging tile and scattered to the padded xf tile by SBUF->SBUF DMA.
"""
import numpy as np
import ml_dtypes

import concourse.bacc as bacc
import concourse.bass as bass
import concourse.tile as tile
from concourse import mybir
from concourse.bass import ds, ts

F32 = mybir.dt.float32
BF16 = mybir.dt.bfloat16
NPBF = np.dtype(ml_dtypes.bfloat16)

B, C, H, W = 8, 64, 256, 256
G, DFC = 25, 64
R = 9
NT = 2 * R + 1            # 19 taps
C0 = 145                  # reversed-K row anchor in K_dram
HW = H * W
XHW = 274                 # xf_pad per-half width: 9 | 256 | 9


def _consts(params):
    """Host-side constant tensors derived from the (small) param inputs."""
    w_icfd = params["w_icfd"].astype(np.float32)
    w_off = params["w_off"].astype(np.float32)
    b_off = params["b_off"].astype(np.float32)
    w_fus = params["w_fus"].astype(np.float32)
    b_fus = float(params["b_fus"])
    w_conv = params["w_conv"].astype(np.float32)
    b_conv = params["b_conv"].astype(np.float32)

    W0 = np.zeros((128, 2), np.float32)
    for half in range(2):
        W0[half * 64:(half + 1) * 64, half] = w_icfd

    W2 = np.zeros((18, 128), np.float32)
    for g in range(2):
        for ky2 in range(3):
            for kx2 in range(3):
                W2[g * 9 + ky2 * 3 + kx2, g * 64:(g + 1) * 64] = w_conv[:, 0, ky2, kx2]
    BC = np.zeros((128, 1), np.float32)
    BC[0:64, 0] = b_conv
    BC[64:128, 0] = b_conv

    taps_rev = (R - np.arange(NT)).astype(np.float32)     # [9, 8, ..., -9]
    taps_fwd = (np.arange(NT) - R).astype(np.float32)     # [-9, ..., 9]
    TAPSF = np.tile(taps_fwd[None, :], (128, 1))
    TAPSR = np.tile(taps_rev[None, :], (128, 1))

    # s-chunk layout: s = c*128 + p, 13 chunks; tail (s>=1600) padded with zeros
    WF = np.zeros((128, 13), np.float32)
    WOFF = np.zeros((128, 26), np.float32)    # cols 0..12 y, 13..25 x
    BOFF = np.zeros((128, 26), np.float32)
    for c in range(13):
        for p in range(128):
            s = c * 128 + p
            if s < 1600:
                WF[p, c] = w_fus[s // 64]
                WOFF[p, c] = w_off[2 * s]
                BOFF[p, c] = b_off[2 * s]
                WOFF[p, 13 + c] = w_off[2 * s + 1]
                BOFF[p, 13 + c] = b_off[2 * s + 1]

    C_total = DFC * b_fus
    return dict(
        W0=W0.astype(NPBF), W2=W2.astype(NPBF), BC=BC,
        TAPSF=TAPSF, TAPSR=TAPSR, WF=WF, WOFF=WOFF, BOFF=BOFF,
        I128=np.eye(128, dtype=np.float32).astype(NPBF),
        ONESR=np.ones((1, 256), np.float32),
        ONESC=np.ones((128, 1), np.float32),
        ONES2=np.ones((2, 128), np.float32),
        CVEC=np.full((1, 128), C_total, np.float32),
        b_icfd=float(params["b_icfd"]),
    )


def build(params, num_devices=8):
    cs = _consts(params)
    nc = bacc.Bacc("TRN2", target_bir_lowering=False, debug=False,
                   num_devices=num_devices)
    xb = nc.dram_tensor("xb", [C, H, W], F32, kind="ExternalInput")
    y = nc.dram_tensor("y", [64, H, W], BF16, kind="ExternalOutput")
    K_dram = nc.dram_tensor("k_scr", [280, NT], BF16, kind="Internal")
    inp_dram = nc.dram_tensor("inp_scr", [260, 792], BF16, kind="Internal")

    ct = {k: nc.inline_tensor(v, name=f"c_{k}") for k, v in cs.items()
          if isinstance(v, np.ndarray)}
    b_icfd = cs["b_icfd"]

    def _graph(tc):
        with (
            tc.tile_pool(name="consts", bufs=1) as cp,
            tc.tile_pool(name="persist", bufs=1) as pp,
        ):
            # ---- load constants ----
            sb = {}
            for i, k in enumerate(("W0", "W2", "BC", "TAPSF", "TAPSR", "WF",
                                   "WOFF", "BOFF", "I128", "ONESR", "ONESC",
                                   "ONES2", "CVEC")):
                dt = BF16 if cs[k].dtype == NPBF else F32
                t = cp.tile(list(cs[k].shape), dt, tag=k, name=f"sb_{k}")
                eng = (nc.sync, nc.scalar)[i % 2]
                eng.dma_start(out=t, in_=ct[k][:, :])
                sb[k] = t
            zbf = cp.tile([128, 792], BF16, tag="zbf")
            nc.vector.memset(zbf, 0.0)
            bic2 = cp.tile([2, 1], F32, tag="bic2")
            nc.vector.memset(bic2, b_icfd)

            # ---- zero scratch DRAM (early, off critical path) ----
            nc.sync.dma_start(
                out=bass.AP(tensor=K_dram, offset=0, ap=[[NT, 128], [1, NT]]),
                in_=zbf[:, 0:NT])
            nc.sync.dma_start(
                out=bass.AP(tensor=K_dram, offset=128 * NT,
                            ap=[[NT, 128], [1, NT]]),
                in_=zbf[:, 0:NT])
            nc.sync.dma_start(
                out=bass.AP(tensor=K_dram, offset=256 * NT,
                            ap=[[NT, 24], [1, NT]]),
                in_=zbf[0:24, 0:NT])
            nc.scalar.dma_start(out=inp_dram[0:128, :], in_=zbf[:, 0:792])
            nc.scalar.dma_start(out=inp_dram[128:256, :], in_=zbf[:, 0:792])
            nc.scalar.dma_start(out=inp_dram[256:260, :], in_=zbf[0:4, 0:792])

            # ---- persistent xf tile: [128, 274*2] bf16, halves side by side
            xf_pad = pp.tile([128, 2 * XHW], BF16, tag="xf_pad")
            nc.vector.memset(xf_pad, 0.0)

            def _heat(n, tag):
                # keep-warm matmuls into a scratch PSUM bank: fill PE idle
                # gaps so real matmuls dispatch into a warm p-state streak
                with tc.tile_pool(name=f"heat_{tag}", bufs=1,
                                  space="PSUM") as hp:
                    pdum = hp.tile([2, 512], F32, tag=f"pdum{tag}")
                    for _ in range(n):
                        nc.tensor.matmul(pdum, sb["W0"], zbf[:, 0:512],
                                         start=True, stop=True)

            _heat(10, "a")
            # ---- phase B: x cast-load + stage-0 matmul + evac + scatter ----
            NCH = 8                       # chunks of 16 rows
            with (
                tc.tile_pool(name="bpool", bufs=3) as bp,
                tc.tile_pool(name="psum0", bufs=4, space="PSUM") as p0p,
            ):
                for ch in range(NCH):
                    sbx = bp.tile([128, 4096], BF16, tag="sbx", bufs=8)
                    for half in range(2):   # partition = half*64 + c
                        srcp = bass.AP(tensor=xb,
                                       offset=(half * 128 + ch * 16) * W,
                                       ap=[[HW, 64], [1, 4096]])
                        nc.gpsimd.dma_start(out=sbx[ts(half, 64), :], in_=srcp)
                    s0b = bp.tile([2, 4096], BF16, tag="s0b", bufs=8)
                    for q in range(4):
                        p0 = p0p.tile([2, 1024], F32, tag="p0", name="p0t")
                        for j in range(2):
                            nc.tensor.matmul(
                                p0[:, ts(j, 512)],
                                sb["W0"],
                                sbx[:, ds(q * 1024 + j * 512, 512)],
                                start=True, stop=True)
                        # evac PSUM -> bf16 staging with b_icfd bias fused
                        if (ch * 4 + q) % 2 == 0:
                            nc.scalar.activation(
                                out=s0b[:, ts(q, 1024)], in_=p0,
                                func=mybir.ActivationFunctionType.Identity,
                                bias=bic2[:, 0:1], scale=1.0)
                        else:
                            nc.vector.tensor_scalar_add(
                                out=s0b[:, ts(q, 1024)], in0=p0,
                                scalar1=bic2[:, 0:1])
                    for half in range(2):   # scatter 16 rows into xf_pad
                        nc.sync.dma_start(
                            out=xf_pad[ch * 16:ch * 16 + 16,
                                       ds(half * XHW + R, 256)],
                            in_=s0b[half:half + 1, :])

            # ---- phase C: column sums -> mean ----
            colsums = pp.tile([128, 1], F32, tag="colsums")
            nc.vector.tensor_reduce(out=colsums,
                                    in_=xf_pad[:, 0:2 * XHW],
                                    axis=mybir.AxisListType.X,
                                    op=mybir.AluOpType.add)
            with tc.tile_pool(name="psA", bufs=1, space="PSUM") as psA:
                pm = psA.tile([1, 1], F32, tag="pm")
                nc.tensor.matmul(pm, colsums, sb["ONESC"], start=True, stop=True)
                ts2 = pp.tile([1, 1], F32, tag="ts2")
                nc.scalar.copy(out=ts2, in_=pm)
                pmb = psA.tile([128, 1], F32, tag="pmb")
                nc.tensor.matmul(pmb, sb["ONES2"][0:1, :], ts2,
                                 start=True, stop=True)
                mean_bc = pp.tile([128, 1], F32, tag="mean_bc")
                nc.scalar.activation(out=mean_bc, in_=pmb,
                                     func=mybir.ActivationFunctionType.Copy,
                                     scale=1.0 / HW)

                # ---- phase D: offsets, hats, K matmul ----
                dyx = pp.tile([128, 26], F32, tag="dyx")
                nc.vector.tensor_scalar_mul(out=dyx, in0=sb["WOFF"],
                                            scalar1=mean_bc[:, 0:1])
                nc.vector.tensor_add(out=dyx, in0=dyx, in1=sb["BOFF"])
                HH = pp.tile([128, 26 * NT], F32, tag="HH")
                HH3 = HH[:].rearrange("p (a b) -> p a b", a=26)
                nc.vector.tensor_tensor(
                    out=HH3[:, 0:13, :],
                    in0=dyx[:, 0:13].unsqueeze(2).to_broadcast([128, 13, NT]),
                    in1=sb["TAPSF"][:].unsqueeze(1).to_broadcast([128, 13, NT]),
                    op=mybir.AluOpType.subtract)
                nc.vector.tensor_tensor(
                    out=HH3[:, 13:26, :],
                    in0=dyx[:, 13:26].unsqueeze(2).to_broadcast([128, 13, NT]),
                    in1=sb["TAPSR"][:].unsqueeze(1).to_broadcast([128, 13, NT]),
                    op=mybir.AluOpType.subtract)
                nc.vector.tensor_scalar(out=HH, in0=HH, scalar1=0.0,
                                        scalar2=None,
                                        op0=mybir.AluOpType.abs_max)
                nc.scalar.activation(out=HH, in_=HH,
                                     func=mybir.ActivationFunctionType.Relu,
                                     scale=-1.0, bias=1.0)
                WHY = pp.tile([128, 13 * NT], F32, tag="WHY")
                nc.vector.tensor_tensor(
                    out=WHY[:].rearrange("p (a b) -> p a b", a=13),
                    in0=HH3[:, 0:13, :],
                    in1=sb["WF"][:].unsqueeze(2).to_broadcast([128, 13, NT]),
                    op=mybir.AluOpType.mult)
                WHY3 = WHY[:].rearrange("p (a b) -> p a b", a=13)
                pK = psA.tile([NT, NT], F32, tag="pK")
                for c in range(13):
                    nc.tensor.matmul(pK, WHY3[:, c, :], HH3[:, 13 + c, :],
                                     start=(c == 0), stop=(c == 12))
                Ksb = pp.tile([NT, NT], BF16, tag="Ksb")
                nc.scalar.copy(out=Ksb, in_=pK)

            # ---- phase E: reversed K write + contiguous staircase T tables
            # K_dram[C0 - d] = Ksb[d]  (bf16 cast during SWDGE write)
            nc.sync.dma_start(
                out=bass.AP(tensor=K_dram, offset=C0 * NT,
                            ap=[[-NT, NT], [1, NT]]),
                in_=Ksb)
            T_A = pp.tile([128, 128 * NT], BF16, tag="T_A")
            T_B = pp.tile([9, 128 * NT], BF16, tag="T_B")
            T_C = pp.tile([9, 128 * NT], BF16, tag="T_C")
            # T_A[p, a, e] = K[9+p-a] = K_dram[C0-9-p+a]
            nc.sync.dma_start(
                out=T_A[:].rearrange("p (a b) -> p a b", a=128),
                in_=bass.AP(tensor=K_dram, offset=(C0 - 9) * NT,
                            ap=[[-NT, 128], [NT, 128], [1, NT]]))
            # T_B[p, a, e] = K[p-a] = K_dram[C0-p+a]
            nc.gpsimd.dma_start(
                out=T_B[:].rearrange("p (a b) -> p a b", a=128),
                in_=bass.AP(tensor=K_dram, offset=C0 * NT,
                            ap=[[-NT, 9], [NT, 128], [1, NT]]))
            # T_C[p, a, e] = K[137+p-a] = K_dram[C0-137-p+a]
            nc.gpsimd.dma_start(
                out=T_C[:].rearrange("p (a b) -> p a b", a=128),
                in_=bass.AP(tensor=K_dram, offset=(C0 - 137) * NT,
                            ap=[[-NT, 9], [NT, 128], [1, NT]]))
            T_A3 = T_A[:].rearrange("p (a b) -> p a b", a=128)
            T_B3 = T_B[:].rearrange("p (a b) -> p a b", a=128)
            T_C3 = T_C[:].rearrange("p (a b) -> p a b", a=128)
            # matmul operands must start at partition 0/32/64: copy the 9
            # boundary rows of half 0 (119..127) into a base-0 tile
            xf_b0 = pp.tile([9, XHW], BF16, tag="xf_b0")
            nc.sync.dma_start(out=xf_b0, in_=xf_pad[119:128, 0:XHW])

            _heat(26, "b")
            # ---- phase F: stage-1 Toeplitz matmuls -> inp_dram ----
            with tc.tile_pool(name="psum1", bufs=2, space="PSUM") as p1p:
                for t in range(2):
                    pinp = p1p.tile([128, W], F32, tag="pinp")
                    nmm = NT * 2 + 2
                    i = 0
                    for kxp in range(NT):
                        sl = 18 - kxp
                        nc.tensor.matmul(pinp, T_A3[:, :, kxp],
                                         xf_pad[:, ds(t * XHW + sl, W)],
                                         start=(i == 0), stop=(i == nmm - 1)); i += 1
                        if t == 0:
                            nc.tensor.matmul(pinp, T_C3[0:9, :, kxp],
                                             xf_pad[0:9, ds(XHW + sl, W)],
                                             start=False, stop=(i == nmm - 1)); i += 1
                        else:
                            nc.tensor.matmul(pinp, T_B3[0:9, :, kxp],
                                             xf_b0[:, ds(sl, W)],
                                             start=False, stop=(i == nmm - 1)); i += 1
                    nc.tensor.matmul(pinp, sb["I128"],
                                     xf_pad[:, ds(t * XHW + R, W)],
                                     start=False, stop=False); i += 1
                    nc.tensor.matmul(pinp, sb["CVEC"], sb["ONESR"][0:1, 0:W],
                                     start=False, stop=True); i += 1
                    s1 = pp.tile([128, W], BF16, tag=f"s1_{t}", name=f"s1stage{t}")
                    nc.vector.tensor_copy(out=s1, in_=pinp)
                    # inp3[r, j, e] = inp_padded[r, j + e]; s1 covers padded
                    # cols 1..256 of rows 1+128t..128+128t.  All 3 shifted
                    # copies in one DMA: j-stride 264-1=263 absorbs the shift.
                    dst = bass.AP(
                        tensor=inp_dram,
                        offset=(1 + 128 * t) * 792 + 1,
                        ap=[[792, 128], [263, 3], [1, W]])
                    eng = (nc.sync, nc.gpsimd)[t]
                    eng.dma_start(out=dst, in_=s1[:].unsqueeze(1).to_broadcast(
                        [128, 3, W]))

            _heat(16, "c")
            # ---- phase G: im2col + stage-2 + store ----
            with (
                tc.tile_pool(name="gpool", bufs=3) as gp,
                tc.tile_pool(name="psum2", bufs=4, space="PSUM") as p2p,
            ):
                for ch in range(8):               # h2-chunks of 16
                    im = gp.tile([18, 4096], BF16, tag="im", bufs=8)
                    for g in range(2):
                        srcp = bass.AP(
                            tensor=inp_dram,
                            offset=(g * 128 + ch * 16) * 792,
                            ap=[[264, 9], [792, 16], [1, W]])
                        nc.sync.dma_start(
                            out=im[ts(g, 9), :].rearrange(
                                "p (d e) -> p d e", d=16),
                            in_=srcp)
                    for pair in range(2):         # 8 rows per store
                        ysb = gp.tile([128, 2048], BF16, tag="ysb",
                                      name="ystage", bufs=8)
                        py = p2p.tile([128, 2048], F32, tag="py", bufs=2)
                        for j in range(4):
                            nc.tensor.matmul(
                                py[:, ts(j, 512)], sb["W2"],
                                im[:, ds(pair * 2048 + j * 512, 512)],
                                start=True, stop=True)
                        # single evac per store unit (+ b_conv bias, bf16)
                        u = ch * 2 + pair
                        if (u + u // 2) % 2 == 0:
                            nc.scalar.activation(
                                out=ysb, in_=py,
                                func=mybir.ActivationFunctionType.Identity,
                                bias=sb["BC"][:, 0:1], scale=1.0)
                        else:
                            nc.vector.tensor_scalar_add(
                                out=ysb, in0=py, scalar1=sb["BC"][:, 0:1])
                        dst = bass.AP(
                            tensor=y,
                            offset=(ch * 16 + pair * 8) * W,
                            ap=[[128 * W, 2], [HW, 64], [1, 2048]])
                        nc.gpsimd.dma_start(out=dst, in_=ysb[:])
    with tile.TileContext(nc) as tc:
        _graph(tc)
    nc.finalize()
    return nc


def kernel(**inputs):
    x = np.ascontiguousarray(inputs["x"], dtype=np.float32)
    params = {k: np.asarray(v) for k, v in inputs.items() if k != "x"}
    nc = build(params, num_devices=8)
    from concourse.bass_utils import run_bass_kernel_spmd
    in_maps = [{"xb": np.ascontiguousarray(x[b])} for b in range(B)]
    res = run_bass_kernel_spmd(nc, in_maps, core_ids=list(range(B)))
    return np.stack([np.asarray(res.results[b]["y"], dtype=np.float32)
                     for b in range(B)])
